# revision 18
# baseline (speedup 1.0000x reference)
"""ODE-GRU encoder Trainium2 Bass kernel.

Model (per reference): B=512, S=128, D=128, H=1024, L=128.
  h = GRUCell(x_0, 0)
  for i in 1..S-1:  4x dopri5 substeps on h' = MLP(h), then h = GRUCell(x_i, h)
  mu = h @ mu_w.T + mu_b ; logvar = h @ lv_w.T + lv_b

Key structural facts exploited:
  * DP_B == DP_A[6] (FSAL): the dopri5 solution point y_6 IS h_new, and the
    7th stage evaluation k_6 is dead code in the reference. So each substep
    needs only 6 MLP evals + the y_6 linear combination.
  * Pure data parallelism: batch 512 -> 8 cores x 64. No collectives.

Device layout (per core, "transposed chunked" form):
  A length-1024 vector per batch element lives as an SBUF tile [128, 8*64]:
  column block c (64 wide) = hidden chunk c, partition p = hidden c*128+p,
  column-within-block j = batch element j.
  Matmul out[m-chunk] = sum_k W.T[k,m].T @ act[k] : lhsT = weight tile
  [128(k), 128(m)] (bf16, resident in SBUF), rhs = activation chunk [128, 64]
  (bf16), PSUM out [128(m), 64] fp32, 8-chunk accumulation per output chunk.
  This chains layers with zero transposes.

Precision: weights bf16, matmul inputs bf16, PSUM accum fp32, all state
(h, k_j, y) fp32 on DVE, tanh/sigmoid on ACT (fp32 in, bf16 out mid-MLP).
Measured end-to-end error vs fp64 in simulation: ~3.4e-3 absmax relative.

dopri5 coefficients (hs * a_sj) are data-dependent (from t): they are loaded
per interval from a small DRAM table as per-partition scalars, so any t works.
"""
import sys
import os
from contextlib import ExitStack

sys.path.insert(0, "/opt/trn_rl_repo")

import numpy as np
import ml_dtypes

B, S, D, H, L = 512, 128, 128, 1024, 128
N_SUB = 4
N_CORES = 8
BL = B // N_CORES  # 64 batch per core
C = H // 128       # 8 hidden chunks

DP_A = (
    (),
    (1/5,),
    (3/40, 9/40),
    (44/45, -56/15, 32/9),
    (19372/6561, -25360/2187, 64448/6561, -212/729),
    (9017/3168, -355/33, 46732/5247, 49/176, -5103/18656),
    (35/384, 0.0, 500/1113, 125/192, -2187/6784, 11/84),
)

NZ_ROWS = [[j for j, a in enumerate(row) if a != 0.0] for row in DP_A]
N_COEF = sum(len(nz) for nz in NZ_ROWS[1:])  # 20
COEF_COLS = 32  # padded

bf16 = ml_dtypes.bfloat16
fp8 = ml_dtypes.float8_e4m3fn  # TRN FP8_EXP4: bit-compatible within +-240

# MLP weights in fp8e4m3 (moving operands stay bf16): halves the LDWEIGHTS
# SBUF read traffic feeding the power governor; numerically safe (measured:
# final rel err unchanged at ~3.5e-3 — the hs=dt/4 factor damps MLP error).
# GRU weights must stay bf16 (fp8 there measured 2.9e-2 > budget).
MLP_W_FP8 = True


def _split_multiwaits(bir_bytes):
    """Rewrite sync_info patterns the TPB 64B encoding can't hold:

    1. >1 sem waits on one instruction (e.g. the Tile For_i back-edge Drain)
       -> all but the last wait move to prepended single-wait NoOps.
    2. a wait together with a `sem-add-imm` update (staggered-reset prebumps
       aggregate bumps into big adds; wait_value and update_value share the
       one `semaphore_value` field) -> all waits move to prepended NoOps.

    Hoisting a wait to a preceding NoOp on the same engine is semantics-
    preserving (engine streams are FIFO). DMA opcodes are left alone.
    """
    import orjson
    j = orjson.loads(bir_bytes)
    ctr = 0
    for fn in j["functions"]:
        for blk in fn["blocks"]:
            out = []
            for ins in blk["instructions"]:
                si = ins.get("sync_info")
                waits = (si or {}).get("on_wait") or []
                updates = (si or {}).get("on_update") or []
                is_dma = ins.get("opcode", "").startswith("DMA")
                clash = (waits and not is_dma and any(
                    u.get("update_mode") == "sem-add-imm" and
                    u.get("update_value", 0) > 1 for u in updates))
                hoist = waits if clash else (
                    waits[:-1] if len(waits) > 1 else [])
                if hoist:
                    for w in hoist:
                        ctr += 1
                        nop = {
                            "engine": ins["engine"],
                            "ins": [],
                            "outs": [],
                            "name": f"waitsplit-{ctr}",
                            "opcode": "NoOp",
                            "sync_info": {"on_update": [], "on_wait": [w]},
                        }
                        if "debug" in ins:
                            nop["debug"] = ins["debug"]
                        out.append(nop)
                    si["on_wait"] = waits[len(hoist):]
                out.append(ins)
            blk["instructions"] = out
    return orjson.dumps(j)


def _patch_to_json(nc):
    from concourse import mybir
    nc.to_json_bytes = lambda: _split_multiwaits(
        mybir.module_to_json_bytes(nc.m))


def _build_program(n_intervals, zero_bias, uniform_dt=False):
    import concourse.bass as bass
    import concourse.tile as tile
    from concourse import mybir

    f32 = mybir.dt.float32
    bf = mybir.dt.bfloat16
    wdt = mybir.dt.float8e4 if MLP_W_FP8 else bf
    Tanh = mybir.ActivationFunctionType.Tanh
    Sigmoid = mybir.ActivationFunctionType.Sigmoid
    Ident = mybir.ActivationFunctionType.Identity
    AO = mybir.AluOpType

    NI = n_intervals

    nc = bass.Bass(trn_type="TRN2", target_bir_lowering=False, debug=False)

    w0t_d = nc.dram_tensor("w0t", [128, 64 * 128], wdt, kind="ExternalInput")
    w1t_d = nc.dram_tensor("w1t", [128, 64 * 128], wdt, kind="ExternalInput")
    w2t_d = nc.dram_tensor("w2t", [128, 64 * 128], wdt, kind="ExternalInput")
    whht_d = nc.dram_tensor("whht", [128, 192 * 128], bf, kind="ExternalInput")
    wiht_d = nc.dram_tensor("wiht", [128, 24 * 128], bf, kind="ExternalInput")
    muwt_d = nc.dram_tensor("muwt", [128, 8 * 128], bf, kind="ExternalInput")
    lvwt_d = nc.dram_tensor("lvwt", [128, 8 * 128], bf, kind="ExternalInput")
    # x resident in SBUF for the whole run: [p=din, (step, batch)] layout,
    # loaded once — no per-interval DMA, and the GRU input-side matmuls can
    # run at the top of the loop body to fill the h-carry dependency stall.
    xT_d = nc.dram_tensor("xT", [128, (NI + 1) * BL], bf,
                          kind="ExternalInput")
    # uniform dt (the harness case: t = arange*0.1): one static coef tile;
    # otherwise a per-interval table DMA'd inside the loop.
    coefs_d = nc.dram_tensor(
        "coefs", [128, COEF_COLS] if uniform_dt
        else [max(NI, 1) * 128, COEF_COLS], f32, kind="ExternalInput")
    # bias pack (fp32): cols 0..7 b0, 8..15 b1, 16..23 b2, 24..47 bih (r,z,n),
    # 48..71 bhh (r,z,n), 72 mu_b, 73 lv_b   (chunked per partition)
    bias_d = nc.dram_tensor("biases", [128, 74], f32, kind="ExternalInput")
    mu_out_d = nc.dram_tensor("mu_out", [128, BL], f32, kind="ExternalOutput")
    lv_out_d = nc.dram_tensor("lv_out", [128, BL], f32, kind="ExternalOutput")

    with ExitStack() as ctx:
        tc = ctx.enter_context(tile.TileContext(nc))
        wpool = ctx.enter_context(tc.tile_pool(name="weights", bufs=1))
        state = ctx.enter_context(tc.tile_pool(name="state", bufs=1))
        dyn = ctx.enter_context(tc.tile_pool(name="dyn", bufs=2))
        mid = ctx.enter_context(tc.tile_pool(name="mid", bufs=3))
        ypool = ctx.enter_context(tc.tile_pool(name="ypool", bufs=2))
        gpool = ctx.enter_context(tc.tile_pool(name="gru", bufs=2))
        pmlp = ctx.enter_context(tc.tile_pool(name="pmlp", bufs=4, space="PSUM"))
        pgru = ctx.enter_context(tc.tile_pool(name="pgru", bufs=1, space="PSUM"))

        w0 = wpool.tile([128, 64 * 128], wdt, tag="w0")
        w1 = wpool.tile([128, 64 * 128], wdt, tag="w1")
        w2 = wpool.tile([128, 64 * 128], wdt, tag="w2")
        whh = wpool.tile([128, 192 * 128], bf, tag="whh")
        wih = wpool.tile([128, 24 * 128], bf, tag="wih")
        muw = wpool.tile([128, 8 * 128], bf, tag="muw")
        lvw = wpool.tile([128, 8 * 128], bf, tag="lvw")
        biases = wpool.tile([128, 74], f32, tag="biases")
        xsb = wpool.tile([128, (NI + 1) * BL], bf, tag="xsb")
        loads = [(w0, w0t_d), (w1, w1t_d), (w2, w2t_d), (whh, whht_d),
                 (wih, wiht_d), (muw, muwt_d), (lvw, lvwt_d),
                 (biases, bias_d), (xsb, xT_d)]
        if uniform_dt:
            coefs_sb = wpool.tile([128, COEF_COLS], f32, tag="coefs_sb")
            loads.append((coefs_sb, coefs_d))
        for sb, dr in loads:
            nc.sync.dma_start(sb[:, :], dr[:, :])

        h = state.tile([128, C * BL], f32, tag="h")
        h_bf = state.tile([128, C * BL], bf, tag="h_bf")
        dummy_in = state.tile([128, 1], f32, tag="dummy_in")
        dummy_out = state.tile([128, 1], bf, tag="dummy_out")
        nc.vector.memset(dummy_in[:, :], 0.0)
        n_arch = 5 if zero_bias else 6
        karch = [state.tile([128, C * BL], f32, tag=f"k{j}", name=f"karch{j}")
                 for j in range(n_arch)]

        nc.vector.memset(h[:, :], 0.0)
        nc.vector.memset(h_bf[:, :], 0.0)

        def bias_col(idx):
            return biases[:, idx:idx + 1]

        HB = C * BL // 2  # half-tile width (256)

        def mm_layer_halves(wt, rhs_bf, psA, psB, nm=C):
            # MLP layer into two half-bank psum tiles: m-chunks 0..3 -> psA,
            # 4..7 -> psB (different banks: ACT consumes psA while PE writes
            # psB). k-OUTER order: the first 8 matmuls consume only rhs chunk
            # 0 (64 cols), so the PE unblocks as soon as the producer's first
            # chunk-grain op lands (producers emit y/h chunks in ascending
            # order). PSUM accumulation group is per BANK: start=True only on
            # the very first matmul into the bank (clears has_written for the
            # whole bank), stop=True on the last; per-element has_written
            # gives first-write-overwrite / then-accumulate for every m
            # region independently.
            for k in range(C):
                for m in range(nm):
                    ps, mo = (psA, m) if m < 4 else (psB, m - 4)
                    t = (k * nm + m) * 128
                    nc.tensor.matmul(
                        ps[:, BL * mo: BL * mo + BL],
                        wt[:, t: t + 128],
                        rhs_bf[:, BL * k: BL * k + BL],
                        start=(k == 0 and mo == 0),
                        stop=(k == C - 1 and mo == 3),
                        skip_group_check=True,
                    )

        def act_halves(out, psA, psB, func, bias_base):
            # out[:, :HB] = func(psA + b), out[:, HB:] = func(psB + b)
            if zero_bias:
                nc.scalar.activation(out[:, 0:HB], psA[:, :], func)
                nc.scalar.activation(out[:, HB:2 * HB], psB[:, :], func)
            else:
                for cc in range(C):
                    ps, co = (psA, cc) if cc < 4 else (psB, cc - 4)
                    nc.scalar.activation(
                        out[:, BL * cc: BL * cc + BL],
                        ps[:, BL * co: BL * co + BL],
                        func, bias=bias_col(bias_base + cc))

        def eval_mlp(rhs_bf):
            ps0a = pmlp.tile([128, HB], f32, tag="ps")
            ps0b = pmlp.tile([128, HB], f32, tag="ps")
            mm_layer_halves(w0, rhs_bf, ps0a, ps0b)
            u = mid.tile([128, C * BL], bf, tag="u")
            act_halves(u, ps0a, ps0b, Tanh, 0)
            ps1a = pmlp.tile([128, HB], f32, tag="ps")
            ps1b = pmlp.tile([128, HB], f32, tag="ps")
            mm_layer_halves(w1, u, ps1a, ps1b)
            v = mid.tile([128, C * BL], bf, tag="v")
            act_halves(v, ps1a, ps1b, Tanh, 8)
            ps2a = pmlp.tile([128, HB], f32, tag="ps")
            ps2b = pmlp.tile([128, HB], f32, tag="ps")
            mm_layer_halves(w2, v, ps2a, ps2b)
            return ps2a, ps2b

        def archive_k(j, ks_psum):
            # karch[j] = ks_psum + b2
            psA, psB = ks_psum
            if zero_bias:
                nc.scalar.copy(karch[j][:, 0:HB], psA[:, :])
                nc.scalar.copy(karch[j][:, HB:2 * HB], psB[:, :])
            else:
                for cc in range(C):
                    ps, co = (psA, cc) if cc < 4 else (psB, cc - 4)
                    nc.scalar.activation(
                        karch[j][:, BL * cc: BL * cc + BL],
                        ps[:, BL * co: BL * co + BL],
                        Ident, bias=bias_col(16 + cc))

        def stt(out, in0, cap, in1, chunked=False):
            # out = in0 * coef + in1; in0 may be a (psA, psB) half pair.
            # chunked: emit 64-col ops in ascending chunk order so the
            # consumer's k-outer matmuls unblock after the first small op.
            if isinstance(in0, tuple):
                psA, psB = in0
                if chunked:
                    for cc in range(C):
                        ps, co = (psA, cc) if cc < 4 else (psB, cc - 4)
                        nc.vector.scalar_tensor_tensor(
                            out[:, BL * cc: BL * cc + BL],
                            ps[:, BL * co: BL * co + BL], cap,
                            in1[:, BL * cc: BL * cc + BL],
                            AO.mult, AO.add)
                else:
                    nc.vector.scalar_tensor_tensor(
                        out[:, 0:HB], psA[:, :], cap, in1[:, 0:HB],
                        AO.mult, AO.add)
                    nc.vector.scalar_tensor_tensor(
                        out[:, HB:2 * HB], psB[:, :], cap, in1[:, HB:2 * HB],
                        AO.mult, AO.add)
            else:
                nc.vector.scalar_tensor_tensor(
                    out[:, :], in0[:, :], cap, in1[:, :], AO.mult, AO.add)

        def substep(coef_tile):
            # h, h_bf updated in place; coefficients at fixed cols 0..19
            cnt = 0

            def next_coef():
                nonlocal cnt
                ap = coef_tile[:, cnt:cnt + 1]
                cnt += 1
                return ap

            ks_psum = eval_mlp(h_bf)  # k_0
            for s in range(1, 7):
                nz = NZ_ROWS[s]
                if s - 1 < n_arch:
                    archive_k(s - 1, ks_psum)
                y_acc = None
                for idx, j in enumerate(nz):
                    cap = next_coef()
                    last = (idx == len(nz) - 1)
                    final_stage = (s == 6)
                    # last term's k comes straight from PSUM in the zero-bias
                    # fast path (j == s-1 always holds for the last term)
                    use_psum = last and zero_bias
                    src = ks_psum if use_psum else karch[j]
                    base = h if y_acc is None else y_acc
                    if last:
                        if final_stage:
                            # y_6 == h_new; emit the bf16 copy FIRST, chunk-
                            # grain, so the next substep's (or GRU's) k-outer
                            # matmuls unblock after one 64-col DVE op
                            stt(h_bf, src, cap, base, chunked=True)
                            stt(h, src, cap, base)
                        else:
                            y_bf = mid.tile([128, C * BL], bf, tag="ybf")
                            stt(y_bf, src, cap, base, chunked=True)
                    else:
                        if y_acc is None:
                            y_acc = ypool.tile([128, C * BL], f32, tag="yacc")
                        stt(y_acc, src, cap, base)
                if s < 6:
                    ks_psum = eval_mlp(y_bf)

        def gru_gi(xt_ap):
            """GRU input-side matmuls (need only x_t): emitted at the TOP of
            the loop body so the PE fills the h_bf-wait stall at the interval
            boundary. Returns the psum tiles; gru_hh accumulates into pr/pz
            later (start=False) and closes the groups."""
            pr = pgru.tile([128, C * BL], f32, tag="pr")
            pz = pgru.tile([128, C * BL], f32, tag="pz")
            pgn = pgru.tile([128, C * BL], f32, tag="pgn")
            pin_ = pgru.tile([128, C * BL], f32, tag="pin")
            for sec, ps in ((0, pr), (1, pz)):
                for m in range(C):
                    mj = sec * 8 + m
                    nc.tensor.matmul(
                        ps[:, BL * m: BL * m + BL],
                        wih[:, mj * 128: mj * 128 + 128],
                        xt_ap,
                        start=(m == 0), stop=False,
                        skip_group_check=True)
            for m in range(C):
                mj = 16 + m
                nc.tensor.matmul(
                    pin_[:, BL * m: BL * m + BL],
                    wih[:, mj * 128: mj * 128 + 128],
                    xt_ap,
                    start=(m == 0), stop=(m == C - 1),
                    skip_group_check=True)
            return pr, pz, pgn, pin_

        def gru_hh(gates):
            pr, pz, pgn, pin_ = gates
            # r and z gates: gi (above) + gh (8k) accumulated per bank; the
            # n gate keeps gh_n and gi_n apart (r gates only the h part).
            # k-OUTER: the first matmuls consume only h_bf chunk 0, which the
            # final-substep stt emits first (chunk-grain).
            for k in range(C):
                for sec, ps in ((0, pr), (1, pz), (2, pgn)):
                    for m in range(C):
                        mj = sec * 8 + m
                        t = (k * 24 + mj) * 128
                        nc.tensor.matmul(
                            ps[:, BL * m: BL * m + BL],
                            whh[:, t: t + 128],
                            h_bf[:, BL * k: BL * k + BL],
                            start=(sec == 2 and k == 0 and m == 0),
                            stop=(k == C - 1 and m == C - 1),
                            skip_group_check=True)

            r = gpool.tile([128, C * BL], f32, tag="r")
            z = gpool.tile([128, C * BL], f32, tag="z")
            n = gpool.tile([128, C * BL], f32, tag="n")
            t1 = gpool.tile([128, C * BL], f32, tag="t1")
            pre = gpool.tile([128, C * BL], f32, tag="pre")
            d = gpool.tile([128, C * BL], f32, tag="d")
            e = gpool.tile([128, C * BL], f32, tag="e")
            if zero_bias:
                # per-half chain for the deep part (r->t1->pre), then chunk
                # grain for n/d/e/h_bf so the next eval's k-outer matmuls
                # start after one small op per chunk
                for hb in range(2):
                    sl = slice(hb * HB, (hb + 1) * HB)
                    nc.scalar.activation(r[:, sl], pr[:, sl], Sigmoid)
                    nc.scalar.activation(z[:, sl], pz[:, sl], Sigmoid)
                    nc.vector.tensor_mul(t1[:, sl], r[:, sl], pgn[:, sl])
                    nc.vector.tensor_add(pre[:, sl], t1[:, sl], pin_[:, sl])
                    for cc in range(4 * hb, 4 * hb + 4):
                        cs = slice(BL * cc, BL * cc + BL)
                        nc.scalar.activation(n[:, cs], pre[:, cs], Tanh)
                        nc.vector.tensor_sub(d[:, cs], h[:, cs], n[:, cs])
                        nc.vector.tensor_mul(e[:, cs], z[:, cs], d[:, cs])
                        nc.vector.tensor_add(h_bf[:, cs], n[:, cs], e[:, cs])
                    nc.vector.tensor_add(h[:, sl], n[:, sl], e[:, sl])
            else:
                for cc in range(C):
                    sl = slice(BL * cc, BL * cc + BL)
                    # bias for r gate = bih_r + bhh_r (host folds the sum into
                    # col 24.. for ih and 48.. for hh; here use both adds)
                    nc.scalar.activation(r[:, sl], pr[:, sl], Sigmoid,
                                         bias=bias_col(24 + cc))
                    nc.scalar.activation(z[:, sl], pz[:, sl], Sigmoid,
                                         bias=bias_col(24 + 8 + cc))
                    # t1 = (pgn + bhh_n) * r
                    nc.vector.scalar_tensor_tensor(
                        t1[:, sl], pgn[:, sl], bias_col(48 + 16 + cc),
                        r[:, sl], AO.add, AO.mult)
                    # pre = (pin + bih_n) + t1
                    nc.vector.scalar_tensor_tensor(
                        pre[:, sl], pin_[:, sl], bias_col(24 + 16 + cc),
                        t1[:, sl], AO.add, AO.add)
                nc.scalar.activation(n[:, :], pre[:, :], Tanh)
                nc.vector.tensor_sub(d[:, :], h[:, :], n[:, :])
                nc.vector.tensor_mul(e[:, :], z[:, :], d[:, :])
                nc.vector.tensor_add(h_bf[:, :], n[:, :], e[:, :])
                nc.vector.tensor_add(h[:, :], n[:, :], e[:, :])

        # ---- prologue: h = GRU(x_0, 0) -------------------------------------
        gru_hh(gru_gi(xsb[:, 0:BL]))

        # ---- main loop over observation intervals --------------------------
        # staggered_reset: no all-engine barrier at the back edge, so the PE
        # can start iteration j+1's stage-0 matmuls while DVE/ACT finish
        # iteration j's GRU tail. Stages = substeps (stage 3 includes GRU).
        if NI > 0:
            with tc.For_i(0, NI, staggered_reset=True,
                          back_edge_label="mainloop",
                          hint_engines=(mybir.EngineType.PE,)) as j:
                # dummy activation: absorbs the per-block ACT_TABLE_LOAD off
                # the critical path (first real tanh would otherwise stall)
                nc.scalar.activation(dummy_out[:, :], dummy_in[:, :], Tanh)
                # GRU input-side matmuls for THIS interval's GRU run first:
                # they depend only on x (SBUF-resident), so they execute
                # while the previous interval's GRU tail still computes h.
                # The register-offset slice goes through an (idle) GPSIMD
                # copy — matmul operands can't take register offsets.
                xt = dyn.tile([128, BL], bf, tag="xt")
                nc.gpsimd.tensor_copy(
                    xt[:, :], xsb[:, bass.ds((j + 1) * BL, BL)])
                gates = gru_gi(xt[:, :])
                if uniform_dt:
                    ct = coefs_sb
                else:
                    ct = dyn.tile([128, COEF_COLS], f32, tag="ct")
                    nc.sync.dma_start(
                        ct[:, :], coefs_d[bass.ds(j * 128, 128), :])
                for si in range(N_SUB):
                    if si > 0:
                        tc.stage_boundary()
                    substep(ct)
                    if si == N_SUB - 1:
                        # arm the PE back-edge branch prefetch while the GRU
                        # matmuls run (body >> one IRAM block)
                        tc.mark_branch_hint_location(
                            "mainloop", engines=(mybir.EngineType.PE,))
                gru_hh(gates)

        # ---- epilogue: mu / logvar ----------------------------------------
        for wt, bcol, out_d in ((muw, 72, mu_out_d), (lvw, 73, lv_out_d)):
            po = pgru.tile([128, BL], f32, tag="pr")
            for k in range(C):
                nc.tensor.matmul(
                    po[:, :], wt[:, k * 128: k * 128 + 128],
                    h_bf[:, BL * k: BL * k + BL],
                    start=(k == 0), stop=(k == C - 1))
            osb = gpool.tile([128, BL], f32, tag="osb")
            if zero_bias:
                nc.scalar.copy(osb[:, :], po[:, :])
            else:
                nc.scalar.activation(osb[:, :], po[:, :], Ident,
                                     bias=bias_col(bcol))
            nc.sync.dma_start(out_d[:, :], osb[:, :])

    return nc


def _chunk_wT(w, dt=bf16):
    """[O, I] weight -> [128, (I/128)*(O/128)*128] tile pack.

    Tile (k, m) at col offset (k*nm + m)*128 holds W[m*128+f, k*128+p] at
    [p, f] (i.e. lhsT = W.T block), so matmul computes W @ act.
    """
    O, I = w.shape
    nk, nm = I // 128, O // 128
    a = np.ascontiguousarray(w.T)          # [I, O]
    a = a.reshape(nk, 128, nm, 128)        # k, p, m, f
    a = np.transpose(a, (1, 0, 2, 3))      # p, k, m, f
    return np.ascontiguousarray(a.reshape(128, nk * nm * 128)).astype(dt)


def _chunk_vec(v):
    """[H] -> [128, C] chunked per-partition layout (col c = chunk c)."""
    return np.ascontiguousarray(v.reshape(-1, 128).T).astype(np.float32)


def host_prep(inputs):
    """Build the per-core in_maps + metadata from the full inputs."""
    x = np.asarray(inputs["x"], np.float32)
    t = np.asarray(inputs["t"], np.float32)

    n_intervals = S - 1
    dts = (t[0, 1:, 0] - t[0, :-1, 0]).astype(np.float32)
    hs = (dts / np.float32(N_SUB)).astype(np.float32)

    coefs = np.zeros((n_intervals, COEF_COLS), np.float32)
    for ji in range(n_intervals):
        cols = []
        for srow in range(1, 7):
            for j in NZ_ROWS[srow]:
                cols.append(np.float32(hs[ji]) * np.float32(DP_A[srow][j]))
        coefs[ji, :len(cols)] = cols
    # uniform observation spacing (the setup_inputs case: t = arange*0.1):
    # every interval shares one coef vector -> keep it static in SBUF
    uniform_dt = bool(np.all(dts == dts[0]))
    if uniform_dt:
        coefs_full = np.repeat(coefs[0:1, :], 128, axis=0)  # [128, COEF_COLS]
    else:
        coefs_full = np.repeat(coefs[:, None, :], 128, axis=1).reshape(
            n_intervals * 128, COEF_COLS)

    bias_names = ("gru_b_ih", "gru_b_hh", "b0", "b1", "b2", "mu_b", "lv_b")
    zero_bias = all(not np.any(np.asarray(inputs[k])) for k in bias_names)

    biases = np.zeros((128, 74), np.float32)
    biases[:, 0:8] = _chunk_vec(np.asarray(inputs["b0"], np.float32))
    biases[:, 8:16] = _chunk_vec(np.asarray(inputs["b1"], np.float32))
    biases[:, 16:24] = _chunk_vec(np.asarray(inputs["b2"], np.float32))
    bih = _chunk_vec(np.asarray(inputs["gru_b_ih"], np.float32))
    bhh = _chunk_vec(np.asarray(inputs["gru_b_hh"], np.float32))
    # r/z gates consume bih+bhh as one folded bias (cols 24..39); the n gate
    # needs them apart: n(ih) at 40..47, n(hh) at 64..71 (within bhh 48..71)
    biases[:, 24:40] = (bih + bhh)[:, 0:16]
    biases[:, 40:48] = bih[:, 16:24]
    biases[:, 48:72] = bhh
    biases[:, 72] = np.asarray(inputs["mu_b"], np.float32)
    biases[:, 73] = np.asarray(inputs["lv_b"], np.float32)

    mwdt = fp8 if MLP_W_FP8 else bf16
    shared = {
        "w0t": _chunk_wT(np.asarray(inputs["w0"], np.float32), mwdt),
        "w1t": _chunk_wT(np.asarray(inputs["w1"], np.float32), mwdt),
        "w2t": _chunk_wT(np.asarray(inputs["w2"], np.float32), mwdt),
        "whht": _chunk_wT(np.asarray(inputs["gru_w_hh"], np.float32)),
        "wiht": _chunk_wT(np.asarray(inputs["gru_w_ih"], np.float32)),
        "muwt": _chunk_wT(np.asarray(inputs["mu_w"], np.float32)),
        "lvwt": _chunk_wT(np.asarray(inputs["lv_w"], np.float32)),
        "coefs": coefs_full,
        "biases": biases,
    }

    in_maps = []
    for cidx in range(N_CORES):
        xc = x[cidx * BL:(cidx + 1) * BL]               # [BL, S, D]
        xT = np.ascontiguousarray(np.transpose(xc, (2, 1, 0)))  # [D, S, BL]
        m = dict(shared)
        m["xT"] = xT.reshape(128, S * BL).astype(bf16)
        in_maps.append(m)
    return in_maps, zero_bias, uniform_dt


def kernel(**inputs):
    from concourse import bass_utils

    in_maps, zero_bias, uniform_dt = host_prep(inputs)
    nc = _build_program(S - 1, zero_bias, uniform_dt)
    _patch_to_json(nc)
    res = bass_utils.run_bass_kernel_spmd(
        nc, in_maps, core_ids=list(range(N_CORES)))
    mu = np.empty((B, L), np.float32)
    lv = np.empty((B, L), np.float32)
    for cidx in range(N_CORES):
        mu[cidx * BL:(cidx + 1) * BL] = np.asarray(
            res.results[cidx]["mu_out"], np.float32).T
        lv[cidx * BL:(cidx + 1) * BL] = np.asarray(
            res.results[cidx]["lv_out"], np.float32).T
    return mu, lv



# revision 25
# speedup vs baseline: 1.0860x; 1.0860x over previous
"""ODE-GRU encoder Trainium2 Bass kernel.

Model (per reference): B=512, S=128, D=128, H=1024, L=128.
  h = GRUCell(x_0, 0)
  for i in 1..S-1:  4x dopri5 substeps on h' = MLP(h), then h = GRUCell(x_i, h)
  mu = h @ mu_w.T + mu_b ; logvar = h @ lv_w.T + lv_b

Key structural facts exploited:
  * DP_B == DP_A[6] (FSAL): the dopri5 solution point y_6 IS h_new, and the
    7th stage evaluation k_6 is dead code in the reference. So each substep
    needs only 6 MLP evals + the y_6 linear combination.
  * Pure data parallelism: batch 512 -> 8 cores x 64. No collectives.

Device layout (per core, "transposed chunked" form):
  A length-1024 vector per batch element lives as an SBUF tile [128, 8*64]:
  column block c (64 wide) = hidden chunk c, partition p = hidden c*128+p,
  column-within-block j = batch element j.
  Matmul out[m-chunk] = sum_k W.T[k,m].T @ act[k] : lhsT = weight tile
  [128(k), 128(m)] (bf16, resident in SBUF), rhs = activation chunk [128, 64]
  (bf16), PSUM out [128(m), 64] fp32, 8-chunk accumulation per output chunk.
  This chains layers with zero transposes.

Precision: weights bf16, matmul inputs bf16, PSUM accum fp32, all state
(h, k_j, y) fp32 on DVE, tanh/sigmoid on ACT (fp32 in, bf16 out mid-MLP).
Measured end-to-end error vs fp64 in simulation: ~3.4e-3 absmax relative.

dopri5 coefficients (hs * a_sj) are data-dependent (from t): they are loaded
per interval from a small DRAM table as per-partition scalars, so any t works.
"""
import sys
import os
from contextlib import ExitStack

sys.path.insert(0, "/opt/trn_rl_repo")

import numpy as np
import ml_dtypes

B, S, D, H, L = 512, 128, 128, 1024, 128
N_SUB = 4
N_CORES = 8
BL = B // N_CORES  # 64 batch per core
C = H // 128       # 8 hidden chunks

DP_A = (
    (),
    (1/5,),
    (3/40, 9/40),
    (44/45, -56/15, 32/9),
    (19372/6561, -25360/2187, 64448/6561, -212/729),
    (9017/3168, -355/33, 46732/5247, 49/176, -5103/18656),
    (35/384, 0.0, 500/1113, 125/192, -2187/6784, 11/84),
)

NZ_ROWS = [[j for j, a in enumerate(row) if a != 0.0] for row in DP_A]
N_COEF = sum(len(nz) for nz in NZ_ROWS[1:])  # 20
COEF_COLS = 32  # padded

bf16 = ml_dtypes.bfloat16
fp8 = ml_dtypes.float8_e4m3fn  # TRN FP8_EXP4: bit-compatible within +-240

# MLP weights in fp8e4m3 (moving operands stay bf16): halves the LDWEIGHTS
# SBUF read traffic feeding the power governor; numerically safe (measured:
# final rel err unchanged at ~3.5e-3 — the hs=dt/4 factor damps MLP error).
# GRU weights must stay bf16 (fp8 there measured 2.9e-2 > budget).
MLP_W_FP8 = True


def _split_multiwaits(bir_bytes):
    """Rewrite sync_info patterns the TPB 64B encoding can't hold:

    1. >1 sem waits on one instruction (e.g. the Tile For_i back-edge Drain)
       -> all but the last wait move to prepended single-wait NoOps.
    2. a wait together with a `sem-add-imm` update (staggered-reset prebumps
       aggregate bumps into big adds; wait_value and update_value share the
       one `semaphore_value` field) -> all waits move to prepended NoOps.

    Hoisting a wait to a preceding NoOp on the same engine is semantics-
    preserving (engine streams are FIFO). DMA opcodes are left alone.
    """
    import orjson
    j = orjson.loads(bir_bytes)
    ctr = 0
    for fn in j["functions"]:
        for blk in fn["blocks"]:
            out = []
            for ins in blk["instructions"]:
                si = ins.get("sync_info")
                waits = (si or {}).get("on_wait") or []
                updates = (si or {}).get("on_update") or []
                is_dma = ins.get("opcode", "").startswith("DMA")
                clash = (waits and not is_dma and any(
                    u.get("update_mode") == "sem-add-imm" and
                    u.get("update_value", 0) > 1 for u in updates))
                hoist = waits if clash else (
                    waits[:-1] if len(waits) > 1 else [])
                if hoist:
                    for w in hoist:
                        ctr += 1
                        nop = {
                            "engine": ins["engine"],
                            "ins": [],
                            "outs": [],
                            "name": f"waitsplit-{ctr}",
                            "opcode": "NoOp",
                            "sync_info": {"on_update": [], "on_wait": [w]},
                        }
                        if "debug" in ins:
                            nop["debug"] = ins["debug"]
                        out.append(nop)
                    si["on_wait"] = waits[len(hoist):]
                out.append(ins)
            blk["instructions"] = out
    return orjson.dumps(j)


def _patch_to_json(nc):
    from concourse import mybir
    nc.to_json_bytes = lambda: _split_multiwaits(
        mybir.module_to_json_bytes(nc.m))


def _build_program(n_intervals, zero_bias, uniform_dt=False):
    import concourse.bass as bass
    import concourse.tile as tile
    from concourse import mybir

    f32 = mybir.dt.float32
    bf = mybir.dt.bfloat16
    wdt = mybir.dt.float8e4 if MLP_W_FP8 else bf
    Tanh = mybir.ActivationFunctionType.Tanh
    Sigmoid = mybir.ActivationFunctionType.Sigmoid
    Ident = mybir.ActivationFunctionType.Identity
    AO = mybir.AluOpType

    NI = n_intervals

    nc = bass.Bass(trn_type="TRN2", target_bir_lowering=False, debug=False)

    w0t_d = nc.dram_tensor("w0t", [128, 64 * 128], wdt, kind="ExternalInput")
    w1t_d = nc.dram_tensor("w1t", [128, 64 * 128], wdt, kind="ExternalInput")
    w2t_d = nc.dram_tensor("w2t", [128, 64 * 128], wdt, kind="ExternalInput")
    whht_d = nc.dram_tensor("whht", [128, 192 * 128], bf, kind="ExternalInput")
    wiht_d = nc.dram_tensor("wiht", [128, 24 * 128], bf, kind="ExternalInput")
    muwt_d = nc.dram_tensor("muwt", [128, 8 * 128], bf, kind="ExternalInput")
    lvwt_d = nc.dram_tensor("lvwt", [128, 8 * 128], bf, kind="ExternalInput")
    # x resident in SBUF for the whole run: [p=din, (step, batch)] layout,
    # loaded once — no per-interval DMA, and the GRU input-side matmuls can
    # run at the top of the loop body to fill the h-carry dependency stall.
    xT_d = nc.dram_tensor("xT", [128, (NI + 1) * BL], bf,
                          kind="ExternalInput")
    # uniform dt (the harness case: t = arange*0.1): one static coef tile;
    # otherwise a per-interval table DMA'd inside the loop.
    coefs_d = nc.dram_tensor(
        "coefs", [128, COEF_COLS] if uniform_dt
        else [max(NI, 1) * 128, COEF_COLS], f32, kind="ExternalInput")
    # bias pack (fp32): cols 0..7 b0, 8..15 b1, 16..23 b2, 24..47 bih (r,z,n),
    # 48..71 bhh (r,z,n), 72 mu_b, 73 lv_b   (chunked per partition)
    bias_d = nc.dram_tensor("biases", [128, 74], f32, kind="ExternalInput")
    mu_out_d = nc.dram_tensor("mu_out", [128, BL], f32, kind="ExternalOutput")
    lv_out_d = nc.dram_tensor("lv_out", [128, BL], f32, kind="ExternalOutput")

    with ExitStack() as ctx:
        tc = ctx.enter_context(tile.TileContext(nc))
        wpool = ctx.enter_context(tc.tile_pool(name="weights", bufs=1))
        state = ctx.enter_context(tc.tile_pool(name="state", bufs=1))
        dyn = ctx.enter_context(tc.tile_pool(name="dyn", bufs=2))
        mid = ctx.enter_context(tc.tile_pool(name="mid", bufs=3))
        ypool = ctx.enter_context(tc.tile_pool(name="ypool", bufs=2))
        gpool = ctx.enter_context(tc.tile_pool(name="gru", bufs=2))
        pmlp = ctx.enter_context(tc.tile_pool(name="pmlp", bufs=4, space="PSUM"))
        pgru = ctx.enter_context(tc.tile_pool(name="pgru", bufs=1, space="PSUM"))

        w0 = wpool.tile([128, 64 * 128], wdt, tag="w0")
        w1 = wpool.tile([128, 64 * 128], wdt, tag="w1")
        w2 = wpool.tile([128, 64 * 128], wdt, tag="w2")
        whh = wpool.tile([128, 192 * 128], bf, tag="whh")
        wih = wpool.tile([128, 24 * 128], bf, tag="wih")
        muw = wpool.tile([128, 8 * 128], bf, tag="muw")
        lvw = wpool.tile([128, 8 * 128], bf, tag="lvw")
        biases = wpool.tile([128, 74], f32, tag="biases")
        xsb = wpool.tile([128, (NI + 1) * BL], bf, tag="xsb")
        loads = [(w0, w0t_d), (w1, w1t_d), (w2, w2t_d), (whh, whht_d),
                 (wih, wiht_d), (muw, muwt_d), (lvw, lvwt_d),
                 (biases, bias_d), (xsb, xT_d)]
        if uniform_dt:
            coefs_sb = wpool.tile([128, COEF_COLS], f32, tag="coefs_sb")
            loads.append((coefs_sb, coefs_d))
        for sb, dr in loads:
            nc.sync.dma_start(sb[:, :], dr[:, :])

        h = state.tile([128, C * BL], f32, tag="h")
        h_bf = state.tile([128, C * BL], bf, tag="h_bf")
        dummy_in = state.tile([128, 1], f32, tag="dummy_in")
        dummy_out = state.tile([128, 1], bf, tag="dummy_out")
        nc.vector.memset(dummy_in[:, :], 0.0)
        n_arch = 5 if zero_bias else 6
        karch = [state.tile([128, C * BL], f32, tag=f"k{j}", name=f"karch{j}")
                 for j in range(n_arch)]

        nc.vector.memset(h[:, :], 0.0)
        nc.vector.memset(h_bf[:, :], 0.0)

        def bias_col(idx):
            return biases[:, idx:idx + 1]

        HB = C * BL // 2  # half-tile width (256)

        def mm_layer_halves(wt, rhs_bf, psA, psB, nm=C):
            # MLP layer into two half-bank psum tiles: m-chunks 0..3 -> psA,
            # 4..7 -> psB (different banks: ACT consumes psA while PE writes
            # psB). k-OUTER order: the first 8 matmuls consume only rhs chunk
            # 0 (64 cols), so the PE unblocks as soon as the producer's first
            # chunk-grain op lands (producers emit y/h chunks in ascending
            # order). PSUM accumulation group is per BANK: start=True only on
            # the very first matmul into the bank (clears has_written for the
            # whole bank), stop=True on the last; per-element has_written
            # gives first-write-overwrite / then-accumulate for every m
            # region independently.
            # 3 blocks: [k0-3 x m0-7] consumes chunks 0-3 at 8-MM granularity
            # (starts right after the producer's chunk-0 op); [k4-7 x m0-3]
            # completes bank A at MM 48 so ACT overlaps the last block;
            # [k4-7 x m4-7] finishes bank B.
            order = [(k, m) for k in range(4) for m in range(nm)]
            order += [(k, m) for k in range(4, C) for m in range(min(4, nm))]
            order += [(k, m) for k in range(4, C) for m in range(4, nm)]
            for k, m in order:
                ps, mo = (psA, m) if m < 4 else (psB, m - 4)
                t = (k * nm + m) * 128
                nc.tensor.matmul(
                    ps[:, BL * mo: BL * mo + BL],
                    wt[:, t: t + 128],
                    rhs_bf[:, BL * k: BL * k + BL],
                    start=(k == 0 and mo == 0),
                    stop=(k == C - 1 and mo == 3),
                    skip_group_check=True,
                )

        def act_halves(out, psA, psB, func, bias_base):
            # out[:, :HB] = func(psA + b), out[:, HB:] = func(psB + b)
            if zero_bias:
                nc.scalar.activation(out[:, 0:HB], psA[:, :], func)
                nc.scalar.activation(out[:, HB:2 * HB], psB[:, :], func)
            else:
                for cc in range(C):
                    ps, co = (psA, cc) if cc < 4 else (psB, cc - 4)
                    nc.scalar.activation(
                        out[:, BL * cc: BL * cc + BL],
                        ps[:, BL * co: BL * co + BL],
                        func, bias=bias_col(bias_base + cc))

        def eval_mlp(rhs_bf):
            ps0a = pmlp.tile([128, HB], f32, tag="ps")
            ps0b = pmlp.tile([128, HB], f32, tag="ps")
            mm_layer_halves(w0, rhs_bf, ps0a, ps0b)
            u = mid.tile([128, C * BL], bf, tag="u")
            act_halves(u, ps0a, ps0b, Tanh, 0)
            ps1a = pmlp.tile([128, HB], f32, tag="ps")
            ps1b = pmlp.tile([128, HB], f32, tag="ps")
            mm_layer_halves(w1, u, ps1a, ps1b)
            v = mid.tile([128, C * BL], bf, tag="v")
            act_halves(v, ps1a, ps1b, Tanh, 8)
            ps2a = pmlp.tile([128, HB], f32, tag="ps")
            ps2b = pmlp.tile([128, HB], f32, tag="ps")
            mm_layer_halves(w2, v, ps2a, ps2b)
            return ps2a, ps2b

        def archive_k(j, ks_psum):
            # karch[j] = ks_psum + b2
            psA, psB = ks_psum
            if zero_bias:
                nc.scalar.copy(karch[j][:, 0:HB], psA[:, :])
                nc.scalar.copy(karch[j][:, HB:2 * HB], psB[:, :])
            else:
                for cc in range(C):
                    ps, co = (psA, cc) if cc < 4 else (psB, cc - 4)
                    nc.scalar.activation(
                        karch[j][:, BL * cc: BL * cc + BL],
                        ps[:, BL * co: BL * co + BL],
                        Ident, bias=bias_col(16 + cc))

        def stt(out, in0, cap, in1, chunked=False):
            # out = in0 * coef + in1; in0 may be a (psA, psB) half pair.
            # chunked: emit 64-col ops in ascending chunk order so the
            # consumer's k-outer matmuls unblock after the first small op.
            if isinstance(in0, tuple):
                psA, psB = in0
                if chunked:
                    for cc in range(C):
                        ps, co = (psA, cc) if cc < 4 else (psB, cc - 4)
                        nc.vector.scalar_tensor_tensor(
                            out[:, BL * cc: BL * cc + BL],
                            ps[:, BL * co: BL * co + BL], cap,
                            in1[:, BL * cc: BL * cc + BL],
                            AO.mult, AO.add)
                else:
                    nc.vector.scalar_tensor_tensor(
                        out[:, 0:HB], psA[:, :], cap, in1[:, 0:HB],
                        AO.mult, AO.add)
                    nc.vector.scalar_tensor_tensor(
                        out[:, HB:2 * HB], psB[:, :], cap, in1[:, HB:2 * HB],
                        AO.mult, AO.add)
            else:
                nc.vector.scalar_tensor_tensor(
                    out[:, :], in0[:, :], cap, in1[:, :], AO.mult, AO.add)

        def substep(coef_tile):
            # h, h_bf updated in place; coefficients at fixed cols 0..19
            cnt = 0

            def next_coef():
                nonlocal cnt
                ap = coef_tile[:, cnt:cnt + 1]
                cnt += 1
                return ap

            ks_psum = eval_mlp(h_bf)  # k_0
            for s in range(1, 7):
                nz = NZ_ROWS[s]
                if s - 1 < n_arch:
                    archive_k(s - 1, ks_psum)
                y_acc = None
                for idx, j in enumerate(nz):
                    cap = next_coef()
                    last = (idx == len(nz) - 1)
                    final_stage = (s == 6)
                    # last term's k comes straight from PSUM in the zero-bias
                    # fast path (j == s-1 always holds for the last term)
                    use_psum = last and zero_bias
                    src = ks_psum if use_psum else karch[j]
                    base = h if y_acc is None else y_acc
                    if last:
                        if final_stage:
                            # y_6 == h_new; emit the bf16 copy FIRST, chunk-
                            # grain, so the next substep's (or GRU's) k-outer
                            # matmuls unblock after one 64-col DVE op
                            stt(h_bf, src, cap, base, chunked=True)
                            stt(h, src, cap, base)
                        else:
                            y_bf = mid.tile([128, C * BL], bf, tag="ybf")
                            stt(y_bf, src, cap, base, chunked=True)
                    else:
                        if y_acc is None:
                            y_acc = ypool.tile([128, C * BL], f32, tag="yacc")
                        stt(y_acc, src, cap, base)
                if s < 6:
                    ks_psum = eval_mlp(y_bf)

        def gru_step(xt_ap):
            pr = pgru.tile([128, C * BL], f32, tag="pr")
            pz = pgru.tile([128, C * BL], f32, tag="pz")
            pgn = pgru.tile([128, C * BL], f32, tag="pgn")
            pin_ = pgru.tile([128, C * BL], f32, tag="pin")
            # hh matmuls, blocked like mm_layer_halves: [k0-3 x all gates]
            # consumes h_bf chunks 0-3 as the final substep's chunk-grain stt
            # emits them; then k4-7 per gate so pr completes first (the tail's
            # r-ACT chain starts while pz/pgn matmuls still run).
            order = [(k, sec, m) for k in range(4)
                     for sec in range(3) for m in range(C)]
            order += [(k, sec, m) for sec in range(3)
                      for k in range(4, C) for m in range(C)]
            for k, sec, m in order:
                ps = (pr, pz, pgn)[sec]
                mj = sec * 8 + m
                t = (k * 24 + mj) * 128
                nc.tensor.matmul(
                    ps[:, BL * m: BL * m + BL],
                    whh[:, t: t + 128],
                    h_bf[:, BL * k: BL * k + BL],
                    start=(k == 0 and m == 0),
                    stop=(sec == 2 and k == C - 1 and m == C - 1),
                    skip_group_check=True)
            # gi for r/z accumulates into the same banks (needs only xt)
            for sec, ps in ((0, pr), (1, pz)):
                for m in range(C):
                    mj = sec * 8 + m
                    nc.tensor.matmul(
                        ps[:, BL * m: BL * m + BL],
                        wih[:, mj * 128: mj * 128 + 128],
                        xt_ap,
                        start=False, stop=(m == C - 1),
                        skip_group_check=True)
            for m in range(C):
                mj = 16 + m
                nc.tensor.matmul(
                    pin_[:, BL * m: BL * m + BL],
                    wih[:, mj * 128: mj * 128 + 128],
                    xt_ap,
                    start=True, stop=True)

            r = gpool.tile([128, C * BL], f32, tag="r")
            z = gpool.tile([128, C * BL], f32, tag="z")
            n = gpool.tile([128, C * BL], f32, tag="n")
            t1 = gpool.tile([128, C * BL], f32, tag="t1")
            pre = gpool.tile([128, C * BL], f32, tag="pre")
            d = gpool.tile([128, C * BL], f32, tag="d")
            e = gpool.tile([128, C * BL], f32, tag="e")
            if zero_bias:
                # per-half chain for the deep part (r->t1->pre), then chunk
                # grain for n/d/e/h_bf so the next eval's k-outer matmuls
                # start after one small op per chunk
                for hb in range(2):
                    sl = slice(hb * HB, (hb + 1) * HB)
                    nc.scalar.activation(r[:, sl], pr[:, sl], Sigmoid)
                    nc.scalar.activation(z[:, sl], pz[:, sl], Sigmoid)
                    nc.vector.tensor_mul(t1[:, sl], r[:, sl], pgn[:, sl])
                    nc.vector.tensor_add(pre[:, sl], t1[:, sl], pin_[:, sl])
                    for cc in range(4 * hb, 4 * hb + 4):
                        cs = slice(BL * cc, BL * cc + BL)
                        nc.scalar.activation(n[:, cs], pre[:, cs], Tanh)
                        nc.vector.tensor_sub(d[:, cs], h[:, cs], n[:, cs])
                        nc.vector.tensor_mul(e[:, cs], z[:, cs], d[:, cs])
                        nc.vector.tensor_add(h_bf[:, cs], n[:, cs], e[:, cs])
                    nc.vector.tensor_add(h[:, sl], n[:, sl], e[:, sl])
            else:
                for cc in range(C):
                    sl = slice(BL * cc, BL * cc + BL)
                    # bias for r gate = bih_r + bhh_r (host folds the sum into
                    # col 24.. for ih and 48.. for hh; here use both adds)
                    nc.scalar.activation(r[:, sl], pr[:, sl], Sigmoid,
                                         bias=bias_col(24 + cc))
                    nc.scalar.activation(z[:, sl], pz[:, sl], Sigmoid,
                                         bias=bias_col(24 + 8 + cc))
                    # t1 = (pgn + bhh_n) * r
                    nc.vector.scalar_tensor_tensor(
                        t1[:, sl], pgn[:, sl], bias_col(48 + 16 + cc),
                        r[:, sl], AO.add, AO.mult)
                    # pre = (pin + bih_n) + t1
                    nc.vector.scalar_tensor_tensor(
                        pre[:, sl], pin_[:, sl], bias_col(24 + 16 + cc),
                        t1[:, sl], AO.add, AO.add)
                nc.scalar.activation(n[:, :], pre[:, :], Tanh)
                nc.vector.tensor_sub(d[:, :], h[:, :], n[:, :])
                nc.vector.tensor_mul(e[:, :], z[:, :], d[:, :])
                nc.vector.tensor_add(h_bf[:, :], n[:, :], e[:, :])
                nc.vector.tensor_add(h[:, :], n[:, :], e[:, :])

        # ---- prologue: h = GRU(x_0, 0) -------------------------------------
        gru_step(xsb[:, 0:BL])

        # ---- main loop over observation intervals --------------------------
        # staggered_reset: no all-engine barrier at the back edge, so the PE
        # can start iteration j+1's stage-0 matmuls while DVE/ACT finish
        # iteration j's GRU tail. Stages = substeps (stage 3 includes GRU).
        if NI > 0:
            with tc.For_i(0, NI, staggered_reset=True,
                          back_edge_label="mainloop",
                          hint_engines=(mybir.EngineType.PE,)) as j:
                # dummy activation: absorbs the per-block ACT_TABLE_LOAD off
                # the critical path (first real tanh would otherwise stall)
                nc.scalar.activation(dummy_out[:, :], dummy_in[:, :], Tanh)
                # stage x_{j+1} out of the SBUF-resident pack early (idle
                # GPSIMD; matmul operands can't take register offsets)
                xt = dyn.tile([128, BL], bf, tag="xt")
                nc.gpsimd.tensor_copy(
                    xt[:, :], xsb[:, bass.ds((j + 1) * BL, BL)])
                if uniform_dt:
                    ct = coefs_sb
                else:
                    ct = dyn.tile([128, COEF_COLS], f32, tag="ct")
                    nc.sync.dma_start(
                        ct[:, :], coefs_d[bass.ds(j * 128, 128), :])
                for si in range(N_SUB):
                    if si > 0:
                        tc.stage_boundary()
                    substep(ct)
                    if si == N_SUB - 1:
                        # arm the PE back-edge branch prefetch while the GRU
                        # matmuls run (body >> one IRAM block)
                        tc.mark_branch_hint_location(
                            "mainloop", engines=(mybir.EngineType.PE,))
                gru_step(xt)

        # ---- epilogue: mu / logvar ----------------------------------------
        for wt, bcol, out_d in ((muw, 72, mu_out_d), (lvw, 73, lv_out_d)):
            po = pgru.tile([128, BL], f32, tag="pr")
            for k in range(C):
                nc.tensor.matmul(
                    po[:, :], wt[:, k * 128: k * 128 + 128],
                    h_bf[:, BL * k: BL * k + BL],
                    start=(k == 0), stop=(k == C - 1))
            osb = gpool.tile([128, BL], f32, tag="osb")
            if zero_bias:
                nc.scalar.copy(osb[:, :], po[:, :])
            else:
                nc.scalar.activation(osb[:, :], po[:, :], Ident,
                                     bias=bias_col(bcol))
            nc.sync.dma_start(out_d[:, :], osb[:, :])

    return nc


def _chunk_wT(w, dt=bf16):
    """[O, I] weight -> [128, (I/128)*(O/128)*128] tile pack.

    Tile (k, m) at col offset (k*nm + m)*128 holds W[m*128+f, k*128+p] at
    [p, f] (i.e. lhsT = W.T block), so matmul computes W @ act.
    """
    O, I = w.shape
    nk, nm = I // 128, O // 128
    a = np.ascontiguousarray(w.T)          # [I, O]
    a = a.reshape(nk, 128, nm, 128)        # k, p, m, f
    a = np.transpose(a, (1, 0, 2, 3))      # p, k, m, f
    return np.ascontiguousarray(a.reshape(128, nk * nm * 128)).astype(dt)


def _chunk_vec(v):
    """[H] -> [128, C] chunked per-partition layout (col c = chunk c)."""
    return np.ascontiguousarray(v.reshape(-1, 128).T).astype(np.float32)


def host_prep(inputs):
    """Build the per-core in_maps + metadata from the full inputs."""
    x = np.asarray(inputs["x"], np.float32)
    t = np.asarray(inputs["t"], np.float32)

    n_intervals = S - 1
    dts = (t[0, 1:, 0] - t[0, :-1, 0]).astype(np.float32)
    hs = (dts / np.float32(N_SUB)).astype(np.float32)

    coefs = np.zeros((n_intervals, COEF_COLS), np.float32)
    for ji in range(n_intervals):
        cols = []
        for srow in range(1, 7):
            for j in NZ_ROWS[srow]:
                cols.append(np.float32(hs[ji]) * np.float32(DP_A[srow][j]))
        coefs[ji, :len(cols)] = cols
    # uniform observation spacing (the setup_inputs case: t = arange*0.1):
    # every interval shares one coef vector -> keep it static in SBUF
    uniform_dt = bool(np.all(dts == dts[0]))
    if uniform_dt:
        coefs_full = np.repeat(coefs[0:1, :], 128, axis=0)  # [128, COEF_COLS]
    else:
        coefs_full = np.repeat(coefs[:, None, :], 128, axis=1).reshape(
            n_intervals * 128, COEF_COLS)

    bias_names = ("gru_b_ih", "gru_b_hh", "b0", "b1", "b2", "mu_b", "lv_b")
    zero_bias = all(not np.any(np.asarray(inputs[k])) for k in bias_names)

    biases = np.zeros((128, 74), np.float32)
    biases[:, 0:8] = _chunk_vec(np.asarray(inputs["b0"], np.float32))
    biases[:, 8:16] = _chunk_vec(np.asarray(inputs["b1"], np.float32))
    biases[:, 16:24] = _chunk_vec(np.asarray(inputs["b2"], np.float32))
    bih = _chunk_vec(np.asarray(inputs["gru_b_ih"], np.float32))
    bhh = _chunk_vec(np.asarray(inputs["gru_b_hh"], np.float32))
    # r/z gates consume bih+bhh as one folded bias (cols 24..39); the n gate
    # needs them apart: n(ih) at 40..47, n(hh) at 64..71 (within bhh 48..71)
    biases[:, 24:40] = (bih + bhh)[:, 0:16]
    biases[:, 40:48] = bih[:, 16:24]
    biases[:, 48:72] = bhh
    biases[:, 72] = np.asarray(inputs["mu_b"], np.float32)
    biases[:, 73] = np.asarray(inputs["lv_b"], np.float32)

    mwdt = fp8 if MLP_W_FP8 else bf16
    shared = {
        "w0t": _chunk_wT(np.asarray(inputs["w0"], np.float32), mwdt),
        "w1t": _chunk_wT(np.asarray(inputs["w1"], np.float32), mwdt),
        "w2t": _chunk_wT(np.asarray(inputs["w2"], np.float32), mwdt),
        "whht": _chunk_wT(np.asarray(inputs["gru_w_hh"], np.float32)),
        "wiht": _chunk_wT(np.asarray(inputs["gru_w_ih"], np.float32)),
        "muwt": _chunk_wT(np.asarray(inputs["mu_w"], np.float32)),
        "lvwt": _chunk_wT(np.asarray(inputs["lv_w"], np.float32)),
        "coefs": coefs_full,
        "biases": biases,
    }

    in_maps = []
    for cidx in range(N_CORES):
        xc = x[cidx * BL:(cidx + 1) * BL]               # [BL, S, D]
        xT = np.ascontiguousarray(np.transpose(xc, (2, 1, 0)))  # [D, S, BL]
        m = dict(shared)
        m["xT"] = xT.reshape(128, S * BL).astype(bf16)
        in_maps.append(m)
    return in_maps, zero_bias, uniform_dt


def kernel(**inputs):
    from concourse import bass_utils

    in_maps, zero_bias, uniform_dt = host_prep(inputs)
    nc = _build_program(S - 1, zero_bias, uniform_dt)
    _patch_to_json(nc)
    res = bass_utils.run_bass_kernel_spmd(
        nc, in_maps, core_ids=list(range(N_CORES)))
    mu = np.empty((B, L), np.float32)
    lv = np.empty((B, L), np.float32)
    for cidx in range(N_CORES):
        mu[cidx * BL:(cidx + 1) * BL] = np.asarray(
            res.results[cidx]["mu_out"], np.float32).T
        lv[cidx * BL:(cidx + 1) * BL] = np.asarray(
            res.results[cidx]["lv_out"], np.float32).T
    return mu, lv



# revision 31
# speedup vs baseline: 1.1699x; 1.0773x over previous
"""ODE-GRU encoder Trainium2 Bass kernel.

Model (per reference): B=512, S=128, D=128, H=1024, L=128.
  h = GRUCell(x_0, 0)
  for i in 1..S-1:  4x dopri5 substeps on h' = MLP(h), then h = GRUCell(x_i, h)
  mu = h @ mu_w.T + mu_b ; logvar = h @ lv_w.T + lv_b

Key structural facts exploited:
  * DP_B == DP_A[6] (FSAL): the dopri5 solution point y_6 IS h_new, and the
    7th stage evaluation k_6 is dead code in the reference. So each substep
    needs only 6 MLP evals + the y_6 linear combination.
  * Pure data parallelism: batch 512 -> 8 cores x 64. No collectives.

Device layout (per core, "transposed chunked" form):
  A length-1024 vector per batch element lives as an SBUF tile [128, 8*64]:
  column block c (64 wide) = hidden chunk c, partition p = hidden c*128+p,
  column-within-block j = batch element j.
  Matmul out[m-chunk] = sum_k W.T[k,m].T @ act[k] : lhsT = weight tile
  [128(k), 128(m)] (bf16, resident in SBUF), rhs = activation chunk [128, 64]
  (bf16), PSUM out [128(m), 64] fp32, 8-chunk accumulation per output chunk.
  This chains layers with zero transposes.

Precision: weights bf16, matmul inputs bf16, PSUM accum fp32, all state
(h, k_j, y) fp32 on DVE, tanh/sigmoid on ACT (fp32 in, bf16 out mid-MLP).
Measured end-to-end error vs fp64 in simulation: ~3.4e-3 absmax relative.

dopri5 coefficients (hs * a_sj) are data-dependent (from t): they are loaded
per interval from a small DRAM table as per-partition scalars, so any t works.
"""
import sys
import os
from contextlib import ExitStack

sys.path.insert(0, "/opt/trn_rl_repo")

import numpy as np
import ml_dtypes

B, S, D, H, L = 512, 128, 128, 1024, 128
N_SUB = 4
N_CORES = 8
BL = B // N_CORES  # 64 batch per core
C = H // 128       # 8 hidden chunks

DP_A = (
    (),
    (1/5,),
    (3/40, 9/40),
    (44/45, -56/15, 32/9),
    (19372/6561, -25360/2187, 64448/6561, -212/729),
    (9017/3168, -355/33, 46732/5247, 49/176, -5103/18656),
    (35/384, 0.0, 500/1113, 125/192, -2187/6784, 11/84),
)

NZ_ROWS = [[j for j, a in enumerate(row) if a != 0.0] for row in DP_A]
N_COEF = sum(len(nz) for nz in NZ_ROWS[1:])  # 20
COEF_COLS = 32  # padded

bf16 = ml_dtypes.bfloat16
fp8 = ml_dtypes.float8_e4m3fn  # TRN FP8_EXP4: bit-compatible within +-240

# MLP weights in fp8e4m3 (moving operands stay bf16): halves the LDWEIGHTS
# SBUF read traffic feeding the power governor; numerically safe (measured:
# final rel err unchanged at ~3.5e-3 — the hs=dt/4 factor damps MLP error).
# GRU weights must stay bf16 (fp8 there measured 2.9e-2 > budget).
MLP_W_FP8 = True


def _split_multiwaits(bir_bytes):
    """Rewrite sync_info patterns the TPB 64B encoding can't hold:

    1. >1 sem waits on one instruction (e.g. the Tile For_i back-edge Drain)
       -> all but the last wait move to prepended single-wait NoOps.
    2. a wait together with a `sem-add-imm` update (staggered-reset prebumps
       aggregate bumps into big adds; wait_value and update_value share the
       one `semaphore_value` field) -> all waits move to prepended NoOps.

    Hoisting a wait to a preceding NoOp on the same engine is semantics-
    preserving (engine streams are FIFO). DMA opcodes are left alone.
    """
    import orjson
    j = orjson.loads(bir_bytes)
    ctr = 0
    for fn in j["functions"]:
        for blk in fn["blocks"]:
            out = []
            for ins in blk["instructions"]:
                si = ins.get("sync_info")
                waits = (si or {}).get("on_wait") or []
                updates = (si or {}).get("on_update") or []
                is_dma = ins.get("opcode", "").startswith("DMA")
                clash = (waits and not is_dma and any(
                    u.get("update_mode") == "sem-add-imm" and
                    u.get("update_value", 0) > 1 for u in updates))
                hoist = waits if clash else (
                    waits[:-1] if len(waits) > 1 else [])
                if hoist:
                    for w in hoist:
                        ctr += 1
                        nop = {
                            "engine": ins["engine"],
                            "ins": [],
                            "outs": [],
                            "name": f"waitsplit-{ctr}",
                            "opcode": "NoOp",
                            "sync_info": {"on_update": [], "on_wait": [w]},
                        }
                        if "debug" in ins:
                            nop["debug"] = ins["debug"]
                        out.append(nop)
                    si["on_wait"] = waits[len(hoist):]
                out.append(ins)
            blk["instructions"] = out
    return orjson.dumps(j)


def _patch_to_json(nc):
    from concourse import mybir
    nc.to_json_bytes = lambda: _split_multiwaits(
        mybir.module_to_json_bytes(nc.m))


def _build_program(n_intervals, zero_bias, uniform_dt=False, coef_vals=None):
    import concourse.bass as bass
    import concourse.tile as tile
    from concourse import mybir

    f32 = mybir.dt.float32
    bf = mybir.dt.bfloat16
    wdt = mybir.dt.float8e4 if MLP_W_FP8 else bf
    Tanh = mybir.ActivationFunctionType.Tanh
    Sigmoid = mybir.ActivationFunctionType.Sigmoid
    Ident = mybir.ActivationFunctionType.Identity
    AO = mybir.AluOpType

    NI = n_intervals

    nc = bass.Bass(trn_type="TRN2", target_bir_lowering=False, debug=False)

    w0t_d = nc.dram_tensor("w0t", [128, 64 * 128], wdt, kind="ExternalInput")
    w1t_d = nc.dram_tensor("w1t", [128, 64 * 128], wdt, kind="ExternalInput")
    w2t_d = nc.dram_tensor("w2t", [128, 64 * 128], wdt, kind="ExternalInput")
    whht_d = nc.dram_tensor("whht", [128, 192 * 128], bf, kind="ExternalInput")
    wiht_d = nc.dram_tensor("wiht", [128, 24 * 128], bf, kind="ExternalInput")
    muwt_d = nc.dram_tensor("muwt", [128, 8 * 128], bf, kind="ExternalInput")
    lvwt_d = nc.dram_tensor("lvwt", [128, 8 * 128], bf, kind="ExternalInput")
    # x resident in SBUF for the whole run: [p=din, (step, batch)] layout,
    # loaded once — no per-interval DMA, and the GRU input-side matmuls can
    # run at the top of the loop body to fill the h-carry dependency stall.
    xT_d = nc.dram_tensor("xT", [128, (NI + 1) * BL], bf,
                          kind="ExternalInput")
    # uniform dt (the harness case: t = arange*0.1): one static coef tile;
    # otherwise a per-interval table DMA'd inside the loop.
    coefs_d = nc.dram_tensor(
        "coefs", [128, COEF_COLS] if uniform_dt
        else [max(NI, 1) * 128, COEF_COLS], f32, kind="ExternalInput")
    # bias pack (fp32): cols 0..7 b0, 8..15 b1, 16..23 b2, 24..47 bih (r,z,n),
    # 48..71 bhh (r,z,n), 72 mu_b, 73 lv_b   (chunked per partition)
    bias_d = nc.dram_tensor("biases", [128, 74], f32, kind="ExternalInput")
    mu_out_d = nc.dram_tensor("mu_out", [128, BL], f32, kind="ExternalOutput")
    lv_out_d = nc.dram_tensor("lv_out", [128, BL], f32, kind="ExternalOutput")

    with ExitStack() as ctx:
        tc = ctx.enter_context(tile.TileContext(nc))
        wpool = ctx.enter_context(tc.tile_pool(name="weights", bufs=1))
        state = ctx.enter_context(tc.tile_pool(name="state", bufs=1))
        dyn = ctx.enter_context(tc.tile_pool(name="dyn", bufs=2))
        mid = ctx.enter_context(tc.tile_pool(name="mid", bufs=3))
        ypool = ctx.enter_context(tc.tile_pool(name="ypool", bufs=2))
        gpool = ctx.enter_context(tc.tile_pool(name="gru", bufs=2))
        pmlp = ctx.enter_context(tc.tile_pool(name="pmlp", bufs=4, space="PSUM"))
        pgru = ctx.enter_context(tc.tile_pool(name="pgru", bufs=1, space="PSUM"))

        w0 = wpool.tile([128, 64 * 128], wdt, tag="w0")
        w1 = wpool.tile([128, 64 * 128], wdt, tag="w1")
        w2 = wpool.tile([128, 64 * 128], wdt, tag="w2")
        whh = wpool.tile([128, 192 * 128], bf, tag="whh")
        wih = wpool.tile([128, 24 * 128], bf, tag="wih")
        muw = wpool.tile([128, 8 * 128], bf, tag="muw")
        lvw = wpool.tile([128, 8 * 128], bf, tag="lvw")
        biases = wpool.tile([128, 74], f32, tag="biases")
        xsb = wpool.tile([128, (NI + 1) * BL], bf, tag="xsb")
        loads = [(w0, w0t_d), (w1, w1t_d), (w2, w2t_d), (whh, whht_d),
                 (wih, wiht_d), (muw, muwt_d), (lvw, lvwt_d),
                 (biases, bias_d), (xsb, xT_d)]
        if uniform_dt:
            coefs_sb = wpool.tile([128, COEF_COLS], f32, tag="coefs_sb")
            loads.append((coefs_sb, coefs_d))
        for sb, dr in loads:
            nc.sync.dma_start(sb[:, :], dr[:, :])

        h = state.tile([128, C * BL], f32, tag="h")
        h_bf = state.tile([128, C * BL], bf, tag="h_bf")
        dummy_in = state.tile([128, 1], f32, tag="dummy_in")
        dummy_out = state.tile([128, 1], bf, tag="dummy_out")
        nc.vector.memset(dummy_in[:, :], 0.0)
        n_arch = 5 if zero_bias else 6
        karch = [state.tile([128, C * BL], f32, tag=f"k{j}", name=f"karch{j}")
                 for j in range(n_arch)]

        nc.vector.memset(h[:, :], 0.0)
        nc.vector.memset(h_bf[:, :], 0.0)

        def bias_col(idx):
            return biases[:, idx:idx + 1]

        HB = C * BL // 2  # half-tile width (256)

        def mm_layer_halves(wt, rhs_bf, psA, psB, nm=C):
            # MLP layer into two half-bank psum tiles: m-chunks 0..3 -> psA,
            # 4..7 -> psB (different banks: ACT consumes psA while PE writes
            # psB). k-OUTER order: the first 8 matmuls consume only rhs chunk
            # 0 (64 cols), so the PE unblocks as soon as the producer's first
            # chunk-grain op lands (producers emit y/h chunks in ascending
            # order). PSUM accumulation group is per BANK: start=True only on
            # the very first matmul into the bank (clears has_written for the
            # whole bank), stop=True on the last; per-element has_written
            # gives first-write-overwrite / then-accumulate for every m
            # region independently.
            # 3 blocks: [k0-3 x m0-7] consumes chunks 0-3 at 8-MM granularity
            # (starts right after the producer's chunk-0 op); [k4-7 x m0-3]
            # completes bank A at MM 48 so ACT overlaps the last block;
            # [k4-7 x m4-7] finishes bank B.
            order = [(k, m) for k in range(4) for m in range(nm)]
            order += [(k, m) for k in range(4, C) for m in range(min(4, nm))]
            order += [(k, m) for k in range(4, C) for m in range(4, nm)]
            for k, m in order:
                ps, mo = (psA, m) if m < 4 else (psB, m - 4)
                t = (k * nm + m) * 128
                nc.tensor.matmul(
                    ps[:, BL * mo: BL * mo + BL],
                    wt[:, t: t + 128],
                    rhs_bf[:, BL * k: BL * k + BL],
                    start=(k == 0 and mo == 0),
                    stop=(k == C - 1 and mo == 3),
                    skip_group_check=True,
                )

        def act_halves(out, psA, psB, func, bias_base):
            # out[:, :HB] = func(psA + b), out[:, HB:] = func(psB + b)
            if zero_bias:
                nc.scalar.activation(out[:, 0:HB], psA[:, :], func)
                nc.scalar.activation(out[:, HB:2 * HB], psB[:, :], func)
            else:
                for cc in range(C):
                    ps, co = (psA, cc) if cc < 4 else (psB, cc - 4)
                    nc.scalar.activation(
                        out[:, BL * cc: BL * cc + BL],
                        ps[:, BL * co: BL * co + BL],
                        func, bias=bias_col(bias_base + cc))

        def eval_mlp(rhs_bf):
            ps0a = pmlp.tile([128, HB], f32, tag="ps")
            ps0b = pmlp.tile([128, HB], f32, tag="ps")
            mm_layer_halves(w0, rhs_bf, ps0a, ps0b)
            u = mid.tile([128, C * BL], bf, tag="u")
            act_halves(u, ps0a, ps0b, Tanh, 0)
            ps1a = pmlp.tile([128, HB], f32, tag="ps")
            ps1b = pmlp.tile([128, HB], f32, tag="ps")
            mm_layer_halves(w1, u, ps1a, ps1b)
            v = mid.tile([128, C * BL], bf, tag="v")
            act_halves(v, ps1a, ps1b, Tanh, 8)
            ps2a = pmlp.tile([128, HB], f32, tag="ps")
            ps2b = pmlp.tile([128, HB], f32, tag="ps")
            mm_layer_halves(w2, v, ps2a, ps2b)
            return ps2a, ps2b

        def archive_k(j, ks_psum):
            # karch[j] = ks_psum + b2
            psA, psB = ks_psum
            if zero_bias:
                nc.scalar.copy(karch[j][:, 0:HB], psA[:, :])
                nc.scalar.copy(karch[j][:, HB:2 * HB], psB[:, :])
            else:
                for cc in range(C):
                    ps, co = (psA, cc) if cc < 4 else (psB, cc - 4)
                    nc.scalar.activation(
                        karch[j][:, BL * cc: BL * cc + BL],
                        ps[:, BL * co: BL * co + BL],
                        Ident, bias=bias_col(16 + cc))

        def stt(out, in0, cap, in1, eng=None):
            # out = in0 * coef + in1; in0 may be a (psA, psB) half pair.
            # eng: engine to emit on (default DVE). The y-accumulation chains
            # go to the otherwise-idle GPSIMD so the DVE (which produces the
            # PE-critical y_bf/h_bf) keeps pace with the PE.
            eng = eng or nc.vector
            if isinstance(in0, tuple):
                psA, psB = in0
                eng.scalar_tensor_tensor(
                    out[:, 0:HB], psA[:, :], cap, in1[:, 0:HB],
                    AO.mult, AO.add)
                eng.scalar_tensor_tensor(
                    out[:, HB:2 * HB], psB[:, :], cap, in1[:, HB:2 * HB],
                    AO.mult, AO.add)
            else:
                eng.scalar_tensor_tensor(
                    out[:, :], in0[:, :], cap, in1[:, :], AO.mult, AO.add)

        def substep(coef_tile):
            # h, h_bf updated in place. Uniform-dt: coefficients are float
            # immediates (compile-time constants) — required for the GPSIMD
            # y-accumulation path (TensorScalarPtr is not a Pool opcode) and
            # saves the per-partition scalar reads. Otherwise: per-partition
            # scalar APs at fixed cols 0..19 of the DMA'd coef tile.
            cnt = 0

            def next_coef():
                nonlocal cnt
                if coef_vals is not None:
                    cap = float(coef_vals[cnt])
                else:
                    cap = coef_tile[:, cnt:cnt + 1]
                cnt += 1
                return cap

            ks_psum = eval_mlp(h_bf)  # k_0
            for s in range(1, 7):
                nz = NZ_ROWS[s]
                if s - 1 < n_arch:
                    archive_k(s - 1, ks_psum)
                y_acc = None
                for idx, j in enumerate(nz):
                    cap = next_coef()
                    last = (idx == len(nz) - 1)
                    final_stage = (s == 6)
                    # last term's k comes straight from PSUM in the zero-bias
                    # fast path (j == s-1 always holds for the last term)
                    use_psum = last and zero_bias
                    src = ks_psum if use_psum else karch[j]
                    base = h if y_acc is None else y_acc
                    if last:
                        if final_stage:
                            # y_6 == h_new; emit the bf16 copy FIRST so the
                            # next substep's matmuls unblock one DVE op sooner
                            stt(h_bf, src, cap, base)
                            stt(h, src, cap, base)
                        else:
                            y_bf = mid.tile([128, C * BL], bf, tag="ybf")
                            stt(y_bf, src, cap, base)
                    else:
                        if y_acc is None:
                            y_acc = ypool.tile([128, C * BL], f32, tag="yacc")
                        stt(y_acc, src, cap, base,
                            eng=nc.gpsimd if coef_vals is not None else None)
                if s < 6:
                    ks_psum = eval_mlp(y_bf)

        def gru_step(xt_ap):
            pr = pgru.tile([128, C * BL], f32, tag="pr")
            pz = pgru.tile([128, C * BL], f32, tag="pz")
            pgn = pgru.tile([128, C * BL], f32, tag="pgn")
            pin_ = pgru.tile([128, C * BL], f32, tag="pin")
            # hh matmuls, blocked like mm_layer_halves: [k0-3 x all gates]
            # consumes h_bf chunks 0-3 as the final substep's chunk-grain stt
            # emits them; then k4-7 per gate so pr completes first (the tail's
            # r-ACT chain starts while pz/pgn matmuls still run).
            order = [(k, sec, m) for k in range(4)
                     for sec in range(3) for m in range(C)]
            order += [(k, sec, m) for sec in range(3)
                      for k in range(4, C) for m in range(C)]
            for k, sec, m in order:
                ps = (pr, pz, pgn)[sec]
                mj = sec * 8 + m
                t = (k * 24 + mj) * 128
                nc.tensor.matmul(
                    ps[:, BL * m: BL * m + BL],
                    whh[:, t: t + 128],
                    h_bf[:, BL * k: BL * k + BL],
                    start=(k == 0 and m == 0),
                    stop=(sec == 2 and k == C - 1 and m == C - 1),
                    skip_group_check=True)
            # gi for r/z accumulates into the same banks (needs only xt)
            for sec, ps in ((0, pr), (1, pz)):
                for m in range(C):
                    mj = sec * 8 + m
                    nc.tensor.matmul(
                        ps[:, BL * m: BL * m + BL],
                        wih[:, mj * 128: mj * 128 + 128],
                        xt_ap,
                        start=False, stop=(m == C - 1),
                        skip_group_check=True)
            for m in range(C):
                mj = 16 + m
                nc.tensor.matmul(
                    pin_[:, BL * m: BL * m + BL],
                    wih[:, mj * 128: mj * 128 + 128],
                    xt_ap,
                    start=True, stop=True)

            r = gpool.tile([128, C * BL], f32, tag="r")
            z = gpool.tile([128, C * BL], f32, tag="z")
            n = gpool.tile([128, C * BL], f32, tag="n")
            t1 = gpool.tile([128, C * BL], f32, tag="t1")
            pre = gpool.tile([128, C * BL], f32, tag="pre")
            d = gpool.tile([128, C * BL], f32, tag="d")
            e = gpool.tile([128, C * BL], f32, tag="e")
            if zero_bias:
                # per-half chain: h_bf half-0 lands early so the next
                # interval's leading matmuls start while half-1 finishes
                for hb in range(2):
                    sl = slice(hb * HB, (hb + 1) * HB)
                    nc.scalar.activation(r[:, sl], pr[:, sl], Sigmoid)
                    nc.scalar.activation(z[:, sl], pz[:, sl], Sigmoid)
                    nc.vector.tensor_mul(t1[:, sl], r[:, sl], pgn[:, sl])
                    nc.vector.tensor_add(pre[:, sl], t1[:, sl], pin_[:, sl])
                    nc.scalar.activation(n[:, sl], pre[:, sl], Tanh)
                    nc.vector.tensor_sub(d[:, sl], h[:, sl], n[:, sl])
                    nc.vector.tensor_mul(e[:, sl], z[:, sl], d[:, sl])
                    nc.vector.tensor_add(h_bf[:, sl], n[:, sl], e[:, sl])
                    nc.vector.tensor_add(h[:, sl], n[:, sl], e[:, sl])
            else:
                for cc in range(C):
                    sl = slice(BL * cc, BL * cc + BL)
                    # bias for r gate = bih_r + bhh_r (host folds the sum into
                    # col 24.. for ih and 48.. for hh; here use both adds)
                    nc.scalar.activation(r[:, sl], pr[:, sl], Sigmoid,
                                         bias=bias_col(24 + cc))
                    nc.scalar.activation(z[:, sl], pz[:, sl], Sigmoid,
                                         bias=bias_col(24 + 8 + cc))
                    # t1 = (pgn + bhh_n) * r
                    nc.vector.scalar_tensor_tensor(
                        t1[:, sl], pgn[:, sl], bias_col(48 + 16 + cc),
                        r[:, sl], AO.add, AO.mult)
                    # pre = (pin + bih_n) + t1
                    nc.vector.scalar_tensor_tensor(
                        pre[:, sl], pin_[:, sl], bias_col(24 + 16 + cc),
                        t1[:, sl], AO.add, AO.add)
                nc.scalar.activation(n[:, :], pre[:, :], Tanh)
                nc.vector.tensor_sub(d[:, :], h[:, :], n[:, :])
                nc.vector.tensor_mul(e[:, :], z[:, :], d[:, :])
                nc.vector.tensor_add(h_bf[:, :], n[:, :], e[:, :])
                nc.vector.tensor_add(h[:, :], n[:, :], e[:, :])

        # ---- prologue: h = GRU(x_0, 0) -------------------------------------
        gru_step(xsb[:, 0:BL])

        # ---- main loop over observation intervals --------------------------
        # staggered_reset: no all-engine barrier at the back edge, so the PE
        # can start iteration j+1's stage-0 matmuls while DVE/ACT finish
        # iteration j's GRU tail. Stages = substeps (stage 3 includes GRU).
        if NI > 0:
            with tc.For_i(0, NI, staggered_reset=True,
                          back_edge_label="mainloop",
                          hint_engines=(mybir.EngineType.PE,)) as j:
                # dummy activation: absorbs the per-block ACT_TABLE_LOAD off
                # the critical path (first real tanh would otherwise stall)
                nc.scalar.activation(dummy_out[:, :], dummy_in[:, :], Tanh)
                # stage x_{j+1} out of the SBUF-resident pack early (idle
                # GPSIMD; matmul operands can't take register offsets)
                xt = dyn.tile([128, BL], bf, tag="xt")
                nc.gpsimd.tensor_copy(
                    xt[:, :], xsb[:, bass.ds((j + 1) * BL, BL)])
                if uniform_dt:
                    ct = coefs_sb
                else:
                    ct = dyn.tile([128, COEF_COLS], f32, tag="ct")
                    nc.sync.dma_start(
                        ct[:, :], coefs_d[bass.ds(j * 128, 128), :])
                for si in range(N_SUB):
                    if si > 0:
                        tc.stage_boundary()
                    substep(ct)
                    if si == N_SUB - 1:
                        # arm the PE back-edge branch prefetch while the GRU
                        # matmuls run (body >> one IRAM block)
                        tc.mark_branch_hint_location(
                            "mainloop", engines=(mybir.EngineType.PE,))
                gru_step(xt)

        # ---- epilogue: mu / logvar ----------------------------------------
        for wt, bcol, out_d in ((muw, 72, mu_out_d), (lvw, 73, lv_out_d)):
            po = pgru.tile([128, BL], f32, tag="pr")
            for k in range(C):
                nc.tensor.matmul(
                    po[:, :], wt[:, k * 128: k * 128 + 128],
                    h_bf[:, BL * k: BL * k + BL],
                    start=(k == 0), stop=(k == C - 1))
            osb = gpool.tile([128, BL], f32, tag="osb")
            if zero_bias:
                nc.scalar.copy(osb[:, :], po[:, :])
            else:
                nc.scalar.activation(osb[:, :], po[:, :], Ident,
                                     bias=bias_col(bcol))
            nc.sync.dma_start(out_d[:, :], osb[:, :])

    return nc


def _chunk_wT(w, dt=bf16):
    """[O, I] weight -> [128, (I/128)*(O/128)*128] tile pack.

    Tile (k, m) at col offset (k*nm + m)*128 holds W[m*128+f, k*128+p] at
    [p, f] (i.e. lhsT = W.T block), so matmul computes W @ act.
    """
    O, I = w.shape
    nk, nm = I // 128, O // 128
    a = np.ascontiguousarray(w.T)          # [I, O]
    a = a.reshape(nk, 128, nm, 128)        # k, p, m, f
    a = np.transpose(a, (1, 0, 2, 3))      # p, k, m, f
    return np.ascontiguousarray(a.reshape(128, nk * nm * 128)).astype(dt)


def _chunk_vec(v):
    """[H] -> [128, C] chunked per-partition layout (col c = chunk c)."""
    return np.ascontiguousarray(v.reshape(-1, 128).T).astype(np.float32)


def host_prep(inputs):
    """Build the per-core in_maps + metadata from the full inputs."""
    x = np.asarray(inputs["x"], np.float32)
    t = np.asarray(inputs["t"], np.float32)

    n_intervals = S - 1
    dts = (t[0, 1:, 0] - t[0, :-1, 0]).astype(np.float32)
    hs = (dts / np.float32(N_SUB)).astype(np.float32)

    coefs = np.zeros((n_intervals, COEF_COLS), np.float32)
    for ji in range(n_intervals):
        cols = []
        for srow in range(1, 7):
            for j in NZ_ROWS[srow]:
                cols.append(np.float32(hs[ji]) * np.float32(DP_A[srow][j]))
        coefs[ji, :len(cols)] = cols
    # uniform observation spacing (the setup_inputs case: t = arange*0.1):
    # every interval shares one coef vector -> keep it static in SBUF
    uniform_dt = bool(np.all(dts == dts[0]))
    if uniform_dt:
        coefs_full = np.repeat(coefs[0:1, :], 128, axis=0)  # [128, COEF_COLS]
    else:
        coefs_full = np.repeat(coefs[:, None, :], 128, axis=1).reshape(
            n_intervals * 128, COEF_COLS)

    bias_names = ("gru_b_ih", "gru_b_hh", "b0", "b1", "b2", "mu_b", "lv_b")
    zero_bias = all(not np.any(np.asarray(inputs[k])) for k in bias_names)

    biases = np.zeros((128, 74), np.float32)
    biases[:, 0:8] = _chunk_vec(np.asarray(inputs["b0"], np.float32))
    biases[:, 8:16] = _chunk_vec(np.asarray(inputs["b1"], np.float32))
    biases[:, 16:24] = _chunk_vec(np.asarray(inputs["b2"], np.float32))
    bih = _chunk_vec(np.asarray(inputs["gru_b_ih"], np.float32))
    bhh = _chunk_vec(np.asarray(inputs["gru_b_hh"], np.float32))
    # r/z gates consume bih+bhh as one folded bias (cols 24..39); the n gate
    # needs them apart: n(ih) at 40..47, n(hh) at 64..71 (within bhh 48..71)
    biases[:, 24:40] = (bih + bhh)[:, 0:16]
    biases[:, 40:48] = bih[:, 16:24]
    biases[:, 48:72] = bhh
    biases[:, 72] = np.asarray(inputs["mu_b"], np.float32)
    biases[:, 73] = np.asarray(inputs["lv_b"], np.float32)

    mwdt = fp8 if MLP_W_FP8 else bf16
    shared = {
        "w0t": _chunk_wT(np.asarray(inputs["w0"], np.float32), mwdt),
        "w1t": _chunk_wT(np.asarray(inputs["w1"], np.float32), mwdt),
        "w2t": _chunk_wT(np.asarray(inputs["w2"], np.float32), mwdt),
        "whht": _chunk_wT(np.asarray(inputs["gru_w_hh"], np.float32)),
        "wiht": _chunk_wT(np.asarray(inputs["gru_w_ih"], np.float32)),
        "muwt": _chunk_wT(np.asarray(inputs["mu_w"], np.float32)),
        "lvwt": _chunk_wT(np.asarray(inputs["lv_w"], np.float32)),
        "coefs": coefs_full,
        "biases": biases,
    }

    in_maps = []
    for cidx in range(N_CORES):
        xc = x[cidx * BL:(cidx + 1) * BL]               # [BL, S, D]
        xT = np.ascontiguousarray(np.transpose(xc, (2, 1, 0)))  # [D, S, BL]
        m = dict(shared)
        m["xT"] = xT.reshape(128, S * BL).astype(bf16)
        in_maps.append(m)
    coef_vals = [float(v) for v in coefs[0, :N_COEF]] if uniform_dt else None
    return in_maps, zero_bias, uniform_dt, coef_vals


def kernel(**inputs):
    from concourse import bass_utils

    in_maps, zero_bias, uniform_dt, coef_vals = host_prep(inputs)
    nc = _build_program(S - 1, zero_bias, uniform_dt, coef_vals)
    _patch_to_json(nc)
    res = bass_utils.run_bass_kernel_spmd(
        nc, in_maps, core_ids=list(range(N_CORES)))
    mu = np.empty((B, L), np.float32)
    lv = np.empty((B, L), np.float32)
    for cidx in range(N_CORES):
        mu[cidx * BL:(cidx + 1) * BL] = np.asarray(
            res.results[cidx]["mu_out"], np.float32).T
        lv[cidx * BL:(cidx + 1) * BL] = np.asarray(
            res.results[cidx]["lv_out"], np.float32).T
    return mu, lv



# revision 38
# speedup vs baseline: 1.1898x; 1.0170x over previous
"""ODE-GRU encoder Trainium2 Bass kernel.

Model (per reference): B=512, S=128, D=128, H=1024, L=128.
  h = GRUCell(x_0, 0)
  for i in 1..S-1:  4x dopri5 substeps on h' = MLP(h), then h = GRUCell(x_i, h)
  mu = h @ mu_w.T + mu_b ; logvar = h @ lv_w.T + lv_b

Key structural facts exploited:
  * DP_B == DP_A[6] (FSAL): the dopri5 solution point y_6 IS h_new, and the
    7th stage evaluation k_6 is dead code in the reference. So each substep
    needs only 6 MLP evals + the y_6 linear combination.
  * Pure data parallelism: batch 512 -> 8 cores x 64. No collectives.

Device layout (per core, "transposed chunked" form):
  A length-1024 vector per batch element lives as an SBUF tile [128, 8*64]:
  column block c (64 wide) = hidden chunk c, partition p = hidden c*128+p,
  column-within-block j = batch element j.
  Matmul out[m-chunk] = sum_k W.T[k,m].T @ act[k] : lhsT = weight tile
  [128(k), 128(m)] (bf16, resident in SBUF), rhs = activation chunk [128, 64]
  (bf16), PSUM out [128(m), 64] fp32, 8-chunk accumulation per output chunk.
  This chains layers with zero transposes.

Precision: weights bf16, matmul inputs bf16, PSUM accum fp32, all state
(h, k_j, y) fp32 on DVE, tanh/sigmoid on ACT (fp32 in, bf16 out mid-MLP).
Measured end-to-end error vs fp64 in simulation: ~3.4e-3 absmax relative.

dopri5 coefficients (hs * a_sj) are data-dependent (from t): they are loaded
per interval from a small DRAM table as per-partition scalars, so any t works.
"""
import sys
import os
from contextlib import ExitStack

sys.path.insert(0, "/opt/trn_rl_repo")

import numpy as np
import ml_dtypes

B, S, D, H, L = 512, 128, 128, 1024, 128
N_SUB = 4
N_CORES = 8
BL = B // N_CORES  # 64 batch per core
C = H // 128       # 8 hidden chunks

DP_A = (
    (),
    (1/5,),
    (3/40, 9/40),
    (44/45, -56/15, 32/9),
    (19372/6561, -25360/2187, 64448/6561, -212/729),
    (9017/3168, -355/33, 46732/5247, 49/176, -5103/18656),
    (35/384, 0.0, 500/1113, 125/192, -2187/6784, 11/84),
)

NZ_ROWS = [[j for j, a in enumerate(row) if a != 0.0] for row in DP_A]
N_COEF = sum(len(nz) for nz in NZ_ROWS[1:])  # 20
COEF_COLS = 32  # padded

bf16 = ml_dtypes.bfloat16
fp8 = ml_dtypes.float8_e4m3fn  # TRN FP8_EXP4: bit-compatible within +-240

# MLP weights in fp8e4m3 (moving operands stay bf16): halves the LDWEIGHTS
# SBUF read traffic feeding the power governor; numerically safe (measured:
# final rel err unchanged at ~3.5e-3 — the hs=dt/4 factor damps MLP error).
# GRU weights must stay bf16 (fp8 there measured 2.9e-2 > budget).
MLP_W_FP8 = True


def _split_multiwaits(bir_bytes):
    """Rewrite sync_info patterns the TPB 64B encoding can't hold:

    1. >1 sem waits on one instruction (e.g. the Tile For_i back-edge Drain)
       -> all but the last wait move to prepended single-wait NoOps.
    2. a wait together with a `sem-add-imm` update (staggered-reset prebumps
       aggregate bumps into big adds; wait_value and update_value share the
       one `semaphore_value` field) -> all waits move to prepended NoOps.

    Hoisting a wait to a preceding NoOp on the same engine is semantics-
    preserving (engine streams are FIFO). DMA opcodes are left alone.
    """
    import orjson
    j = orjson.loads(bir_bytes)
    ctr = 0
    for fn in j["functions"]:
        for blk in fn["blocks"]:
            out = []
            for ins in blk["instructions"]:
                si = ins.get("sync_info")
                waits = (si or {}).get("on_wait") or []
                updates = (si or {}).get("on_update") or []
                is_dma = ins.get("opcode", "").startswith("DMA")
                clash = (waits and not is_dma and any(
                    u.get("update_mode") == "sem-add-imm" and
                    u.get("update_value", 0) > 1 for u in updates))
                hoist = waits if clash else (
                    waits[:-1] if len(waits) > 1 else [])
                if hoist:
                    for w in hoist:
                        ctr += 1
                        nop = {
                            "engine": ins["engine"],
                            "ins": [],
                            "outs": [],
                            "name": f"waitsplit-{ctr}",
                            "opcode": "NoOp",
                            "sync_info": {"on_update": [], "on_wait": [w]},
                        }
                        if "debug" in ins:
                            nop["debug"] = ins["debug"]
                        out.append(nop)
                    si["on_wait"] = waits[len(hoist):]
                out.append(ins)
            blk["instructions"] = out
    return orjson.dumps(j)


def _patch_to_json(nc):
    from concourse import mybir
    nc.to_json_bytes = lambda: _split_multiwaits(
        mybir.module_to_json_bytes(nc.m))


def _build_program(n_intervals, zero_bias, uniform_dt=False, coef_vals=None):
    import concourse.bass as bass
    import concourse.tile as tile
    from concourse import mybir

    f32 = mybir.dt.float32
    bf = mybir.dt.bfloat16
    wdt = mybir.dt.float8e4 if MLP_W_FP8 else bf
    Tanh = mybir.ActivationFunctionType.Tanh
    Sigmoid = mybir.ActivationFunctionType.Sigmoid
    Ident = mybir.ActivationFunctionType.Identity
    AO = mybir.AluOpType

    NI = n_intervals

    nc = bass.Bass(trn_type="TRN2", target_bir_lowering=False, debug=False)

    w0t_d = nc.dram_tensor("w0t", [128, 64 * 128], wdt, kind="ExternalInput")
    w1t_d = nc.dram_tensor("w1t", [128, 64 * 128], wdt, kind="ExternalInput")
    w2t_d = nc.dram_tensor("w2t", [128, 64 * 128], wdt, kind="ExternalInput")
    whht_d = nc.dram_tensor("whht", [128, 192 * 128], bf, kind="ExternalInput")
    wiht_d = nc.dram_tensor("wiht", [128, 24 * 128], bf, kind="ExternalInput")
    muwt_d = nc.dram_tensor("muwt", [128, 8 * 128], bf, kind="ExternalInput")
    lvwt_d = nc.dram_tensor("lvwt", [128, 8 * 128], bf, kind="ExternalInput")
    # x resident in SBUF for the whole run: [p=din, (step, batch)] layout,
    # loaded once — no per-interval DMA, and the GRU input-side matmuls can
    # run at the top of the loop body to fill the h-carry dependency stall.
    xT_d = nc.dram_tensor("xT", [128, (NI + 1) * BL], bf,
                          kind="ExternalInput")
    # uniform dt (the harness case: t = arange*0.1): one static coef tile;
    # otherwise a per-interval table DMA'd inside the loop.
    coefs_d = nc.dram_tensor(
        "coefs", [128, COEF_COLS] if uniform_dt
        else [max(NI, 1) * 128, COEF_COLS], f32, kind="ExternalInput")
    # bias pack (fp32): cols 0..7 b0, 8..15 b1, 16..23 b2, 24..47 bih (r,z,n),
    # 48..71 bhh (r,z,n), 72 mu_b, 73 lv_b   (chunked per partition)
    bias_d = nc.dram_tensor("biases", [128, 74], f32, kind="ExternalInput")
    mu_out_d = nc.dram_tensor("mu_out", [128, BL], f32, kind="ExternalOutput")
    lv_out_d = nc.dram_tensor("lv_out", [128, BL], f32, kind="ExternalOutput")

    with ExitStack() as ctx:
        tc = ctx.enter_context(tile.TileContext(nc))
        wpool = ctx.enter_context(tc.tile_pool(name="weights", bufs=1))
        state = ctx.enter_context(tc.tile_pool(name="state", bufs=1))
        dyn = ctx.enter_context(tc.tile_pool(name="dyn", bufs=2))
        mid = ctx.enter_context(tc.tile_pool(name="mid", bufs=3))
        ypool = ctx.enter_context(tc.tile_pool(name="ypool", bufs=2))
        gpool = ctx.enter_context(tc.tile_pool(name="gru", bufs=2))
        pmlp = ctx.enter_context(tc.tile_pool(name="pmlp", bufs=4, space="PSUM"))
        pgru = ctx.enter_context(tc.tile_pool(name="pgru", bufs=1, space="PSUM"))

        w0 = wpool.tile([128, 64 * 128], wdt, tag="w0")
        w1 = wpool.tile([128, 64 * 128], wdt, tag="w1")
        w2 = wpool.tile([128, 64 * 128], wdt, tag="w2")
        whh = wpool.tile([128, 192 * 128], bf, tag="whh")
        wih = wpool.tile([128, 24 * 128], bf, tag="wih")
        muw = wpool.tile([128, 8 * 128], bf, tag="muw")
        lvw = wpool.tile([128, 8 * 128], bf, tag="lvw")
        biases = wpool.tile([128, 74], f32, tag="biases")
        xsb = wpool.tile([128, (NI + 1) * BL], bf, tag="xsb")
        loads = [(w0, w0t_d), (w1, w1t_d), (w2, w2t_d), (whh, whht_d),
                 (wih, wiht_d), (muw, muwt_d), (lvw, lvwt_d),
                 (biases, bias_d), (xsb, xT_d)]
        if uniform_dt:
            coefs_sb = wpool.tile([128, COEF_COLS], f32, tag="coefs_sb")
            loads.append((coefs_sb, coefs_d))
        for sb, dr in loads:
            nc.sync.dma_start(sb[:, :], dr[:, :])

        h = state.tile([128, C * BL], f32, tag="h")
        h_bf = state.tile([128, C * BL], bf, tag="h_bf")
        dummy_in = state.tile([128, 1], f32, tag="dummy_in")
        dummy_out = state.tile([128, 1], bf, tag="dummy_out")
        nc.vector.memset(dummy_in[:, :], 0.0)
        n_arch = 5 if zero_bias else 6
        karch = [state.tile([128, C * BL], f32, tag=f"k{j}", name=f"karch{j}")
                 for j in range(n_arch)]

        nc.vector.memset(h[:, :], 0.0)
        nc.vector.memset(h_bf[:, :], 0.0)

        def bias_col(idx):
            return biases[:, idx:idx + 1]

        HB = C * BL // 2  # half-tile width (256)

        def mm_layer_halves(wt, rhs_bf, psA, psB, nm=C):
            # MLP layer into two half-bank psum tiles: m-chunks 0..3 -> psA,
            # 4..7 -> psB (different banks: ACT consumes psA while PE writes
            # psB). k-OUTER order: the first 8 matmuls consume only rhs chunk
            # 0 (64 cols), so the PE unblocks as soon as the producer's first
            # chunk-grain op lands (producers emit y/h chunks in ascending
            # order). PSUM accumulation group is per BANK: start=True only on
            # the very first matmul into the bank (clears has_written for the
            # whole bank), stop=True on the last; per-element has_written
            # gives first-write-overwrite / then-accumulate for every m
            # region independently.
            # 3 blocks: [k0-3 x m0-7] consumes chunks 0-3 at 8-MM granularity
            # (starts right after the producer's chunk-0 op); [k4-7 x m0-3]
            # completes bank A at MM 48 so ACT overlaps the last block;
            # [k4-7 x m4-7] finishes bank B.
            order = [(k, m) for k in range(4) for m in range(nm)]
            order += [(k, m) for k in range(4, C) for m in range(min(4, nm))]
            order += [(k, m) for k in range(4, C) for m in range(4, nm)]
            for k, m in order:
                ps, mo = (psA, m) if m < 4 else (psB, m - 4)
                t = (k * nm + m) * 128
                nc.tensor.matmul(
                    ps[:, BL * mo: BL * mo + BL],
                    wt[:, t: t + 128],
                    rhs_bf[:, BL * k: BL * k + BL],
                    start=(k == 0 and mo == 0),
                    stop=(k == C - 1 and mo == 3),
                    skip_group_check=True,
                )

        def act_halves(out, psA, psB, func, bias_base):
            # out[:, :HB] = func(psA + b), out[:, HB:] = func(psB + b)
            if zero_bias:
                nc.scalar.activation(out[:, 0:HB], psA[:, 0:HB], func)
                nc.scalar.activation(out[:, HB:2 * HB], psB[:, 0:HB], func)
            else:
                for cc in range(C):
                    ps, co = (psA, cc) if cc < 4 else (psB, cc - 4)
                    nc.scalar.activation(
                        out[:, BL * cc: BL * cc + BL],
                        ps[:, BL * co: BL * co + BL],
                        func, bias=bias_col(bias_base + cc))

        def eval_mlp(rhs_bf):
            # Each psum half-tile is allocated as a FULL bank ([128, 512]
            # fp32) with only the first HB columns used: PSUM reader gating
            # is bank-granular, so sharing a bank between the two halves
            # made every consumer (ACT, final stt) wait for the whole
            # layer's last matmul (~600-800ns stall per layer boundary and
            # per eval tail). Bank-aligned halves unblock at their own
            # accumulation stop (16 MMs early for bank A).
            ps0a = pmlp.tile([128, 2 * HB], f32, tag="ps")
            ps0b = pmlp.tile([128, 2 * HB], f32, tag="ps")
            mm_layer_halves(w0, rhs_bf, ps0a, ps0b)
            u = mid.tile([128, C * BL], bf, tag="u")
            act_halves(u, ps0a, ps0b, Tanh, 0)
            ps1a = pmlp.tile([128, 2 * HB], f32, tag="ps")
            ps1b = pmlp.tile([128, 2 * HB], f32, tag="ps")
            mm_layer_halves(w1, u, ps1a, ps1b)
            v = mid.tile([128, C * BL], bf, tag="v")
            act_halves(v, ps1a, ps1b, Tanh, 8)
            ps2a = pmlp.tile([128, 2 * HB], f32, tag="ps")
            ps2b = pmlp.tile([128, 2 * HB], f32, tag="ps")
            mm_layer_halves(w2, v, ps2a, ps2b)
            return ps2a, ps2b

        def archive_k(j, ks_psum):
            # karch[j] = ks_psum + b2
            psA, psB = ks_psum
            if zero_bias:
                nc.scalar.copy(karch[j][:, 0:HB], psA[:, 0:HB])
                nc.scalar.copy(karch[j][:, HB:2 * HB], psB[:, 0:HB])
            else:
                for cc in range(C):
                    ps, co = (psA, cc) if cc < 4 else (psB, cc - 4)
                    nc.scalar.activation(
                        karch[j][:, BL * cc: BL * cc + BL],
                        ps[:, BL * co: BL * co + BL],
                        Ident, bias=bias_col(16 + cc))

        def stt(out, in0, cap, in1, eng=None):
            # out = in0 * coef + in1; in0 may be a (psA, psB) half pair.
            # eng: engine to emit on (default DVE). The y-accumulation chains
            # go to the otherwise-idle GPSIMD so the DVE (which produces the
            # PE-critical y_bf/h_bf) keeps pace with the PE.
            eng = eng or nc.vector
            if isinstance(in0, tuple):
                psA, psB = in0
                eng.scalar_tensor_tensor(
                    out[:, 0:HB], psA[:, 0:HB], cap, in1[:, 0:HB],
                    AO.mult, AO.add)
                eng.scalar_tensor_tensor(
                    out[:, HB:2 * HB], psB[:, 0:HB], cap, in1[:, HB:2 * HB],
                    AO.mult, AO.add)
            else:
                eng.scalar_tensor_tensor(
                    out[:, :], in0[:, :], cap, in1[:, :], AO.mult, AO.add)

        def substep(coef_tile):
            # h, h_bf updated in place. Uniform-dt: coefficients are float
            # immediates (compile-time constants) — required for the GPSIMD
            # y-accumulation path (TensorScalarPtr is not a Pool opcode) and
            # saves the per-partition scalar reads. Otherwise: per-partition
            # scalar APs at fixed cols 0..19 of the DMA'd coef tile.
            cnt = 0

            def next_coef():
                nonlocal cnt
                if coef_vals is not None:
                    cap = float(coef_vals[cnt])
                else:
                    cap = coef_tile[:, cnt:cnt + 1]
                cnt += 1
                return cap

            ks_psum = eval_mlp(h_bf)  # k_0
            for s in range(1, 7):
                nz = NZ_ROWS[s]
                if s - 1 < n_arch:
                    archive_k(s - 1, ks_psum)
                y_acc = None
                for idx, j in enumerate(nz):
                    cap = next_coef()
                    last = (idx == len(nz) - 1)
                    final_stage = (s == 6)
                    # last term's k comes straight from PSUM in the zero-bias
                    # fast path (j == s-1 always holds for the last term)
                    use_psum = last and zero_bias
                    src = ks_psum if use_psum else karch[j]
                    base = h if y_acc is None else y_acc
                    if last:
                        if final_stage:
                            # y_6 == h_new; emit the bf16 copy FIRST so the
                            # next substep's matmuls unblock one DVE op sooner
                            stt(h_bf, src, cap, base)
                            stt(h, src, cap, base)
                        else:
                            y_bf = mid.tile([128, C * BL], bf, tag="ybf")
                            stt(y_bf, src, cap, base)
                    else:
                        if y_acc is None:
                            y_acc = ypool.tile([128, C * BL], f32, tag="yacc")
                        stt(y_acc, src, cap, base)
                if s < 6:
                    ks_psum = eval_mlp(y_bf)

        def gru_step(xt_ap):
            pr = pgru.tile([128, C * BL], f32, tag="pr")
            pz = pgru.tile([128, C * BL], f32, tag="pz")
            pgn = pgru.tile([128, C * BL], f32, tag="pgn")
            pin_ = pgru.tile([128, C * BL], f32, tag="pin")
            # gi first (needs only xt, staged at body top): opens each gate's
            # accumulation group. Then hh: [k0-3 x all gates] consumes h_bf
            # chunks 0-3 (emitted first by the final substep), then k4-7 per
            # gate ordered r, n, z: pr's bank closes ~64 MMs early so the
            # tail's r-ACT starts while the n/z matmuls still run; z (only
            # needed late in the tail chain) closes last.
            for sec, ps in ((0, pr), (1, pz), (2, pin_)):
                for m in range(C):
                    mj = (sec if sec < 2 else 2) * 8 + m
                    nc.tensor.matmul(
                        ps[:, BL * m: BL * m + BL],
                        wih[:, mj * 128: mj * 128 + 128],
                        xt_ap,
                        start=(m == 0),
                        stop=(sec == 2 and m == C - 1),
                        skip_group_check=True)
            order = [(k, sec, m) for k in range(4)
                     for sec in range(3) for m in range(C)]
            order += [(k, sec, m) for sec in (0, 2, 1)
                      for k in range(4, C) for m in range(C)]
            last_of = {0: order[-65], 2: order[-33], 1: order[-1]}
            for k, sec, m in order:
                ps = (pr, pz, pgn)[sec]
                mj = sec * 8 + m
                t = (k * 24 + mj) * 128
                nc.tensor.matmul(
                    ps[:, BL * m: BL * m + BL],
                    whh[:, t: t + 128],
                    h_bf[:, BL * k: BL * k + BL],
                    start=(sec == 2 and k == 0 and m == 0),
                    stop=((k, sec, m) == last_of[sec]),
                    skip_group_check=True)

            r = gpool.tile([128, C * BL], f32, tag="r")
            z = gpool.tile([128, C * BL], f32, tag="z")
            n = gpool.tile([128, C * BL], f32, tag="n")
            t1 = gpool.tile([128, C * BL], f32, tag="t1")
            pre = gpool.tile([128, C * BL], f32, tag="pre")
            d = gpool.tile([128, C * BL], f32, tag="d")
            e = gpool.tile([128, C * BL], f32, tag="e")
            if zero_bias:
                # op order matched to gate-bank close order (pr early, pz
                # last): r/t1/pre for both halves start while n/z matmuls
                # run; z-ACTs sit behind the n-ACTs so they don't block the
                # queue; z-dependent DVE ops (e, h_bf, h) come last.
                H0, H1 = slice(0, HB), slice(HB, 2 * HB)
                nc.scalar.activation(r[:, H0], pr[:, H0], Sigmoid)
                nc.scalar.activation(r[:, H1], pr[:, H1], Sigmoid)
                nc.vector.tensor_mul(t1[:, H0], r[:, H0], pgn[:, H0])
                nc.vector.tensor_add(pre[:, H0], t1[:, H0], pin_[:, H0])
                nc.vector.tensor_mul(t1[:, H1], r[:, H1], pgn[:, H1])
                nc.vector.tensor_add(pre[:, H1], t1[:, H1], pin_[:, H1])
                nc.scalar.activation(n[:, H0], pre[:, H0], Tanh)
                nc.scalar.activation(n[:, H1], pre[:, H1], Tanh)
                nc.scalar.activation(z[:, H0], pz[:, H0], Sigmoid)
                nc.scalar.activation(z[:, H1], pz[:, H1], Sigmoid)
                nc.vector.tensor_sub(d[:, H0], h[:, H0], n[:, H0])
                nc.vector.tensor_sub(d[:, H1], h[:, H1], n[:, H1])
                nc.vector.tensor_mul(e[:, H0], z[:, H0], d[:, H0])
                nc.vector.tensor_add(h_bf[:, H0], n[:, H0], e[:, H0])
                nc.vector.tensor_mul(e[:, H1], z[:, H1], d[:, H1])
                nc.vector.tensor_add(h_bf[:, H1], n[:, H1], e[:, H1])
                nc.vector.tensor_add(h[:, H0], n[:, H0], e[:, H0])
                nc.vector.tensor_add(h[:, H1], n[:, H1], e[:, H1])
            else:
                for cc in range(C):
                    sl = slice(BL * cc, BL * cc + BL)
                    # bias for r gate = bih_r + bhh_r (host folds the sum into
                    # col 24.. for ih and 48.. for hh; here use both adds)
                    nc.scalar.activation(r[:, sl], pr[:, sl], Sigmoid,
                                         bias=bias_col(24 + cc))
                    nc.scalar.activation(z[:, sl], pz[:, sl], Sigmoid,
                                         bias=bias_col(24 + 8 + cc))
                    # t1 = (pgn + bhh_n) * r
                    nc.vector.scalar_tensor_tensor(
                        t1[:, sl], pgn[:, sl], bias_col(48 + 16 + cc),
                        r[:, sl], AO.add, AO.mult)
                    # pre = (pin + bih_n) + t1
                    nc.vector.scalar_tensor_tensor(
                        pre[:, sl], pin_[:, sl], bias_col(24 + 16 + cc),
                        t1[:, sl], AO.add, AO.add)
                nc.scalar.activation(n[:, :], pre[:, :], Tanh)
                nc.vector.tensor_sub(d[:, :], h[:, :], n[:, :])
                nc.vector.tensor_mul(e[:, :], z[:, :], d[:, :])
                nc.vector.tensor_add(h_bf[:, :], n[:, :], e[:, :])
                nc.vector.tensor_add(h[:, :], n[:, :], e[:, :])

        # ---- prologue: h = GRU(x_0, 0) -------------------------------------
        gru_step(xsb[:, 0:BL])

        # ---- main loop over observation intervals --------------------------
        # staggered_reset: no all-engine barrier at the back edge, so the PE
        # can start iteration j+1's stage-0 matmuls while DVE/ACT finish
        # iteration j's GRU tail. Stages = substeps (stage 3 includes GRU).
        if NI > 0:
            with tc.For_i(0, NI, staggered_reset=True,
                          back_edge_label="mainloop",
                          hint_engines=(mybir.EngineType.PE,)) as j:
                # dummy activation: absorbs the per-block ACT_TABLE_LOAD off
                # the critical path (first real tanh would otherwise stall)
                nc.scalar.activation(dummy_out[:, :], dummy_in[:, :], Tanh)
                # stage x_{j+1} out of the SBUF-resident pack early (idle
                # GPSIMD; matmul operands can't take register offsets)
                xt = dyn.tile([128, BL], bf, tag="xt")
                nc.gpsimd.tensor_copy(
                    xt[:, :], xsb[:, bass.ds((j + 1) * BL, BL)])
                if uniform_dt:
                    ct = coefs_sb
                else:
                    ct = dyn.tile([128, COEF_COLS], f32, tag="ct")
                    nc.sync.dma_start(
                        ct[:, :], coefs_d[bass.ds(j * 128, 128), :])
                for si in range(N_SUB):
                    if si > 0:
                        tc.stage_boundary()
                    substep(ct)
                    if si == N_SUB - 1:
                        # arm the PE back-edge branch prefetch while the GRU
                        # matmuls run (body >> one IRAM block)
                        tc.mark_branch_hint_location(
                            "mainloop", engines=(mybir.EngineType.PE,))
                gru_step(xt)

        # ---- epilogue: mu / logvar ----------------------------------------
        for wt, bcol, out_d in ((muw, 72, mu_out_d), (lvw, 73, lv_out_d)):
            po = pgru.tile([128, BL], f32, tag="pr")
            for k in range(C):
                nc.tensor.matmul(
                    po[:, :], wt[:, k * 128: k * 128 + 128],
                    h_bf[:, BL * k: BL * k + BL],
                    start=(k == 0), stop=(k == C - 1))
            osb = gpool.tile([128, BL], f32, tag="osb")
            if zero_bias:
                nc.scalar.copy(osb[:, :], po[:, :])
            else:
                nc.scalar.activation(osb[:, :], po[:, :], Ident,
                                     bias=bias_col(bcol))
            nc.sync.dma_start(out_d[:, :], osb[:, :])

    return nc


def _chunk_wT(w, dt=bf16):
    """[O, I] weight -> [128, (I/128)*(O/128)*128] tile pack.

    Tile (k, m) at col offset (k*nm + m)*128 holds W[m*128+f, k*128+p] at
    [p, f] (i.e. lhsT = W.T block), so matmul computes W @ act.
    """
    O, I = w.shape
    nk, nm = I // 128, O // 128
    a = np.ascontiguousarray(w.T)          # [I, O]
    a = a.reshape(nk, 128, nm, 128)        # k, p, m, f
    a = np.transpose(a, (1, 0, 2, 3))      # p, k, m, f
    return np.ascontiguousarray(a.reshape(128, nk * nm * 128)).astype(dt)


def _chunk_vec(v):
    """[H] -> [128, C] chunked per-partition layout (col c = chunk c)."""
    return np.ascontiguousarray(v.reshape(-1, 128).T).astype(np.float32)


def host_prep(inputs):
    """Build the per-core in_maps + metadata from the full inputs."""
    x = np.asarray(inputs["x"], np.float32)
    t = np.asarray(inputs["t"], np.float32)

    n_intervals = S - 1
    dts = (t[0, 1:, 0] - t[0, :-1, 0]).astype(np.float32)
    hs = (dts / np.float32(N_SUB)).astype(np.float32)

    coefs = np.zeros((n_intervals, COEF_COLS), np.float32)
    for ji in range(n_intervals):
        cols = []
        for srow in range(1, 7):
            for j in NZ_ROWS[srow]:
                cols.append(np.float32(hs[ji]) * np.float32(DP_A[srow][j]))
        coefs[ji, :len(cols)] = cols
    # uniform observation spacing (the setup_inputs case: t = arange*0.1 —
    # fp32 rounding makes consecutive diffs differ in the last ulp, so use
    # allclose; the ~1e-7 relative dt perturbation is far below the error
    # scale): every interval shares one coef vector -> compile-time floats
    uniform_dt = bool(np.allclose(dts, dts[0], rtol=1e-5, atol=0.0))
    if uniform_dt:
        coefs_full = np.repeat(coefs[0:1, :], 128, axis=0)  # [128, COEF_COLS]
    else:
        coefs_full = np.repeat(coefs[:, None, :], 128, axis=1).reshape(
            n_intervals * 128, COEF_COLS)

    bias_names = ("gru_b_ih", "gru_b_hh", "b0", "b1", "b2", "mu_b", "lv_b")
    zero_bias = all(not np.any(np.asarray(inputs[k])) for k in bias_names)

    biases = np.zeros((128, 74), np.float32)
    biases[:, 0:8] = _chunk_vec(np.asarray(inputs["b0"], np.float32))
    biases[:, 8:16] = _chunk_vec(np.asarray(inputs["b1"], np.float32))
    biases[:, 16:24] = _chunk_vec(np.asarray(inputs["b2"], np.float32))
    bih = _chunk_vec(np.asarray(inputs["gru_b_ih"], np.float32))
    bhh = _chunk_vec(np.asarray(inputs["gru_b_hh"], np.float32))
    # r/z gates consume bih+bhh as one folded bias (cols 24..39); the n gate
    # needs them apart: n(ih) at 40..47, n(hh) at 64..71 (within bhh 48..71)
    biases[:, 24:40] = (bih + bhh)[:, 0:16]
    biases[:, 40:48] = bih[:, 16:24]
    biases[:, 48:72] = bhh
    biases[:, 72] = np.asarray(inputs["mu_b"], np.float32)
    biases[:, 73] = np.asarray(inputs["lv_b"], np.float32)

    mwdt = fp8 if MLP_W_FP8 else bf16
    shared = {
        "w0t": _chunk_wT(np.asarray(inputs["w0"], np.float32), mwdt),
        "w1t": _chunk_wT(np.asarray(inputs["w1"], np.float32), mwdt),
        "w2t": _chunk_wT(np.asarray(inputs["w2"], np.float32), mwdt),
        "whht": _chunk_wT(np.asarray(inputs["gru_w_hh"], np.float32)),
        "wiht": _chunk_wT(np.asarray(inputs["gru_w_ih"], np.float32)),
        "muwt": _chunk_wT(np.asarray(inputs["mu_w"], np.float32)),
        "lvwt": _chunk_wT(np.asarray(inputs["lv_w"], np.float32)),
        "coefs": coefs_full,
        "biases": biases,
    }

    in_maps = []
    for cidx in range(N_CORES):
        xc = x[cidx * BL:(cidx + 1) * BL]               # [BL, S, D]
        xT = np.ascontiguousarray(np.transpose(xc, (2, 1, 0)))  # [D, S, BL]
        m = dict(shared)
        m["xT"] = xT.reshape(128, S * BL).astype(bf16)
        in_maps.append(m)
    coef_vals = [float(v) for v in coefs[0, :N_COEF]] if uniform_dt else None
    return in_maps, zero_bias, uniform_dt, coef_vals


def kernel(**inputs):
    from concourse import bass_utils

    in_maps, zero_bias, uniform_dt, coef_vals = host_prep(inputs)
    nc = _build_program(S - 1, zero_bias, uniform_dt, coef_vals)
    _patch_to_json(nc)
    res = bass_utils.run_bass_kernel_spmd(
        nc, in_maps, core_ids=list(range(N_CORES)))
    mu = np.empty((B, L), np.float32)
    lv = np.empty((B, L), np.float32)
    for cidx in range(N_CORES):
        mu[cidx * BL:(cidx + 1) * BL] = np.asarray(
            res.results[cidx]["mu_out"], np.float32).T
        lv[cidx * BL:(cidx + 1) * BL] = np.asarray(
            res.results[cidx]["lv_out"], np.float32).T
    return mu, lv



# revision 47
# speedup vs baseline: 1.1920x; 1.0019x over previous
"""ODE-GRU encoder Trainium2 Bass kernel.

Model (per reference): B=512, S=128, D=128, H=1024, L=128.
  h = GRUCell(x_0, 0)
  for i in 1..S-1:  4x dopri5 substeps on h' = MLP(h), then h = GRUCell(x_i, h)
  mu = h @ mu_w.T + mu_b ; logvar = h @ lv_w.T + lv_b

Key structural facts exploited:
  * DP_B == DP_A[6] (FSAL): the dopri5 solution point y_6 IS h_new, and the
    7th stage evaluation k_6 is dead code in the reference. So each substep
    needs only 6 MLP evals + the y_6 linear combination.
  * Pure data parallelism: batch 512 -> 8 cores x 64. No collectives.

Device layout (per core, "transposed chunked" form):
  A length-1024 vector per batch element lives as an SBUF tile [128, 8*64]:
  column block c (64 wide) = hidden chunk c, partition p = hidden c*128+p,
  column-within-block j = batch element j.
  Matmul out[m-chunk] = sum_k W.T[k,m].T @ act[k] : lhsT = weight tile
  [128(k), 128(m)] (bf16, resident in SBUF), rhs = activation chunk [128, 64]
  (bf16), PSUM out [128(m), 64] fp32, 8-chunk accumulation per output chunk.
  This chains layers with zero transposes.

Precision: weights bf16, matmul inputs bf16, PSUM accum fp32, all state
(h, k_j, y) fp32 on DVE, tanh/sigmoid on ACT (fp32 in, bf16 out mid-MLP).
Measured end-to-end error vs fp64 in simulation: ~3.4e-3 absmax relative.

dopri5 coefficients (hs * a_sj) are data-dependent (from t): they are loaded
per interval from a small DRAM table as per-partition scalars, so any t works.
"""
import sys
import os
from contextlib import ExitStack

sys.path.insert(0, "/opt/trn_rl_repo")

import numpy as np
import ml_dtypes

B, S, D, H, L = 512, 128, 128, 1024, 128
N_SUB = 4
N_CORES = 8
BL = B // N_CORES  # 64 batch per core
C = H // 128       # 8 hidden chunks

DP_A = (
    (),
    (1/5,),
    (3/40, 9/40),
    (44/45, -56/15, 32/9),
    (19372/6561, -25360/2187, 64448/6561, -212/729),
    (9017/3168, -355/33, 46732/5247, 49/176, -5103/18656),
    (35/384, 0.0, 500/1113, 125/192, -2187/6784, 11/84),
)

NZ_ROWS = [[j for j, a in enumerate(row) if a != 0.0] for row in DP_A]
N_COEF = sum(len(nz) for nz in NZ_ROWS[1:])  # 20
COEF_COLS = 32  # padded

bf16 = ml_dtypes.bfloat16
fp8 = ml_dtypes.float8_e4m3fn  # TRN FP8_EXP4: bit-compatible within +-240

# MLP weights in fp8e4m3 (moving operands stay bf16): halves the LDWEIGHTS
# SBUF read traffic feeding the power governor; numerically safe (measured:
# final rel err unchanged at ~3.5e-3 — the hs=dt/4 factor damps MLP error).
# GRU weights must stay bf16 (fp8 there measured 2.9e-2 > budget).
MLP_W_FP8 = True


def _split_multiwaits(bir_bytes):
    """Rewrite sync_info patterns the TPB 64B encoding can't hold:

    1. >1 sem waits on one instruction (e.g. the Tile For_i back-edge Drain)
       -> all but the last wait move to prepended single-wait NoOps.
    2. a wait together with a `sem-add-imm` update (staggered-reset prebumps
       aggregate bumps into big adds; wait_value and update_value share the
       one `semaphore_value` field) -> all waits move to prepended NoOps.

    Hoisting a wait to a preceding NoOp on the same engine is semantics-
    preserving (engine streams are FIFO). DMA opcodes are left alone.
    """
    import orjson
    j = orjson.loads(bir_bytes)
    ctr = 0
    for fn in j["functions"]:
        for blk in fn["blocks"]:
            out = []
            for ins in blk["instructions"]:
                si = ins.get("sync_info")
                waits = (si or {}).get("on_wait") or []
                updates = (si or {}).get("on_update") or []
                is_dma = ins.get("opcode", "").startswith("DMA")
                clash = (waits and not is_dma and any(
                    u.get("update_mode") == "sem-add-imm" and
                    u.get("update_value", 0) > 1 for u in updates))
                hoist = waits if clash else (
                    waits[:-1] if len(waits) > 1 else [])
                if hoist:
                    for w in hoist:
                        ctr += 1
                        nop = {
                            "engine": ins["engine"],
                            "ins": [],
                            "outs": [],
                            "name": f"waitsplit-{ctr}",
                            "opcode": "NoOp",
                            "sync_info": {"on_update": [], "on_wait": [w]},
                        }
                        if "debug" in ins:
                            nop["debug"] = ins["debug"]
                        out.append(nop)
                    si["on_wait"] = waits[len(hoist):]
                out.append(ins)
            blk["instructions"] = out
    return orjson.dumps(j)


def _patch_to_json(nc):
    from concourse import mybir
    nc.to_json_bytes = lambda: _split_multiwaits(
        mybir.module_to_json_bytes(nc.m))


def _build_program(n_intervals, zero_bias, uniform_dt=False, coef_vals=None):
    import concourse.bass as bass
    import concourse.tile as tile
    from concourse import mybir

    f32 = mybir.dt.float32
    bf = mybir.dt.bfloat16
    wdt = mybir.dt.float8e4 if MLP_W_FP8 else bf
    Tanh = mybir.ActivationFunctionType.Tanh
    Sigmoid = mybir.ActivationFunctionType.Sigmoid
    Ident = mybir.ActivationFunctionType.Identity
    AO = mybir.AluOpType

    NI = n_intervals

    nc = bass.Bass(trn_type="TRN2", target_bir_lowering=False, debug=False)

    w0t_d = nc.dram_tensor("w0t", [128, 64 * 128], wdt, kind="ExternalInput")
    w1t_d = nc.dram_tensor("w1t", [128, 64 * 128], wdt, kind="ExternalInput")
    w2t_d = nc.dram_tensor("w2t", [128, 64 * 128], wdt, kind="ExternalInput")
    whht_d = nc.dram_tensor("whht", [128, 192 * 128], bf, kind="ExternalInput")
    wiht_d = nc.dram_tensor("wiht", [128, 24 * 128], bf, kind="ExternalInput")
    muwt_d = nc.dram_tensor("muwt", [128, 8 * 128], bf, kind="ExternalInput")
    lvwt_d = nc.dram_tensor("lvwt", [128, 8 * 128], bf, kind="ExternalInput")
    # x resident in SBUF for the whole run: [p=din, (step, batch)] layout,
    # loaded once — no per-interval DMA, and the GRU input-side matmuls can
    # run at the top of the loop body to fill the h-carry dependency stall.
    xT_d = nc.dram_tensor("xT", [128, (NI + 1) * BL], bf,
                          kind="ExternalInput")
    # uniform dt (the harness case: t = arange*0.1): one static coef tile;
    # otherwise a per-interval table DMA'd inside the loop.
    coefs_d = nc.dram_tensor(
        "coefs", [128, COEF_COLS] if uniform_dt
        else [max(NI, 1) * 128, COEF_COLS], f32, kind="ExternalInput")
    # bias pack (fp32): cols 0..7 b0, 8..15 b1, 16..23 b2, 24..47 bih (r,z,n),
    # 48..71 bhh (r,z,n), 72 mu_b, 73 lv_b   (chunked per partition)
    bias_d = nc.dram_tensor("biases", [128, 74], f32, kind="ExternalInput")
    mu_out_d = nc.dram_tensor("mu_out", [128, BL], f32, kind="ExternalOutput")
    lv_out_d = nc.dram_tensor("lv_out", [128, BL], f32, kind="ExternalOutput")

    with ExitStack() as ctx:
        tc = ctx.enter_context(tile.TileContext(nc))
        wpool = ctx.enter_context(tc.tile_pool(name="weights", bufs=1))
        state = ctx.enter_context(tc.tile_pool(name="state", bufs=1))
        dyn = ctx.enter_context(tc.tile_pool(name="dyn", bufs=2))
        mid = ctx.enter_context(tc.tile_pool(name="mid", bufs=3))
        ypool = ctx.enter_context(tc.tile_pool(name="ypool", bufs=2))
        gpool = ctx.enter_context(tc.tile_pool(name="gru", bufs=2))
        pmlp = ctx.enter_context(tc.tile_pool(name="pmlp", bufs=4, space="PSUM"))
        pgru = ctx.enter_context(tc.tile_pool(name="pgru", bufs=1, space="PSUM"))

        w0 = wpool.tile([128, 64 * 128], wdt, tag="w0")
        w1 = wpool.tile([128, 64 * 128], wdt, tag="w1")
        w2 = wpool.tile([128, 64 * 128], wdt, tag="w2")
        whh = wpool.tile([128, 192 * 128], bf, tag="whh")
        wih = wpool.tile([128, 24 * 128], bf, tag="wih")
        muw = wpool.tile([128, 8 * 128], bf, tag="muw")
        lvw = wpool.tile([128, 8 * 128], bf, tag="lvw")
        biases = wpool.tile([128, 74], f32, tag="biases")
        xsb = wpool.tile([128, (NI + 1) * BL], bf, tag="xsb")
        loads = [(w0, w0t_d), (w1, w1t_d), (w2, w2t_d), (whh, whht_d),
                 (wih, wiht_d), (muw, muwt_d), (lvw, lvwt_d),
                 (biases, bias_d), (xsb, xT_d)]
        if uniform_dt:
            coefs_sb = wpool.tile([128, COEF_COLS], f32, tag="coefs_sb")
            loads.append((coefs_sb, coefs_d))
        for sb, dr in loads:
            nc.sync.dma_start(sb[:, :], dr[:, :])

        h = state.tile([128, C * BL], f32, tag="h")
        h_bf = state.tile([128, C * BL], bf, tag="h_bf")
        dummy_in = state.tile([128, 1], f32, tag="dummy_in")
        dummy_out = state.tile([128, 1], bf, tag="dummy_out")
        nc.vector.memset(dummy_in[:, :], 0.0)
        n_arch = 5 if zero_bias else 6
        karch = [state.tile([128, C * BL], f32, tag=f"k{j}", name=f"karch{j}")
                 for j in range(n_arch)]

        nc.vector.memset(h[:, :], 0.0)
        nc.vector.memset(h_bf[:, :], 0.0)

        def bias_col(idx):
            return biases[:, idx:idx + 1]

        HB = C * BL // 2  # half-tile width (256)

        def mm_layer_halves(wt, rhs_bf, psA, psB, nm=C,
                            after_A=None, after_B=None):
            # MLP layer into two half-bank psum tiles: m-chunks 0..3 -> psA,
            # 4..7 -> psB (different banks: ACT consumes psA while PE writes
            # psB). k-OUTER order: the first 8 matmuls consume only rhs chunk
            # 0 (64 cols), so the PE unblocks as soon as the producer's first
            # chunk-grain op lands (producers emit y/h chunks in ascending
            # order). PSUM accumulation group is per BANK: start=True only on
            # the very first matmul into the bank (clears has_written for the
            # whole bank), stop=True on the last; per-element has_written
            # gives first-write-overwrite / then-accumulate for every m
            # region independently.
            # 3 blocks: [k0-3 x m0-7] consumes chunks 0-3 at 8-MM granularity
            # (starts right after the producer's chunk-0 op); [k4-7 x m0-3]
            # completes bank A at MM 48 so its consumers (emitted via the
            # after_A hook RIGHT HERE in program order — Tile's aggregated
            # sem bumps only resolve at dependency edges, so the hook
            # placement is what lets the consumer start before the layer
            # ends) overlap the last block; [k4-7 x m4-7] finishes bank B.
            def emit(block):
                for k, m in block:
                    ps, mo = (psA, m) if m < 4 else (psB, m - 4)
                    t = (k * nm + m) * 128
                    nc.tensor.matmul(
                        ps[:, BL * mo: BL * mo + BL],
                        wt[:, t: t + 128],
                        rhs_bf[:, BL * k: BL * k + BL],
                        start=(k == 0 and mo == 0),
                        stop=(k == C - 1 and mo == 3),
                        skip_group_check=True,
                    )
            emit([(k, m) for k in range(4) for m in range(nm)])
            emit([(k, m) for k in range(4, C) for m in range(min(4, nm))])
            if after_A is not None:
                after_A()
            emit([(k, m) for k in range(4, C) for m in range(4, nm)])
            if after_B is not None:
                after_B()

        def act_half(out, ps, hb, func, bias_base):
            # out[:, hb half] = func(ps + b)
            if zero_bias:
                nc.scalar.activation(
                    out[:, hb * HB:(hb + 1) * HB], ps[:, 0:HB], func)
            else:
                for co in range(4):
                    cc = hb * 4 + co
                    nc.scalar.activation(
                        out[:, BL * cc: BL * cc + BL],
                        ps[:, BL * co: BL * co + BL],
                        func, bias=bias_col(bias_base + cc))

        def eval_mlp(rhs_bf, after_A=None, after_B=None):
            # Each psum half-tile is allocated as a FULL bank ([128, 512]
            # fp32) with only the first HB columns used, so each half's
            # consumers gate on its own bank. The mid-layer tanh halves are
            # emitted via the after-bank hooks (see mm_layer_halves); the
            # caller's hooks receive the w2 psum halves the same way.
            ps0a = pmlp.tile([128, 2 * HB], f32, tag="ps")
            ps0b = pmlp.tile([128, 2 * HB], f32, tag="ps")
            u = mid.tile([128, C * BL], bf, tag="u")
            mm_layer_halves(w0, rhs_bf, ps0a, ps0b,
                            after_A=lambda: act_half(u, ps0a, 0, Tanh, 0),
                            after_B=lambda: act_half(u, ps0b, 1, Tanh, 0))
            ps1a = pmlp.tile([128, 2 * HB], f32, tag="ps")
            ps1b = pmlp.tile([128, 2 * HB], f32, tag="ps")
            v = mid.tile([128, C * BL], bf, tag="v")
            mm_layer_halves(w1, u, ps1a, ps1b,
                            after_A=lambda: act_half(v, ps1a, 0, Tanh, 8),
                            after_B=lambda: act_half(v, ps1b, 1, Tanh, 8))
            ps2a = pmlp.tile([128, 2 * HB], f32, tag="ps")
            ps2b = pmlp.tile([128, 2 * HB], f32, tag="ps")
            mm_layer_halves(
                w2, v, ps2a, ps2b,
                after_A=(lambda: after_A(ps2a)) if after_A else None,
                after_B=(lambda: after_B(ps2b)) if after_B else None)
            return ps2a, ps2b

        def archive_half(j, ps, hb):
            # karch[j] half hb = ps + b2
            if zero_bias:
                nc.scalar.copy(karch[j][:, hb * HB:(hb + 1) * HB],
                               ps[:, 0:HB])
            else:
                for co in range(4):
                    cc = hb * 4 + co
                    nc.scalar.activation(
                        karch[j][:, BL * cc: BL * cc + BL],
                        ps[:, BL * co: BL * co + BL],
                        Ident, bias=bias_col(16 + cc))

        def stt(out, in0, cap, in1):
            # out = in0 * coef + in1 (full width, DVE)
            nc.vector.scalar_tensor_tensor(
                out[:, :], in0[:, :], cap, in1[:, :], AO.mult, AO.add)

        def stt_half(out, ps, j, cap, in1, hb):
            # out[half hb] = k * coef + in1[half hb], where k comes from the
            # psum bank directly (zero-bias) or the biased archive karch[j]
            sl = slice(hb * HB, (hb + 1) * HB)
            in0 = ps[:, 0:HB] if zero_bias else karch[j][:, sl]
            nc.vector.scalar_tensor_tensor(
                out[:, sl], in0, cap, in1[:, sl], AO.mult, AO.add)

        def substep(coef_tile):
            # h, h_bf updated in place. Uniform-dt: coefficients are float
            # immediates (compile-time constants) — required for the GPSIMD
            # y-accumulation path (TensorScalarPtr is not a Pool opcode) and
            # saves the per-partition scalar reads. Otherwise: per-partition
            # scalar APs at fixed cols 0..19 of the DMA'd coef tile.
            cnt = 0

            def next_coef():
                nonlocal cnt
                if coef_vals is not None:
                    cap = float(coef_vals[cnt])
                else:
                    cap = coef_tile[:, cnt:cnt + 1]
                cnt += 1
                return cap

            # eval e computes k_e; stage s=e+1 consumes it. The y_acc chain
            # for stage s (karch reads only) is emitted BEFORE eval e; the
            # final y_s = k_e*c + y_acc halves are emitted INSIDE eval e via
            # the after-bank hooks, so each half fires as soon as its w2
            # psum bank closes (16 MMs before the layer ends for bank A).
            rhs = h_bf
            for e in range(6):
                s = e + 1
                nz = NZ_ROWS[s]
                caps = [next_coef() for _ in nz]
                y_acc = None
                base = h
                for idx, j in enumerate(nz[:-1]):
                    if y_acc is None:
                        y_acc = ypool.tile([128, C * BL], f32, tag="yacc")
                    stt(y_acc, karch[j], caps[idx], base)
                    base = y_acc
                fcap = caps[-1]
                fbase = base
                y_out = None
                if s < 6:
                    y_out = mid.tile([128, C * BL], bf, tag="ybf",
                                     name=f"ybf{s}")

                def hook(ps, hb, e=e, s=s, fcap=fcap, fbase=fbase,
                         y_out=y_out):
                    if e < n_arch:
                        archive_half(e, ps, hb)
                    if s == 6:
                        # y_6 == h_new; bf16 copy first (PE-critical)
                        stt_half(h_bf, ps, e, fcap, fbase, hb)
                        stt_half(h, ps, e, fcap, fbase, hb)
                    else:
                        stt_half(y_out, ps, e, fcap, fbase, hb)

                ks_psum = eval_mlp(rhs,
                                   after_A=lambda ps, h=hook: h(ps, 0),
                                   after_B=lambda ps, h=hook: h(ps, 1))
                rhs = y_out

        def gru_step(xt_ap):
            pr = pgru.tile([128, C * BL], f32, tag="pr")
            pz = pgru.tile([128, C * BL], f32, tag="pz")
            pgn = pgru.tile([128, C * BL], f32, tag="pgn")
            pin_ = pgru.tile([128, C * BL], f32, tag="pin")
            # gi first (needs only xt, staged at body top): opens each gate's
            # accumulation group. Then hh: [k0-3 x all gates] consumes h_bf
            # chunks 0-3 (emitted first by the final substep), then k4-7 per
            # gate ordered r, n, z: pr's bank closes ~64 MMs early so the
            # tail's r-ACT starts while the n/z matmuls still run; z (only
            # needed late in the tail chain) closes last.
            for sec, ps in ((0, pr), (1, pz), (2, pin_)):
                for m in range(C):
                    mj = (sec if sec < 2 else 2) * 8 + m
                    nc.tensor.matmul(
                        ps[:, BL * m: BL * m + BL],
                        wih[:, mj * 128: mj * 128 + 128],
                        xt_ap,
                        start=(m == 0),
                        stop=(sec == 2 and m == C - 1),
                        skip_group_check=True)
            r = gpool.tile([128, C * BL], f32, tag="r")
            z = gpool.tile([128, C * BL], f32, tag="z")
            n = gpool.tile([128, C * BL], f32, tag="n")
            t1 = gpool.tile([128, C * BL], f32, tag="t1")
            pre = gpool.tile([128, C * BL], f32, tag="pre")
            d = gpool.tile([128, C * BL], f32, tag="d")
            e = gpool.tile([128, C * BL], f32, tag="e")
            H0, H1 = slice(0, HB), slice(HB, 2 * HB)

            def emit_hh(block, stop_sec=None):
                for k, sec, m in block:
                    ps = (pr, pz, pgn)[sec]
                    mj = sec * 8 + m
                    t = (k * 24 + mj) * 128
                    nc.tensor.matmul(
                        ps[:, BL * m: BL * m + BL],
                        whh[:, t: t + 128],
                        h_bf[:, BL * k: BL * k + BL],
                        start=(sec == 2 and k == 0 and m == 0),
                        stop=(sec == stop_sec and k == C - 1 and m == C - 1),
                        skip_group_check=True)

            emit_hh([(k, sec, m) for k in range(4)
                     for sec in range(3) for m in range(C)])
            emit_hh([(k, 0, m) for k in range(4, C) for m in range(C)],
                    stop_sec=0)
            if zero_bias:
                # tail ops emitted right after the gate bank they need
                # closes (pr -> pgn -> pz): the chain starts ~2us before the
                # GRU matmuls end instead of after them.
                nc.scalar.activation(r[:, H0], pr[:, H0], Sigmoid)
                nc.scalar.activation(r[:, H1], pr[:, H1], Sigmoid)
            emit_hh([(k, 2, m) for k in range(4, C) for m in range(C)],
                    stop_sec=2)
            if zero_bias:
                nc.vector.tensor_mul(t1[:, H0], r[:, H0], pgn[:, H0])
                nc.vector.tensor_add(pre[:, H0], t1[:, H0], pin_[:, H0])
                nc.vector.tensor_mul(t1[:, H1], r[:, H1], pgn[:, H1])
                nc.vector.tensor_add(pre[:, H1], t1[:, H1], pin_[:, H1])
                nc.scalar.activation(n[:, H0], pre[:, H0], Tanh)
                nc.scalar.activation(n[:, H1], pre[:, H1], Tanh)
                nc.vector.tensor_sub(d[:, H0], h[:, H0], n[:, H0])
                nc.vector.tensor_sub(d[:, H1], h[:, H1], n[:, H1])
            emit_hh([(k, 1, m) for k in range(4, C) for m in range(C)],
                    stop_sec=1)
            if zero_bias:
                nc.scalar.activation(z[:, H0], pz[:, H0], Sigmoid)
                nc.scalar.activation(z[:, H1], pz[:, H1], Sigmoid)
                nc.vector.tensor_mul(e[:, H0], z[:, H0], d[:, H0])
                nc.vector.tensor_add(h_bf[:, H0], n[:, H0], e[:, H0])
                nc.vector.tensor_mul(e[:, H1], z[:, H1], d[:, H1])
                nc.vector.tensor_add(h_bf[:, H1], n[:, H1], e[:, H1])
                nc.vector.tensor_add(h[:, H0], n[:, H0], e[:, H0])
                nc.vector.tensor_add(h[:, H1], n[:, H1], e[:, H1])
            else:
                for cc in range(C):
                    sl = slice(BL * cc, BL * cc + BL)
                    # bias for r gate = bih_r + bhh_r (host folds the sum into
                    # col 24.. for ih and 48.. for hh; here use both adds)
                    nc.scalar.activation(r[:, sl], pr[:, sl], Sigmoid,
                                         bias=bias_col(24 + cc))
                    nc.scalar.activation(z[:, sl], pz[:, sl], Sigmoid,
                                         bias=bias_col(24 + 8 + cc))
                    # t1 = (pgn + bhh_n) * r
                    nc.vector.scalar_tensor_tensor(
                        t1[:, sl], pgn[:, sl], bias_col(48 + 16 + cc),
                        r[:, sl], AO.add, AO.mult)
                    # pre = (pin + bih_n) + t1
                    nc.vector.scalar_tensor_tensor(
                        pre[:, sl], pin_[:, sl], bias_col(24 + 16 + cc),
                        t1[:, sl], AO.add, AO.add)
                nc.scalar.activation(n[:, :], pre[:, :], Tanh)
                nc.vector.tensor_sub(d[:, :], h[:, :], n[:, :])
                nc.vector.tensor_mul(e[:, :], z[:, :], d[:, :])
                nc.vector.tensor_add(h_bf[:, :], n[:, :], e[:, :])
                nc.vector.tensor_add(h[:, :], n[:, :], e[:, :])

        # ---- prologue: h = GRU(x_0, 0) -------------------------------------
        gru_step(xsb[:, 0:BL])

        # ---- main loop over observation intervals --------------------------
        # staggered_reset: no all-engine barrier at the back edge, so the PE
        # can start iteration j+1's stage-0 matmuls while DVE/ACT finish
        # iteration j's GRU tail. Stages = substeps (stage 3 includes GRU).
        if NI > 0:
            with tc.For_i(0, NI, staggered_reset=True,
                          back_edge_label="mainloop",
                          hint_engines=(mybir.EngineType.PE,)) as j:
                # dummy activation: absorbs the per-block ACT_TABLE_LOAD off
                # the critical path (first real tanh would otherwise stall)
                nc.scalar.activation(dummy_out[:, :], dummy_in[:, :], Tanh)
                # stage x_{j+1} out of the SBUF-resident pack early (idle
                # GPSIMD; matmul operands can't take register offsets)
                xt = dyn.tile([128, BL], bf, tag="xt")
                nc.gpsimd.tensor_copy(
                    xt[:, :], xsb[:, bass.ds((j + 1) * BL, BL)])
                if uniform_dt:
                    ct = coefs_sb
                else:
                    ct = dyn.tile([128, COEF_COLS], f32, tag="ct")
                    nc.sync.dma_start(
                        ct[:, :], coefs_d[bass.ds(j * 128, 128), :])
                for si in range(N_SUB):
                    if si > 0:
                        tc.stage_boundary()
                    substep(ct)
                    if si == N_SUB - 1:
                        # arm the PE back-edge branch prefetch while the GRU
                        # matmuls run (body >> one IRAM block)
                        tc.mark_branch_hint_location(
                            "mainloop", engines=(mybir.EngineType.PE,))
                gru_step(xt)

        # ---- epilogue: mu / logvar ----------------------------------------
        for wt, bcol, out_d in ((muw, 72, mu_out_d), (lvw, 73, lv_out_d)):
            po = pgru.tile([128, BL], f32, tag="pr")
            for k in range(C):
                nc.tensor.matmul(
                    po[:, :], wt[:, k * 128: k * 128 + 128],
                    h_bf[:, BL * k: BL * k + BL],
                    start=(k == 0), stop=(k == C - 1))
            osb = gpool.tile([128, BL], f32, tag="osb")
            if zero_bias:
                nc.scalar.copy(osb[:, :], po[:, :])
            else:
                nc.scalar.activation(osb[:, :], po[:, :], Ident,
                                     bias=bias_col(bcol))
            nc.sync.dma_start(out_d[:, :], osb[:, :])

    return nc


def _chunk_wT(w, dt=bf16):
    """[O, I] weight -> [128, (I/128)*(O/128)*128] tile pack.

    Tile (k, m) at col offset (k*nm + m)*128 holds W[m*128+f, k*128+p] at
    [p, f] (i.e. lhsT = W.T block), so matmul computes W @ act.
    """
    O, I = w.shape
    nk, nm = I // 128, O // 128
    a = np.ascontiguousarray(w.T)          # [I, O]
    a = a.reshape(nk, 128, nm, 128)        # k, p, m, f
    a = np.transpose(a, (1, 0, 2, 3))      # p, k, m, f
    return np.ascontiguousarray(a.reshape(128, nk * nm * 128)).astype(dt)


def _chunk_vec(v):
    """[H] -> [128, C] chunked per-partition layout (col c = chunk c)."""
    return np.ascontiguousarray(v.reshape(-1, 128).T).astype(np.float32)


def host_prep(inputs):
    """Build the per-core in_maps + metadata from the full inputs."""
    x = np.asarray(inputs["x"], np.float32)
    t = np.asarray(inputs["t"], np.float32)

    n_intervals = S - 1
    dts = (t[0, 1:, 0] - t[0, :-1, 0]).astype(np.float32)
    hs = (dts / np.float32(N_SUB)).astype(np.float32)

    coefs = np.zeros((n_intervals, COEF_COLS), np.float32)
    for ji in range(n_intervals):
        cols = []
        for srow in range(1, 7):
            for j in NZ_ROWS[srow]:
                cols.append(np.float32(hs[ji]) * np.float32(DP_A[srow][j]))
        coefs[ji, :len(cols)] = cols
    # uniform observation spacing (the setup_inputs case: t = arange*0.1 —
    # fp32 rounding makes consecutive diffs differ in the last ulp, so use
    # allclose; the ~1e-7 relative dt perturbation is far below the error
    # scale): every interval shares one coef vector -> compile-time floats
    uniform_dt = bool(np.allclose(dts, dts[0], rtol=1e-5, atol=0.0))
    if uniform_dt:
        coefs_full = np.repeat(coefs[0:1, :], 128, axis=0)  # [128, COEF_COLS]
    else:
        coefs_full = np.repeat(coefs[:, None, :], 128, axis=1).reshape(
            n_intervals * 128, COEF_COLS)

    bias_names = ("gru_b_ih", "gru_b_hh", "b0", "b1", "b2", "mu_b", "lv_b")
    zero_bias = all(not np.any(np.asarray(inputs[k])) for k in bias_names)

    biases = np.zeros((128, 74), np.float32)
    biases[:, 0:8] = _chunk_vec(np.asarray(inputs["b0"], np.float32))
    biases[:, 8:16] = _chunk_vec(np.asarray(inputs["b1"], np.float32))
    biases[:, 16:24] = _chunk_vec(np.asarray(inputs["b2"], np.float32))
    bih = _chunk_vec(np.asarray(inputs["gru_b_ih"], np.float32))
    bhh = _chunk_vec(np.asarray(inputs["gru_b_hh"], np.float32))
    # r/z gates consume bih+bhh as one folded bias (cols 24..39); the n gate
    # needs them apart: n(ih) at 40..47, n(hh) at 64..71 (within bhh 48..71)
    biases[:, 24:40] = (bih + bhh)[:, 0:16]
    biases[:, 40:48] = bih[:, 16:24]
    biases[:, 48:72] = bhh
    biases[:, 72] = np.asarray(inputs["mu_b"], np.float32)
    biases[:, 73] = np.asarray(inputs["lv_b"], np.float32)

    mwdt = fp8 if MLP_W_FP8 else bf16
    shared = {
        "w0t": _chunk_wT(np.asarray(inputs["w0"], np.float32), mwdt),
        "w1t": _chunk_wT(np.asarray(inputs["w1"], np.float32), mwdt),
        "w2t": _chunk_wT(np.asarray(inputs["w2"], np.float32), mwdt),
        "whht": _chunk_wT(np.asarray(inputs["gru_w_hh"], np.float32)),
        "wiht": _chunk_wT(np.asarray(inputs["gru_w_ih"], np.float32)),
        "muwt": _chunk_wT(np.asarray(inputs["mu_w"], np.float32)),
        "lvwt": _chunk_wT(np.asarray(inputs["lv_w"], np.float32)),
        "coefs": coefs_full,
        "biases": biases,
    }

    in_maps = []
    for cidx in range(N_CORES):
        xc = x[cidx * BL:(cidx + 1) * BL]               # [BL, S, D]
        xT = np.ascontiguousarray(np.transpose(xc, (2, 1, 0)))  # [D, S, BL]
        m = dict(shared)
        m["xT"] = xT.reshape(128, S * BL).astype(bf16)
        in_maps.append(m)
    coef_vals = [float(v) for v in coefs[0, :N_COEF]] if uniform_dt else None
    return in_maps, zero_bias, uniform_dt, coef_vals


def kernel(**inputs):
    from concourse import bass_utils

    in_maps, zero_bias, uniform_dt, coef_vals = host_prep(inputs)
    nc = _build_program(S - 1, zero_bias, uniform_dt, coef_vals)
    _patch_to_json(nc)
    res = bass_utils.run_bass_kernel_spmd(
        nc, in_maps, core_ids=list(range(N_CORES)))
    mu = np.empty((B, L), np.float32)
    lv = np.empty((B, L), np.float32)
    for cidx in range(N_CORES):
        mu[cidx * BL:(cidx + 1) * BL] = np.asarray(
            res.results[cidx]["mu_out"], np.float32).T
        lv[cidx * BL:(cidx + 1) * BL] = np.asarray(
            res.results[cidx]["lv_out"], np.float32).T
    return mu, lv



# revision 49
# speedup vs baseline: 1.2411x; 1.0412x over previous
"""ODE-GRU encoder Trainium2 Bass kernel.

Model (per reference): B=512, S=128, D=128, H=1024, L=128.
  h = GRUCell(x_0, 0)
  for i in 1..S-1:  4x dopri5 substeps on h' = MLP(h), then h = GRUCell(x_i, h)
  mu = h @ mu_w.T + mu_b ; logvar = h @ lv_w.T + lv_b

Key structural facts exploited:
  * DP_B == DP_A[6] (FSAL): the dopri5 solution point y_6 IS h_new, and the
    7th stage evaluation k_6 is dead code in the reference. So each substep
    needs only 6 MLP evals + the y_6 linear combination.
  * Pure data parallelism: batch 512 -> 8 cores x 64. No collectives.

Device layout (per core, "transposed chunked" form):
  A length-1024 vector per batch element lives as an SBUF tile [128, 8*64]:
  column block c (64 wide) = hidden chunk c, partition p = hidden c*128+p,
  column-within-block j = batch element j.
  Matmul out[m-chunk] = sum_k W.T[k,m].T @ act[k] : lhsT = weight tile
  [128(k), 128(m)] (bf16, resident in SBUF), rhs = activation chunk [128, 64]
  (bf16), PSUM out [128(m), 64] fp32, 8-chunk accumulation per output chunk.
  This chains layers with zero transposes.

Precision: weights bf16, matmul inputs bf16, PSUM accum fp32, all state
(h, k_j, y) fp32 on DVE, tanh/sigmoid on ACT (fp32 in, bf16 out mid-MLP).
Measured end-to-end error vs fp64 in simulation: ~3.4e-3 absmax relative.

dopri5 coefficients (hs * a_sj) are data-dependent (from t): they are loaded
per interval from a small DRAM table as per-partition scalars, so any t works.
"""
import sys
import os
from contextlib import ExitStack

sys.path.insert(0, "/opt/trn_rl_repo")

import numpy as np
import ml_dtypes

B, S, D, H, L = 512, 128, 128, 1024, 128
N_SUB = 4
N_CORES = 8
BL = B // N_CORES  # 64 batch per core
C = H // 128       # 8 hidden chunks

DP_A = (
    (),
    (1/5,),
    (3/40, 9/40),
    (44/45, -56/15, 32/9),
    (19372/6561, -25360/2187, 64448/6561, -212/729),
    (9017/3168, -355/33, 46732/5247, 49/176, -5103/18656),
    (35/384, 0.0, 500/1113, 125/192, -2187/6784, 11/84),
)

NZ_ROWS = [[j for j, a in enumerate(row) if a != 0.0] for row in DP_A]
N_COEF = sum(len(nz) for nz in NZ_ROWS[1:])  # 20
COEF_COLS = 32  # padded

bf16 = ml_dtypes.bfloat16
fp8 = ml_dtypes.float8_e4m3fn  # TRN FP8_EXP4: bit-compatible within +-240

# MLP weights in fp8e4m3 (moving operands stay bf16): halves the LDWEIGHTS
# SBUF read traffic feeding the power governor; numerically safe (measured:
# final rel err unchanged at ~3.5e-3 — the hs=dt/4 factor damps MLP error).
# GRU weights must stay bf16 (fp8 there measured 2.9e-2 > budget).
MLP_W_FP8 = True


def _split_multiwaits(bir_bytes):
    """Rewrite sync_info patterns the TPB 64B encoding can't hold:

    1. >1 sem waits on one instruction (e.g. the Tile For_i back-edge Drain)
       -> all but the last wait move to prepended single-wait NoOps.
    2. a wait together with a `sem-add-imm` update (staggered-reset prebumps
       aggregate bumps into big adds; wait_value and update_value share the
       one `semaphore_value` field) -> all waits move to prepended NoOps.

    Hoisting a wait to a preceding NoOp on the same engine is semantics-
    preserving (engine streams are FIFO). DMA opcodes are left alone.
    """
    import orjson
    j = orjson.loads(bir_bytes)
    ctr = 0
    for fn in j["functions"]:
        for blk in fn["blocks"]:
            out = []
            for ins in blk["instructions"]:
                si = ins.get("sync_info")
                waits = (si or {}).get("on_wait") or []
                updates = (si or {}).get("on_update") or []
                is_dma = ins.get("opcode", "").startswith("DMA")
                clash = (waits and not is_dma and any(
                    u.get("update_mode") == "sem-add-imm" and
                    u.get("update_value", 0) > 1 for u in updates))
                hoist = waits if clash else (
                    waits[:-1] if len(waits) > 1 else [])
                if hoist:
                    for w in hoist:
                        ctr += 1
                        nop = {
                            "engine": ins["engine"],
                            "ins": [],
                            "outs": [],
                            "name": f"waitsplit-{ctr}",
                            "opcode": "NoOp",
                            "sync_info": {"on_update": [], "on_wait": [w]},
                        }
                        if "debug" in ins:
                            nop["debug"] = ins["debug"]
                        out.append(nop)
                    si["on_wait"] = waits[len(hoist):]
                out.append(ins)
            blk["instructions"] = out
    return orjson.dumps(j)


def _patch_to_json(nc):
    from concourse import mybir
    nc.to_json_bytes = lambda: _split_multiwaits(
        mybir.module_to_json_bytes(nc.m))


def _build_program(n_intervals, zero_bias, uniform_dt=False, coef_vals=None):
    import concourse.bass as bass
    import concourse.tile as tile
    from concourse import mybir

    f32 = mybir.dt.float32
    bf = mybir.dt.bfloat16
    wdt = mybir.dt.float8e4 if MLP_W_FP8 else bf
    Tanh = mybir.ActivationFunctionType.Tanh
    Sigmoid = mybir.ActivationFunctionType.Sigmoid
    Ident = mybir.ActivationFunctionType.Identity
    AO = mybir.AluOpType

    NI = n_intervals

    nc = bass.Bass(trn_type="TRN2", target_bir_lowering=False, debug=False)

    w0t_d = nc.dram_tensor("w0t", [128, 64 * 128], wdt, kind="ExternalInput")
    w1t_d = nc.dram_tensor("w1t", [128, 64 * 128], wdt, kind="ExternalInput")
    w2t_d = nc.dram_tensor("w2t", [128, 64 * 128], wdt, kind="ExternalInput")
    whht_d = nc.dram_tensor("whht", [128, 192 * 128], bf, kind="ExternalInput")
    wiht_d = nc.dram_tensor("wiht", [128, 24 * 128], bf, kind="ExternalInput")
    muwt_d = nc.dram_tensor("muwt", [128, 8 * 128], bf, kind="ExternalInput")
    lvwt_d = nc.dram_tensor("lvwt", [128, 8 * 128], bf, kind="ExternalInput")
    # x resident in SBUF for the whole run: [p=din, (step, batch)] layout,
    # loaded once — no per-interval DMA, and the GRU input-side matmuls can
    # run at the top of the loop body to fill the h-carry dependency stall.
    xT_d = nc.dram_tensor("xT", [128, (NI + 1) * BL], bf,
                          kind="ExternalInput")
    # uniform dt (the harness case: t = arange*0.1): one static coef tile;
    # otherwise a per-interval table DMA'd inside the loop.
    coefs_d = nc.dram_tensor(
        "coefs", [128, COEF_COLS] if uniform_dt
        else [max(NI, 1) * 128, COEF_COLS], f32, kind="ExternalInput")
    # bias pack (fp32): cols 0..7 b0, 8..15 b1, 16..23 b2, 24..47 bih (r,z,n),
    # 48..71 bhh (r,z,n), 72 mu_b, 73 lv_b   (chunked per partition)
    bias_d = nc.dram_tensor("biases", [128, 74], f32, kind="ExternalInput")
    mu_out_d = nc.dram_tensor("mu_out", [128, BL], f32, kind="ExternalOutput")
    lv_out_d = nc.dram_tensor("lv_out", [128, BL], f32, kind="ExternalOutput")

    with ExitStack() as ctx:
        tc = ctx.enter_context(tile.TileContext(nc))
        wpool = ctx.enter_context(tc.tile_pool(name="weights", bufs=1))
        state = ctx.enter_context(tc.tile_pool(name="state", bufs=1))
        dyn = ctx.enter_context(tc.tile_pool(name="dyn", bufs=2))
        mid = ctx.enter_context(tc.tile_pool(name="mid", bufs=3))
        ypool = ctx.enter_context(tc.tile_pool(name="ypool", bufs=2))
        gpool = ctx.enter_context(tc.tile_pool(name="gru", bufs=2))
        pmlp = ctx.enter_context(tc.tile_pool(name="pmlp", bufs=4, space="PSUM"))
        pgru = ctx.enter_context(tc.tile_pool(name="pgru", bufs=1, space="PSUM"))

        w0 = wpool.tile([128, 64 * 128], wdt, tag="w0")
        w1 = wpool.tile([128, 64 * 128], wdt, tag="w1")
        w2 = wpool.tile([128, 64 * 128], wdt, tag="w2")
        whh = wpool.tile([128, 192 * 128], bf, tag="whh")
        wih = wpool.tile([128, 24 * 128], bf, tag="wih")
        muw = wpool.tile([128, 8 * 128], bf, tag="muw")
        lvw = wpool.tile([128, 8 * 128], bf, tag="lvw")
        biases = wpool.tile([128, 74], f32, tag="biases")
        xsb = wpool.tile([128, (NI + 1) * BL], bf, tag="xsb")
        loads = [(w0, w0t_d), (w1, w1t_d), (w2, w2t_d), (whh, whht_d),
                 (wih, wiht_d), (muw, muwt_d), (lvw, lvwt_d),
                 (biases, bias_d), (xsb, xT_d)]
        if uniform_dt:
            coefs_sb = wpool.tile([128, COEF_COLS], f32, tag="coefs_sb")
            loads.append((coefs_sb, coefs_d))
        for sb, dr in loads:
            nc.sync.dma_start(sb[:, :], dr[:, :])

        h = state.tile([128, C * BL], f32, tag="h")
        h_bf = state.tile([128, C * BL], bf, tag="h_bf")
        dummy_in = state.tile([128, 1], f32, tag="dummy_in")
        dummy_out = state.tile([128, 1], bf, tag="dummy_out")
        nc.vector.memset(dummy_in[:, :], 0.0)
        n_arch = 5 if zero_bias else 6
        karch = [state.tile([128, C * BL], f32, tag=f"k{j}", name=f"karch{j}")
                 for j in range(n_arch)]

        nc.vector.memset(h[:, :], 0.0)
        nc.vector.memset(h_bf[:, :], 0.0)

        def bias_col(idx):
            return biases[:, idx:idx + 1]

        HB = C * BL // 2  # half-tile width (256)

        def mm_layer_halves(wt, rhs_bf, psA, psB, nm=C,
                            after_A=None, after_B=None):
            # MLP layer into two half-bank psum tiles: m-chunks 0..3 -> psA,
            # 4..7 -> psB (different banks: ACT consumes psA while PE writes
            # psB). k-OUTER order: the first 8 matmuls consume only rhs chunk
            # 0 (64 cols), so the PE unblocks as soon as the producer's first
            # chunk-grain op lands (producers emit y/h chunks in ascending
            # order). PSUM accumulation group is per BANK: start=True only on
            # the very first matmul into the bank (clears has_written for the
            # whole bank), stop=True on the last; per-element has_written
            # gives first-write-overwrite / then-accumulate for every m
            # region independently.
            # 3 blocks: [k0-3 x m0-7] consumes chunks 0-3 at 8-MM granularity
            # (starts right after the producer's chunk-0 op); [k4-7 x m0-3]
            # completes bank A at MM 48 so its consumers (emitted via the
            # after_A hook RIGHT HERE in program order — Tile's aggregated
            # sem bumps only resolve at dependency edges, so the hook
            # placement is what lets the consumer start before the layer
            # ends) overlap the last block; [k4-7 x m4-7] finishes bank B.
            def emit(block):
                for k, m in block:
                    ps, mo = (psA, m) if m < 4 else (psB, m - 4)
                    t = (k * nm + m) * 128
                    nc.tensor.matmul(
                        ps[:, BL * mo: BL * mo + BL],
                        wt[:, t: t + 128],
                        rhs_bf[:, BL * k: BL * k + BL],
                        start=(k == 0 and mo == 0),
                        stop=(k == C - 1 and mo == 3),
                        skip_group_check=True,
                    )
            emit([(k, m) for k in range(4) for m in range(nm)])
            emit([(k, m) for k in range(4, C) for m in range(min(4, nm))])
            if after_A is not None:
                after_A()
            emit([(k, m) for k in range(4, C) for m in range(4, nm)])
            if after_B is not None:
                after_B()

        def act_half(out, ps, hb, func, bias_base):
            # out[:, hb half] = func(ps + b)
            if zero_bias:
                nc.scalar.activation(
                    out[:, hb * HB:(hb + 1) * HB], ps[:, 0:HB], func)
            else:
                for co in range(4):
                    cc = hb * 4 + co
                    nc.scalar.activation(
                        out[:, BL * cc: BL * cc + BL],
                        ps[:, BL * co: BL * co + BL],
                        func, bias=bias_col(bias_base + cc))

        def eval_mlp(rhs_bf, after_A=None, after_B=None):
            # Each psum half-tile is allocated as a FULL bank ([128, 512]
            # fp32) with only the first HB columns used, so each half's
            # consumers gate on its own bank. The mid-layer tanh halves are
            # emitted via the after-bank hooks (see mm_layer_halves); the
            # caller's hooks receive the w2 psum halves the same way.
            ps0a = pmlp.tile([128, 2 * HB], f32, tag="ps")
            ps0b = pmlp.tile([128, 2 * HB], f32, tag="ps")
            u = mid.tile([128, C * BL], bf, tag="u")
            mm_layer_halves(w0, rhs_bf, ps0a, ps0b,
                            after_A=lambda: act_half(u, ps0a, 0, Tanh, 0),
                            after_B=lambda: act_half(u, ps0b, 1, Tanh, 0))
            ps1a = pmlp.tile([128, 2 * HB], f32, tag="ps")
            ps1b = pmlp.tile([128, 2 * HB], f32, tag="ps")
            v = mid.tile([128, C * BL], bf, tag="v")
            mm_layer_halves(w1, u, ps1a, ps1b,
                            after_A=lambda: act_half(v, ps1a, 0, Tanh, 8),
                            after_B=lambda: act_half(v, ps1b, 1, Tanh, 8))
            ps2a = pmlp.tile([128, 2 * HB], f32, tag="ps")
            ps2b = pmlp.tile([128, 2 * HB], f32, tag="ps")
            mm_layer_halves(
                w2, v, ps2a, ps2b,
                after_A=(lambda: after_A(ps2a)) if after_A else None,
                after_B=(lambda: after_B(ps2b)) if after_B else None)
            return ps2a, ps2b

        def archive_half(j, ps, hb):
            # karch[j] half hb = ps + b2
            if zero_bias:
                nc.scalar.copy(karch[j][:, hb * HB:(hb + 1) * HB],
                               ps[:, 0:HB])
            else:
                for co in range(4):
                    cc = hb * 4 + co
                    nc.scalar.activation(
                        karch[j][:, BL * cc: BL * cc + BL],
                        ps[:, BL * co: BL * co + BL],
                        Ident, bias=bias_col(16 + cc))

        def stt(out, in0, cap, in1):
            # out = in0 * coef + in1 (full width, DVE)
            nc.vector.scalar_tensor_tensor(
                out[:, :], in0[:, :], cap, in1[:, :], AO.mult, AO.add)

        def stt_half(out, ps, j, cap, in1, hb, chunked=False):
            # out[half hb] = k * coef + in1[half hb], where k comes from the
            # psum bank directly (zero-bias) or the biased archive karch[j].
            # chunked: emit 64-col pieces in ascending order — the consumer's
            # first matmuls restart after one small DVE op (~280ns) instead
            # of a full half op (~430ns); the waits all clear at the same
            # aggregated PE sem bump, so only the first op is latency-
            # critical while the rest stream ahead of the PE's 8-MM-per-
            # chunk consumption pace.
            sl = slice(hb * HB, (hb + 1) * HB)
            if chunked:
                for co in range(4):
                    cc = hb * 4 + co
                    cs = slice(BL * cc, BL * cc + BL)
                    ps_cs = slice(BL * co, BL * co + BL)
                    in0 = ps[:, ps_cs] if zero_bias else karch[j][:, cs]
                    nc.vector.scalar_tensor_tensor(
                        out[:, cs], in0, cap, in1[:, cs], AO.mult, AO.add)
            else:
                in0 = ps[:, 0:HB] if zero_bias else karch[j][:, sl]
                nc.vector.scalar_tensor_tensor(
                    out[:, sl], in0, cap, in1[:, sl], AO.mult, AO.add)

        def substep(coef_tile):
            # h, h_bf updated in place. Uniform-dt: coefficients are float
            # immediates (compile-time constants) — required for the GPSIMD
            # y-accumulation path (TensorScalarPtr is not a Pool opcode) and
            # saves the per-partition scalar reads. Otherwise: per-partition
            # scalar APs at fixed cols 0..19 of the DMA'd coef tile.
            cnt = 0

            def next_coef():
                nonlocal cnt
                if coef_vals is not None:
                    cap = float(coef_vals[cnt])
                else:
                    cap = coef_tile[:, cnt:cnt + 1]
                cnt += 1
                return cap

            # eval e computes k_e; stage s=e+1 consumes it. The y_acc chain
            # for stage s (karch reads only) is emitted BEFORE eval e; the
            # final y_s = k_e*c + y_acc halves are emitted INSIDE eval e via
            # the after-bank hooks, so each half fires as soon as its w2
            # psum bank closes (16 MMs before the layer ends for bank A).
            rhs = h_bf
            for e in range(6):
                s = e + 1
                nz = NZ_ROWS[s]
                caps = [next_coef() for _ in nz]
                y_acc = None
                base = h
                for idx, j in enumerate(nz[:-1]):
                    if y_acc is None:
                        y_acc = ypool.tile([128, C * BL], f32, tag="yacc")
                    stt(y_acc, karch[j], caps[idx], base)
                    base = y_acc
                fcap = caps[-1]
                fbase = base
                y_out = None
                if s < 6:
                    y_out = mid.tile([128, C * BL], bf, tag="ybf",
                                     name=f"ybf{s}")

                def hook(ps, hb, e=e, s=s, fcap=fcap, fbase=fbase,
                         y_out=y_out):
                    if s == 6:
                        # y_6 == h_new; bf16 copy first (PE-critical)
                        stt_half(h_bf, ps, e, fcap, fbase, hb, chunked=(hb == 0))
                        if e < n_arch:
                            archive_half(e, ps, hb)
                        stt_half(h, ps, e, fcap, fbase, hb)
                    else:
                        stt_half(y_out, ps, e, fcap, fbase, hb, chunked=(hb == 0))
                        if e < n_arch:
                            archive_half(e, ps, hb)

                ks_psum = eval_mlp(rhs,
                                   after_A=lambda ps, h=hook: h(ps, 0),
                                   after_B=lambda ps, h=hook: h(ps, 1))
                rhs = y_out

        def gru_step(xt_ap):
            pr = pgru.tile([128, C * BL], f32, tag="pr")
            pz = pgru.tile([128, C * BL], f32, tag="pz")
            pgn = pgru.tile([128, C * BL], f32, tag="pgn")
            pin_ = pgru.tile([128, C * BL], f32, tag="pin")
            # gi first (needs only xt, staged at body top): opens each gate's
            # accumulation group. Then hh: [k0-3 x all gates] consumes h_bf
            # chunks 0-3 (emitted first by the final substep), then k4-7 per
            # gate ordered r, n, z: pr's bank closes ~64 MMs early so the
            # tail's r-ACT starts while the n/z matmuls still run; z (only
            # needed late in the tail chain) closes last.
            for sec, ps in ((0, pr), (1, pz), (2, pin_)):
                for m in range(C):
                    mj = (sec if sec < 2 else 2) * 8 + m
                    nc.tensor.matmul(
                        ps[:, BL * m: BL * m + BL],
                        wih[:, mj * 128: mj * 128 + 128],
                        xt_ap,
                        start=(m == 0),
                        stop=(sec == 2 and m == C - 1),
                        skip_group_check=True)
            r = gpool.tile([128, C * BL], f32, tag="r")
            z = gpool.tile([128, C * BL], f32, tag="z")
            n = gpool.tile([128, C * BL], f32, tag="n")
            t1 = gpool.tile([128, C * BL], f32, tag="t1")
            pre = gpool.tile([128, C * BL], f32, tag="pre")
            d = gpool.tile([128, C * BL], f32, tag="d")
            e = gpool.tile([128, C * BL], f32, tag="e")
            H0, H1 = slice(0, HB), slice(HB, 2 * HB)

            def emit_hh(block, stop_sec=None):
                for k, sec, m in block:
                    ps = (pr, pz, pgn)[sec]
                    mj = sec * 8 + m
                    t = (k * 24 + mj) * 128
                    nc.tensor.matmul(
                        ps[:, BL * m: BL * m + BL],
                        whh[:, t: t + 128],
                        h_bf[:, BL * k: BL * k + BL],
                        start=(sec == 2 and k == 0 and m == 0),
                        stop=(sec == stop_sec and k == C - 1 and m == C - 1),
                        skip_group_check=True)

            emit_hh([(k, sec, m) for k in range(4)
                     for sec in range(3) for m in range(C)])
            emit_hh([(k, 0, m) for k in range(4, C) for m in range(C)],
                    stop_sec=0)
            if zero_bias:
                # tail ops emitted right after the gate bank they need
                # closes (pr -> pgn -> pz): the chain starts ~2us before the
                # GRU matmuls end instead of after them.
                nc.scalar.activation(r[:, H0], pr[:, H0], Sigmoid)
                nc.scalar.activation(r[:, H1], pr[:, H1], Sigmoid)
            emit_hh([(k, 2, m) for k in range(4, C) for m in range(C)],
                    stop_sec=2)
            if zero_bias:
                nc.vector.tensor_mul(t1[:, H0], r[:, H0], pgn[:, H0])
                nc.vector.tensor_add(pre[:, H0], t1[:, H0], pin_[:, H0])
                nc.vector.tensor_mul(t1[:, H1], r[:, H1], pgn[:, H1])
                nc.vector.tensor_add(pre[:, H1], t1[:, H1], pin_[:, H1])
                nc.scalar.activation(n[:, H0], pre[:, H0], Tanh)
                nc.scalar.activation(n[:, H1], pre[:, H1], Tanh)
                nc.vector.tensor_sub(d[:, H0], h[:, H0], n[:, H0])
                nc.vector.tensor_sub(d[:, H1], h[:, H1], n[:, H1])
            emit_hh([(k, 1, m) for k in range(4, C) for m in range(C)],
                    stop_sec=1)
            if zero_bias:
                nc.scalar.activation(z[:, H0], pz[:, H0], Sigmoid)
                nc.scalar.activation(z[:, H1], pz[:, H1], Sigmoid)
                nc.vector.tensor_mul(e[:, H0], z[:, H0], d[:, H0])
                nc.vector.tensor_add(h_bf[:, H0], n[:, H0], e[:, H0])
                nc.vector.tensor_mul(e[:, H1], z[:, H1], d[:, H1])
                nc.vector.tensor_add(h_bf[:, H1], n[:, H1], e[:, H1])
                nc.vector.tensor_add(h[:, H0], n[:, H0], e[:, H0])
                nc.vector.tensor_add(h[:, H1], n[:, H1], e[:, H1])
            else:
                for cc in range(C):
                    sl = slice(BL * cc, BL * cc + BL)
                    # bias for r gate = bih_r + bhh_r (host folds the sum into
                    # col 24.. for ih and 48.. for hh; here use both adds)
                    nc.scalar.activation(r[:, sl], pr[:, sl], Sigmoid,
                                         bias=bias_col(24 + cc))
                    nc.scalar.activation(z[:, sl], pz[:, sl], Sigmoid,
                                         bias=bias_col(24 + 8 + cc))
                    # t1 = (pgn + bhh_n) * r
                    nc.vector.scalar_tensor_tensor(
                        t1[:, sl], pgn[:, sl], bias_col(48 + 16 + cc),
                        r[:, sl], AO.add, AO.mult)
                    # pre = (pin + bih_n) + t1
                    nc.vector.scalar_tensor_tensor(
                        pre[:, sl], pin_[:, sl], bias_col(24 + 16 + cc),
                        t1[:, sl], AO.add, AO.add)
                nc.scalar.activation(n[:, :], pre[:, :], Tanh)
                nc.vector.tensor_sub(d[:, :], h[:, :], n[:, :])
                nc.vector.tensor_mul(e[:, :], z[:, :], d[:, :])
                nc.vector.tensor_add(h_bf[:, :], n[:, :], e[:, :])
                nc.vector.tensor_add(h[:, :], n[:, :], e[:, :])

        # ---- prologue: h = GRU(x_0, 0) -------------------------------------
        gru_step(xsb[:, 0:BL])

        # ---- main loop over observation intervals --------------------------
        # staggered_reset: no all-engine barrier at the back edge, so the PE
        # can start iteration j+1's stage-0 matmuls while DVE/ACT finish
        # iteration j's GRU tail. Stages = substeps (stage 3 includes GRU).
        if NI > 0:
            with tc.For_i(0, NI, staggered_reset=True,
                          back_edge_label="mainloop",
                          hint_engines=(mybir.EngineType.PE,)) as j:
                # dummy activation: absorbs the per-block ACT_TABLE_LOAD off
                # the critical path (first real tanh would otherwise stall)
                nc.scalar.activation(dummy_out[:, :], dummy_in[:, :], Tanh)
                # stage x_{j+1} out of the SBUF-resident pack early (idle
                # GPSIMD; matmul operands can't take register offsets)
                xt = dyn.tile([128, BL], bf, tag="xt")
                nc.gpsimd.tensor_copy(
                    xt[:, :], xsb[:, bass.ds((j + 1) * BL, BL)])
                if uniform_dt:
                    ct = coefs_sb
                else:
                    ct = dyn.tile([128, COEF_COLS], f32, tag="ct")
                    nc.sync.dma_start(
                        ct[:, :], coefs_d[bass.ds(j * 128, 128), :])
                for si in range(N_SUB):
                    if si > 0:
                        tc.stage_boundary()
                    substep(ct)
                    if si == N_SUB - 1:
                        # arm the PE back-edge branch prefetch while the GRU
                        # matmuls run (body >> one IRAM block)
                        tc.mark_branch_hint_location(
                            "mainloop", engines=(mybir.EngineType.PE,))
                gru_step(xt)

        # ---- epilogue: mu / logvar ----------------------------------------
        for wt, bcol, out_d in ((muw, 72, mu_out_d), (lvw, 73, lv_out_d)):
            po = pgru.tile([128, BL], f32, tag="pr")
            for k in range(C):
                nc.tensor.matmul(
                    po[:, :], wt[:, k * 128: k * 128 + 128],
                    h_bf[:, BL * k: BL * k + BL],
                    start=(k == 0), stop=(k == C - 1))
            osb = gpool.tile([128, BL], f32, tag="osb")
            if zero_bias:
                nc.scalar.copy(osb[:, :], po[:, :])
            else:
                nc.scalar.activation(osb[:, :], po[:, :], Ident,
                                     bias=bias_col(bcol))
            nc.sync.dma_start(out_d[:, :], osb[:, :])

    return nc


def _chunk_wT(w, dt=bf16):
    """[O, I] weight -> [128, (I/128)*(O/128)*128] tile pack.

    Tile (k, m) at col offset (k*nm + m)*128 holds W[m*128+f, k*128+p] at
    [p, f] (i.e. lhsT = W.T block), so matmul computes W @ act.
    """
    O, I = w.shape
    nk, nm = I // 128, O // 128
    a = np.ascontiguousarray(w.T)          # [I, O]
    a = a.reshape(nk, 128, nm, 128)        # k, p, m, f
    a = np.transpose(a, (1, 0, 2, 3))      # p, k, m, f
    return np.ascontiguousarray(a.reshape(128, nk * nm * 128)).astype(dt)


def _chunk_vec(v):
    """[H] -> [128, C] chunked per-partition layout (col c = chunk c)."""
    return np.ascontiguousarray(v.reshape(-1, 128).T).astype(np.float32)


def host_prep(inputs):
    """Build the per-core in_maps + metadata from the full inputs."""
    x = np.asarray(inputs["x"], np.float32)
    t = np.asarray(inputs["t"], np.float32)

    n_intervals = S - 1
    dts = (t[0, 1:, 0] - t[0, :-1, 0]).astype(np.float32)
    hs = (dts / np.float32(N_SUB)).astype(np.float32)

    coefs = np.zeros((n_intervals, COEF_COLS), np.float32)
    for ji in range(n_intervals):
        cols = []
        for srow in range(1, 7):
            for j in NZ_ROWS[srow]:
                cols.append(np.float32(hs[ji]) * np.float32(DP_A[srow][j]))
        coefs[ji, :len(cols)] = cols
    # uniform observation spacing (the setup_inputs case: t = arange*0.1 —
    # fp32 rounding makes consecutive diffs differ in the last ulp, so use
    # allclose; the ~1e-7 relative dt perturbation is far below the error
    # scale): every interval shares one coef vector -> compile-time floats
    uniform_dt = bool(np.allclose(dts, dts[0], rtol=1e-5, atol=0.0))
    if uniform_dt:
        coefs_full = np.repeat(coefs[0:1, :], 128, axis=0)  # [128, COEF_COLS]
    else:
        coefs_full = np.repeat(coefs[:, None, :], 128, axis=1).reshape(
            n_intervals * 128, COEF_COLS)

    bias_names = ("gru_b_ih", "gru_b_hh", "b0", "b1", "b2", "mu_b", "lv_b")
    zero_bias = all(not np.any(np.asarray(inputs[k])) for k in bias_names)

    biases = np.zeros((128, 74), np.float32)
    biases[:, 0:8] = _chunk_vec(np.asarray(inputs["b0"], np.float32))
    biases[:, 8:16] = _chunk_vec(np.asarray(inputs["b1"], np.float32))
    biases[:, 16:24] = _chunk_vec(np.asarray(inputs["b2"], np.float32))
    bih = _chunk_vec(np.asarray(inputs["gru_b_ih"], np.float32))
    bhh = _chunk_vec(np.asarray(inputs["gru_b_hh"], np.float32))
    # r/z gates consume bih+bhh as one folded bias (cols 24..39); the n gate
    # needs them apart: n(ih) at 40..47, n(hh) at 64..71 (within bhh 48..71)
    biases[:, 24:40] = (bih + bhh)[:, 0:16]
    biases[:, 40:48] = bih[:, 16:24]
    biases[:, 48:72] = bhh
    biases[:, 72] = np.asarray(inputs["mu_b"], np.float32)
    biases[:, 73] = np.asarray(inputs["lv_b"], np.float32)

    mwdt = fp8 if MLP_W_FP8 else bf16
    shared = {
        "w0t": _chunk_wT(np.asarray(inputs["w0"], np.float32), mwdt),
        "w1t": _chunk_wT(np.asarray(inputs["w1"], np.float32), mwdt),
        "w2t": _chunk_wT(np.asarray(inputs["w2"], np.float32), mwdt),
        "whht": _chunk_wT(np.asarray(inputs["gru_w_hh"], np.float32)),
        "wiht": _chunk_wT(np.asarray(inputs["gru_w_ih"], np.float32)),
        "muwt": _chunk_wT(np.asarray(inputs["mu_w"], np.float32)),
        "lvwt": _chunk_wT(np.asarray(inputs["lv_w"], np.float32)),
        "coefs": coefs_full,
        "biases": biases,
    }

    in_maps = []
    for cidx in range(N_CORES):
        xc = x[cidx * BL:(cidx + 1) * BL]               # [BL, S, D]
        xT = np.ascontiguousarray(np.transpose(xc, (2, 1, 0)))  # [D, S, BL]
        m = dict(shared)
        m["xT"] = xT.reshape(128, S * BL).astype(bf16)
        in_maps.append(m)
    coef_vals = [float(v) for v in coefs[0, :N_COEF]] if uniform_dt else None
    return in_maps, zero_bias, uniform_dt, coef_vals


def kernel(**inputs):
    from concourse import bass_utils

    in_maps, zero_bias, uniform_dt, coef_vals = host_prep(inputs)
    nc = _build_program(S - 1, zero_bias, uniform_dt, coef_vals)
    _patch_to_json(nc)
    res = bass_utils.run_bass_kernel_spmd(
        nc, in_maps, core_ids=list(range(N_CORES)))
    mu = np.empty((B, L), np.float32)
    lv = np.empty((B, L), np.float32)
    for cidx in range(N_CORES):
        mu[cidx * BL:(cidx + 1) * BL] = np.asarray(
            res.results[cidx]["mu_out"], np.float32).T
        lv[cidx * BL:(cidx + 1) * BL] = np.asarray(
            res.results[cidx]["lv_out"], np.float32).T
    return mu, lv



# revision 52
# speedup vs baseline: 1.2637x; 1.0182x over previous
"""ODE-GRU encoder Trainium2 Bass kernel.

Model (per reference): B=512, S=128, D=128, H=1024, L=128.
  h = GRUCell(x_0, 0)
  for i in 1..S-1:  4x dopri5 substeps on h' = MLP(h), then h = GRUCell(x_i, h)
  mu = h @ mu_w.T + mu_b ; logvar = h @ lv_w.T + lv_b

Key structural facts exploited:
  * DP_B == DP_A[6] (FSAL): the dopri5 solution point y_6 IS h_new, and the
    7th stage evaluation k_6 is dead code in the reference. So each substep
    needs only 6 MLP evals + the y_6 linear combination.
  * Pure data parallelism: batch 512 -> 8 cores x 64. No collectives.

Device layout (per core, "transposed chunked" form):
  A length-1024 vector per batch element lives as an SBUF tile [128, 8*64]:
  column block c (64 wide) = hidden chunk c, partition p = hidden c*128+p,
  column-within-block j = batch element j.
  Matmul out[m-chunk] = sum_k W.T[k,m].T @ act[k] : lhsT = weight tile
  [128(k), 128(m)] (bf16, resident in SBUF), rhs = activation chunk [128, 64]
  (bf16), PSUM out [128(m), 64] fp32, 8-chunk accumulation per output chunk.
  This chains layers with zero transposes.

Precision: MLP weights fp8e4m3 (2x faster FWL weight loads; error unchanged
because the hs=dt/4 factor damps MLP matmul error), GRU weights bf16 (fp8
there breaks the 2e-2 budget), moving operands bf16, PSUM accum fp32, state
(h, k_j, y) fp32 on DVE, tanh/sigmoid on ACT. Measured end-to-end error vs
fp64: ~3.7e-3 absmax relative (budget 2e-2).

Scheduling notes (from HW traces): per-MM cost is ~34ns (64-cycle moving
stream + dispatch + LDWEIGHTS residual) — PE busy ~20.4ms is the floor for
this decomposition. The rest is dependency-stall engineering: psum halves
in separate banks, consumer ops emitted right after the producing bank's
stop matmul, the final y/h combine chunked 64-cols-first so the next eval's
k-outer matmuls restart one small DVE op after the layer's sem bump, and
GRU gate banks closed in r->n->z order with the tail chain interleaved.

dopri5 coefficients (hs * a_sj): uniform-dt inputs (the harness case) bake
them in as compile-time immediates; non-uniform t falls back to a DMA'd
per-interval table of per-partition scalars, so any t works.
"""
import sys
import os
from contextlib import ExitStack

sys.path.insert(0, "/opt/trn_rl_repo")

import numpy as np
import ml_dtypes

B, S, D, H, L = 512, 128, 128, 1024, 128
N_SUB = 4
N_CORES = 8
BL = B // N_CORES  # 64 batch per core
C = H // 128       # 8 hidden chunks

DP_A = (
    (),
    (1/5,),
    (3/40, 9/40),
    (44/45, -56/15, 32/9),
    (19372/6561, -25360/2187, 64448/6561, -212/729),
    (9017/3168, -355/33, 46732/5247, 49/176, -5103/18656),
    (35/384, 0.0, 500/1113, 125/192, -2187/6784, 11/84),
)

NZ_ROWS = [[j for j, a in enumerate(row) if a != 0.0] for row in DP_A]
N_COEF = sum(len(nz) for nz in NZ_ROWS[1:])  # 20
COEF_COLS = 32  # padded

bf16 = ml_dtypes.bfloat16
fp8 = ml_dtypes.float8_e4m3fn  # TRN FP8_EXP4: bit-compatible within +-240

# MLP weights in fp8e4m3 (moving operands stay bf16): halves the LDWEIGHTS
# SBUF read traffic feeding the power governor; numerically safe (measured:
# final rel err unchanged at ~3.5e-3 — the hs=dt/4 factor damps MLP error).
# GRU weights must stay bf16 (fp8 there measured 2.9e-2 > budget).
MLP_W_FP8 = True


def _split_multiwaits(bir_bytes):
    """Rewrite sync_info patterns the TPB 64B encoding can't hold:

    1. >1 sem waits on one instruction (e.g. the Tile For_i back-edge Drain)
       -> all but the last wait move to prepended single-wait NoOps.
    2. a wait together with a `sem-add-imm` update (staggered-reset prebumps
       aggregate bumps into big adds; wait_value and update_value share the
       one `semaphore_value` field) -> all waits move to prepended NoOps.

    Hoisting a wait to a preceding NoOp on the same engine is semantics-
    preserving (engine streams are FIFO). DMA opcodes are left alone.
    """
    import orjson
    j = orjson.loads(bir_bytes)
    ctr = 0
    for fn in j["functions"]:
        for blk in fn["blocks"]:
            out = []
            for ins in blk["instructions"]:
                si = ins.get("sync_info")
                waits = (si or {}).get("on_wait") or []
                updates = (si or {}).get("on_update") or []
                is_dma = ins.get("opcode", "").startswith("DMA")
                clash = (waits and not is_dma and any(
                    u.get("update_mode") == "sem-add-imm" and
                    u.get("update_value", 0) > 1 for u in updates))
                hoist = waits if clash else (
                    waits[:-1] if len(waits) > 1 else [])
                if hoist:
                    for w in hoist:
                        ctr += 1
                        nop = {
                            "engine": ins["engine"],
                            "ins": [],
                            "outs": [],
                            "name": f"waitsplit-{ctr}",
                            "opcode": "NoOp",
                            "sync_info": {"on_update": [], "on_wait": [w]},
                        }
                        if "debug" in ins:
                            nop["debug"] = ins["debug"]
                        out.append(nop)
                    si["on_wait"] = waits[len(hoist):]
                out.append(ins)
            blk["instructions"] = out
    return orjson.dumps(j)


def _patch_to_json(nc):
    from concourse import mybir
    nc.to_json_bytes = lambda: _split_multiwaits(
        mybir.module_to_json_bytes(nc.m))


def _build_program(n_intervals, zero_bias, uniform_dt=False, coef_vals=None):
    import concourse.bass as bass
    import concourse.tile as tile
    from concourse import mybir

    f32 = mybir.dt.float32
    bf = mybir.dt.bfloat16
    wdt = mybir.dt.float8e4 if MLP_W_FP8 else bf
    Tanh = mybir.ActivationFunctionType.Tanh
    Sigmoid = mybir.ActivationFunctionType.Sigmoid
    Ident = mybir.ActivationFunctionType.Identity
    AO = mybir.AluOpType

    NI = n_intervals

    nc = bass.Bass(trn_type="TRN2", target_bir_lowering=False, debug=False)

    w0t_d = nc.dram_tensor("w0t", [128, 64 * 128], wdt, kind="ExternalInput")
    w1t_d = nc.dram_tensor("w1t", [128, 64 * 128], wdt, kind="ExternalInput")
    w2t_d = nc.dram_tensor("w2t", [128, 64 * 128], wdt, kind="ExternalInput")
    whht_d = nc.dram_tensor("whht", [128, 192 * 128], bf, kind="ExternalInput")
    wiht_d = nc.dram_tensor("wiht", [128, 24 * 128], bf, kind="ExternalInput")
    muwt_d = nc.dram_tensor("muwt", [128, 8 * 128], bf, kind="ExternalInput")
    lvwt_d = nc.dram_tensor("lvwt", [128, 8 * 128], bf, kind="ExternalInput")
    # x resident in SBUF for the whole run: [p=din, (step, batch)] layout,
    # loaded once — no per-interval DMA, and the GRU input-side matmuls can
    # run at the top of the loop body to fill the h-carry dependency stall.
    xT_d = nc.dram_tensor("xT", [128, (NI + 1) * BL], bf,
                          kind="ExternalInput")
    # uniform dt (the harness case: t = arange*0.1): one static coef tile;
    # otherwise a per-interval table DMA'd inside the loop.
    coefs_d = nc.dram_tensor(
        "coefs", [128, COEF_COLS] if uniform_dt
        else [max(NI, 1) * 128, COEF_COLS], f32, kind="ExternalInput")
    # bias pack (fp32): cols 0..7 b0, 8..15 b1, 16..23 b2, 24..47 bih (r,z,n),
    # 48..71 bhh (r,z,n), 72 mu_b, 73 lv_b   (chunked per partition)
    bias_d = nc.dram_tensor("biases", [128, 74], f32, kind="ExternalInput")
    mu_out_d = nc.dram_tensor("mu_out", [128, BL], f32, kind="ExternalOutput")
    lv_out_d = nc.dram_tensor("lv_out", [128, BL], f32, kind="ExternalOutput")

    with ExitStack() as ctx:
        tc = ctx.enter_context(tile.TileContext(nc))
        wpool = ctx.enter_context(tc.tile_pool(name="weights", bufs=1))
        state = ctx.enter_context(tc.tile_pool(name="state", bufs=1))
        dyn = ctx.enter_context(tc.tile_pool(name="dyn", bufs=2))
        mid = ctx.enter_context(tc.tile_pool(name="mid", bufs=3))
        ypool = ctx.enter_context(tc.tile_pool(name="ypool", bufs=2))
        gpool = ctx.enter_context(tc.tile_pool(name="gru", bufs=2))
        pmlp = ctx.enter_context(tc.tile_pool(name="pmlp", bufs=4, space="PSUM"))
        pgru = ctx.enter_context(tc.tile_pool(name="pgru", bufs=1, space="PSUM"))

        w0 = wpool.tile([128, 64 * 128], wdt, tag="w0")
        w1 = wpool.tile([128, 64 * 128], wdt, tag="w1")
        w2 = wpool.tile([128, 64 * 128], wdt, tag="w2")
        whh = wpool.tile([128, 192 * 128], bf, tag="whh")
        wih = wpool.tile([128, 24 * 128], bf, tag="wih")
        muw = wpool.tile([128, 8 * 128], bf, tag="muw")
        lvw = wpool.tile([128, 8 * 128], bf, tag="lvw")
        biases = wpool.tile([128, 74], f32, tag="biases")
        xsb = wpool.tile([128, (NI + 1) * BL], bf, tag="xsb")
        loads = [(w0, w0t_d), (w1, w1t_d), (w2, w2t_d), (whh, whht_d),
                 (wih, wiht_d), (muw, muwt_d), (lvw, lvwt_d),
                 (biases, bias_d), (xsb, xT_d)]
        if uniform_dt:
            coefs_sb = wpool.tile([128, COEF_COLS], f32, tag="coefs_sb")
            loads.append((coefs_sb, coefs_d))
        for sb, dr in loads:
            nc.sync.dma_start(sb[:, :], dr[:, :])

        h = state.tile([128, C * BL], f32, tag="h")
        h_bf = state.tile([128, C * BL], bf, tag="h_bf")
        dummy_in = state.tile([128, 1], f32, tag="dummy_in")
        dummy_out = state.tile([128, 1], bf, tag="dummy_out")
        nc.vector.memset(dummy_in[:, :], 0.0)
        n_arch = 5 if zero_bias else 6
        karch = [state.tile([128, C * BL], f32, tag=f"k{j}", name=f"karch{j}")
                 for j in range(n_arch)]

        nc.vector.memset(h[:, :], 0.0)
        nc.vector.memset(h_bf[:, :], 0.0)

        def bias_col(idx):
            return biases[:, idx:idx + 1]

        HB = C * BL // 2  # half-tile width (256)

        def mm_layer_halves(wt, rhs_bf, psA, psB, nm=C,
                            after_A=None, after_B=None):
            # MLP layer into two half-bank psum tiles: m-chunks 0..3 -> psA,
            # 4..7 -> psB (different banks: ACT consumes psA while PE writes
            # psB). k-OUTER order: the first 8 matmuls consume only rhs chunk
            # 0 (64 cols), so the PE unblocks as soon as the producer's first
            # chunk-grain op lands (producers emit y/h chunks in ascending
            # order). PSUM accumulation group is per BANK: start=True only on
            # the very first matmul into the bank (clears has_written for the
            # whole bank), stop=True on the last; per-element has_written
            # gives first-write-overwrite / then-accumulate for every m
            # region independently.
            # 3 blocks: [k0-3 x m0-7] consumes chunks 0-3 at 8-MM granularity
            # (starts right after the producer's chunk-0 op); [k4-7 x m0-3]
            # completes bank A at MM 48 so its consumers (emitted via the
            # after_A hook RIGHT HERE in program order — Tile's aggregated
            # sem bumps only resolve at dependency edges, so the hook
            # placement is what lets the consumer start before the layer
            # ends) overlap the last block; [k4-7 x m4-7] finishes bank B.
            def emit(block):
                for k, m in block:
                    ps, mo = (psA, m) if m < 4 else (psB, m - 4)
                    t = (k * nm + m) * 128
                    nc.tensor.matmul(
                        ps[:, BL * mo: BL * mo + BL],
                        wt[:, t: t + 128],
                        rhs_bf[:, BL * k: BL * k + BL],
                        start=(k == 0 and mo == 0),
                        stop=(k == C - 1 and mo == 3),
                        skip_group_check=True,
                    )
            emit([(k, m) for k in range(4) for m in range(nm)])
            emit([(k, m) for k in range(4, C) for m in range(min(4, nm))])
            if after_A is not None:
                after_A()
            emit([(k, m) for k in range(4, C) for m in range(4, nm)])
            if after_B is not None:
                after_B()

        def act_half(out, ps, hb, func, bias_base):
            # out[:, hb half] = func(ps + b)
            if zero_bias:
                nc.scalar.activation(
                    out[:, hb * HB:(hb + 1) * HB], ps[:, 0:HB], func)
            else:
                for co in range(4):
                    cc = hb * 4 + co
                    nc.scalar.activation(
                        out[:, BL * cc: BL * cc + BL],
                        ps[:, BL * co: BL * co + BL],
                        func, bias=bias_col(bias_base + cc))

        def eval_mlp(rhs_bf, after_A=None, after_B=None):
            # Each psum half-tile is allocated as a FULL bank ([128, 512]
            # fp32) with only the first HB columns used, so each half's
            # consumers gate on its own bank. The mid-layer tanh halves are
            # emitted via the after-bank hooks (see mm_layer_halves); the
            # caller's hooks receive the w2 psum halves the same way.
            ps0a = pmlp.tile([128, 2 * HB], f32, tag="ps")
            ps0b = pmlp.tile([128, 2 * HB], f32, tag="ps")
            u = mid.tile([128, C * BL], bf, tag="u")
            mm_layer_halves(w0, rhs_bf, ps0a, ps0b,
                            after_A=lambda: act_half(u, ps0a, 0, Tanh, 0),
                            after_B=lambda: act_half(u, ps0b, 1, Tanh, 0))
            ps1a = pmlp.tile([128, 2 * HB], f32, tag="ps")
            ps1b = pmlp.tile([128, 2 * HB], f32, tag="ps")
            v = mid.tile([128, C * BL], bf, tag="v")
            mm_layer_halves(w1, u, ps1a, ps1b,
                            after_A=lambda: act_half(v, ps1a, 0, Tanh, 8),
                            after_B=lambda: act_half(v, ps1b, 1, Tanh, 8))
            ps2a = pmlp.tile([128, 2 * HB], f32, tag="ps")
            ps2b = pmlp.tile([128, 2 * HB], f32, tag="ps")
            mm_layer_halves(
                w2, v, ps2a, ps2b,
                after_A=(lambda: after_A(ps2a)) if after_A else None,
                after_B=(lambda: after_B(ps2b)) if after_B else None)
            return ps2a, ps2b

        def archive_half(j, ps, hb):
            # karch[j] half hb = ps + b2
            if zero_bias:
                nc.scalar.copy(karch[j][:, hb * HB:(hb + 1) * HB],
                               ps[:, 0:HB])
            else:
                for co in range(4):
                    cc = hb * 4 + co
                    nc.scalar.activation(
                        karch[j][:, BL * cc: BL * cc + BL],
                        ps[:, BL * co: BL * co + BL],
                        Ident, bias=bias_col(16 + cc))

        def stt(out, in0, cap, in1):
            # out = in0 * coef + in1 (full width, DVE)
            nc.vector.scalar_tensor_tensor(
                out[:, :], in0[:, :], cap, in1[:, :], AO.mult, AO.add)

        def stt_half(out, ps, j, cap, in1, hb, chunked=False):
            # out[half hb] = k * coef + in1[half hb], where k comes from the
            # psum bank directly (zero-bias) or the biased archive karch[j].
            # chunked: emit 64-col pieces in ascending order — the consumer's
            # first matmuls restart after one small DVE op (~280ns) instead
            # of a full half op (~430ns); the waits all clear at the same
            # aggregated PE sem bump, so only the first op is latency-
            # critical while the rest stream ahead of the PE's 8-MM-per-
            # chunk consumption pace.
            sl = slice(hb * HB, (hb + 1) * HB)
            if chunked:
                for co in range(4):
                    cc = hb * 4 + co
                    cs = slice(BL * cc, BL * cc + BL)
                    ps_cs = slice(BL * co, BL * co + BL)
                    in0 = ps[:, ps_cs] if zero_bias else karch[j][:, cs]
                    nc.vector.scalar_tensor_tensor(
                        out[:, cs], in0, cap, in1[:, cs], AO.mult, AO.add)
            else:
                in0 = ps[:, 0:HB] if zero_bias else karch[j][:, sl]
                nc.vector.scalar_tensor_tensor(
                    out[:, sl], in0, cap, in1[:, sl], AO.mult, AO.add)

        def substep(coef_tile):
            # h, h_bf updated in place. Uniform-dt: coefficients are float
            # immediates (compile-time constants) — required for the GPSIMD
            # y-accumulation path (TensorScalarPtr is not a Pool opcode) and
            # saves the per-partition scalar reads. Otherwise: per-partition
            # scalar APs at fixed cols 0..19 of the DMA'd coef tile.
            cnt = 0

            def next_coef():
                nonlocal cnt
                if coef_vals is not None:
                    cap = float(coef_vals[cnt])
                else:
                    cap = coef_tile[:, cnt:cnt + 1]
                cnt += 1
                return cap

            # eval e computes k_e; stage s=e+1 consumes it. The y_acc chain
            # for stage s (karch reads only) is emitted BEFORE eval e; the
            # final y_s = k_e*c + y_acc halves are emitted INSIDE eval e via
            # the after-bank hooks, so each half fires as soon as its w2
            # psum bank closes (16 MMs before the layer ends for bank A).
            rhs = h_bf
            for e in range(6):
                s = e + 1
                nz = NZ_ROWS[s]
                caps = [next_coef() for _ in nz]
                y_acc = None
                base = h
                for idx, j in enumerate(nz[:-1]):
                    if y_acc is None:
                        y_acc = ypool.tile([128, C * BL], f32, tag="yacc")
                    stt(y_acc, karch[j], caps[idx], base)
                    base = y_acc
                fcap = caps[-1]
                fbase = base
                y_out = None
                if s < 6:
                    y_out = mid.tile([128, C * BL], bf, tag="ybf",
                                     name=f"ybf{s}")

                def hook(ps, hb, e=e, s=s, fcap=fcap, fbase=fbase,
                         y_out=y_out):
                    if s == 6:
                        # y_6 == h_new; bf16 copy first (PE-critical)
                        stt_half(h_bf, ps, e, fcap, fbase, hb, chunked=(hb == 0))
                        if e < n_arch:
                            archive_half(e, ps, hb)
                        stt_half(h, ps, e, fcap, fbase, hb)
                    else:
                        stt_half(y_out, ps, e, fcap, fbase, hb, chunked=(hb == 0))
                        if e < n_arch:
                            archive_half(e, ps, hb)

                ks_psum = eval_mlp(rhs,
                                   after_A=lambda ps, h=hook: h(ps, 0),
                                   after_B=lambda ps, h=hook: h(ps, 1))
                rhs = y_out

        def gru_step(xt_ap):
            pr = pgru.tile([128, C * BL], f32, tag="pr")
            pz = pgru.tile([128, C * BL], f32, tag="pz")
            pgn = pgru.tile([128, C * BL], f32, tag="pgn")
            pin_ = pgru.tile([128, C * BL], f32, tag="pin")
            # gi first (needs only xt, staged at body top): opens each gate's
            # accumulation group. Then hh: [k0-3 x all gates] consumes h_bf
            # chunks 0-3 (emitted first by the final substep), then k4-7 per
            # gate ordered r, n, z: pr's bank closes ~64 MMs early so the
            # tail's r-ACT starts while the n/z matmuls still run; z (only
            # needed late in the tail chain) closes last.
            for sec, ps in ((0, pr), (1, pz), (2, pin_)):
                for m in range(C):
                    mj = (sec if sec < 2 else 2) * 8 + m
                    nc.tensor.matmul(
                        ps[:, BL * m: BL * m + BL],
                        wih[:, mj * 128: mj * 128 + 128],
                        xt_ap,
                        start=(m == 0),
                        stop=(sec == 2 and m == C - 1),
                        skip_group_check=True)
            r = gpool.tile([128, C * BL], f32, tag="r")
            z = gpool.tile([128, C * BL], f32, tag="z")
            n = gpool.tile([128, C * BL], f32, tag="n")
            t1 = gpool.tile([128, C * BL], f32, tag="t1")
            pre = gpool.tile([128, C * BL], f32, tag="pre")
            d = gpool.tile([128, C * BL], f32, tag="d")
            e = gpool.tile([128, C * BL], f32, tag="e")
            H0, H1 = slice(0, HB), slice(HB, 2 * HB)

            def emit_hh(block, stop_sec=None):
                for k, sec, m in block:
                    ps = (pr, pz, pgn)[sec]
                    mj = sec * 8 + m
                    t = (k * 24 + mj) * 128
                    nc.tensor.matmul(
                        ps[:, BL * m: BL * m + BL],
                        whh[:, t: t + 128],
                        h_bf[:, BL * k: BL * k + BL],
                        start=(sec == 2 and k == 0 and m == 0),
                        stop=(sec == stop_sec and k == C - 1 and m == C - 1),
                        skip_group_check=True)

            emit_hh([(k, sec, m) for k in range(4)
                     for sec in range(3) for m in range(C)])
            emit_hh([(k, 0, m) for k in range(4, C) for m in range(C)],
                    stop_sec=0)
            if zero_bias:
                # tail ops emitted right after the gate bank they need
                # closes (pr -> pgn -> pz): the chain starts ~2us before the
                # GRU matmuls end instead of after them.
                nc.scalar.activation(r[:, H0], pr[:, H0], Sigmoid)
                nc.scalar.activation(r[:, H1], pr[:, H1], Sigmoid)
            emit_hh([(k, 2, m) for k in range(4, C) for m in range(C)],
                    stop_sec=2)
            if zero_bias:
                nc.vector.tensor_mul(t1[:, H0], r[:, H0], pgn[:, H0])
                nc.vector.tensor_add(pre[:, H0], t1[:, H0], pin_[:, H0])
                nc.vector.tensor_mul(t1[:, H1], r[:, H1], pgn[:, H1])
                nc.vector.tensor_add(pre[:, H1], t1[:, H1], pin_[:, H1])
                nc.scalar.activation(n[:, H0], pre[:, H0], Tanh)
                nc.scalar.activation(n[:, H1], pre[:, H1], Tanh)
                nc.vector.tensor_sub(d[:, H0], h[:, H0], n[:, H0])
                nc.vector.tensor_sub(d[:, H1], h[:, H1], n[:, H1])
            emit_hh([(k, 1, m) for k in range(4, C) for m in range(C)],
                    stop_sec=1)
            if zero_bias:
                nc.scalar.activation(z[:, H0], pz[:, H0], Sigmoid)
                nc.scalar.activation(z[:, H1], pz[:, H1], Sigmoid)
                # chunk e/h_bf half-0: the next interval's first matmuls
                # restart ~2 small DVE ops after z instead of 2 half ops
                for cc in range(4):
                    cs = slice(BL * cc, BL * cc + BL)
                    nc.vector.tensor_mul(e[:, cs], z[:, cs], d[:, cs])
                    nc.vector.tensor_add(h_bf[:, cs], n[:, cs], e[:, cs])
                nc.vector.tensor_mul(e[:, H1], z[:, H1], d[:, H1])
                nc.vector.tensor_add(h_bf[:, H1], n[:, H1], e[:, H1])
                nc.vector.tensor_add(h[:, H0], n[:, H0], e[:, H0])
                nc.vector.tensor_add(h[:, H1], n[:, H1], e[:, H1])
            else:
                for cc in range(C):
                    sl = slice(BL * cc, BL * cc + BL)
                    # bias for r gate = bih_r + bhh_r (host folds the sum into
                    # col 24.. for ih and 48.. for hh; here use both adds)
                    nc.scalar.activation(r[:, sl], pr[:, sl], Sigmoid,
                                         bias=bias_col(24 + cc))
                    nc.scalar.activation(z[:, sl], pz[:, sl], Sigmoid,
                                         bias=bias_col(24 + 8 + cc))
                    # t1 = (pgn + bhh_n) * r
                    nc.vector.scalar_tensor_tensor(
                        t1[:, sl], pgn[:, sl], bias_col(48 + 16 + cc),
                        r[:, sl], AO.add, AO.mult)
                    # pre = (pin + bih_n) + t1
                    nc.vector.scalar_tensor_tensor(
                        pre[:, sl], pin_[:, sl], bias_col(24 + 16 + cc),
                        t1[:, sl], AO.add, AO.add)
                nc.scalar.activation(n[:, :], pre[:, :], Tanh)
                nc.vector.tensor_sub(d[:, :], h[:, :], n[:, :])
                nc.vector.tensor_mul(e[:, :], z[:, :], d[:, :])
                nc.vector.tensor_add(h_bf[:, :], n[:, :], e[:, :])
                nc.vector.tensor_add(h[:, :], n[:, :], e[:, :])

        # ---- prologue: h = GRU(x_0, 0) -------------------------------------
        gru_step(xsb[:, 0:BL])

        # ---- main loop over observation intervals --------------------------
        # staggered_reset: no all-engine barrier at the back edge, so the PE
        # can start iteration j+1's stage-0 matmuls while DVE/ACT finish
        # iteration j's GRU tail. Stages = substeps (stage 3 includes GRU).
        if NI > 0:
            with tc.For_i(0, NI, staggered_reset=True,
                          back_edge_label="mainloop",
                          hint_engines=(mybir.EngineType.PE,)) as j:
                # dummy activation: absorbs the per-block ACT_TABLE_LOAD off
                # the critical path (first real tanh would otherwise stall)
                nc.scalar.activation(dummy_out[:, :], dummy_in[:, :], Tanh)
                # stage x_{j+1} out of the SBUF-resident pack early (idle
                # GPSIMD; matmul operands can't take register offsets)
                xt = dyn.tile([128, BL], bf, tag="xt")
                nc.gpsimd.tensor_copy(
                    xt[:, :], xsb[:, bass.ds((j + 1) * BL, BL)])
                if uniform_dt:
                    ct = coefs_sb
                else:
                    ct = dyn.tile([128, COEF_COLS], f32, tag="ct")
                    nc.sync.dma_start(
                        ct[:, :], coefs_d[bass.ds(j * 128, 128), :])
                # no stage_boundary between substeps: the dataflow deps
                # already order them, and the stage-entry NOPs cost the PE
                # ~1.5us per boundary (the wait quantizes to the slowest
                # engine's stage drain)
                for si in range(N_SUB):
                    substep(ct)
                    if si == N_SUB - 1:
                        # arm the PE back-edge branch prefetch while the GRU
                        # matmuls run (body >> one IRAM block)
                        tc.mark_branch_hint_location(
                            "mainloop", engines=(mybir.EngineType.PE,))
                gru_step(xt)

        # ---- epilogue: mu / logvar ----------------------------------------
        for wt, bcol, out_d in ((muw, 72, mu_out_d), (lvw, 73, lv_out_d)):
            po = pgru.tile([128, BL], f32, tag="pr")
            for k in range(C):
                nc.tensor.matmul(
                    po[:, :], wt[:, k * 128: k * 128 + 128],
                    h_bf[:, BL * k: BL * k + BL],
                    start=(k == 0), stop=(k == C - 1))
            osb = gpool.tile([128, BL], f32, tag="osb")
            if zero_bias:
                nc.scalar.copy(osb[:, :], po[:, :])
            else:
                nc.scalar.activation(osb[:, :], po[:, :], Ident,
                                     bias=bias_col(bcol))
            nc.sync.dma_start(out_d[:, :], osb[:, :])

    return nc


def _chunk_wT(w, dt=bf16):
    """[O, I] weight -> [128, (I/128)*(O/128)*128] tile pack.

    Tile (k, m) at col offset (k*nm + m)*128 holds W[m*128+f, k*128+p] at
    [p, f] (i.e. lhsT = W.T block), so matmul computes W @ act.
    """
    O, I = w.shape
    nk, nm = I // 128, O // 128
    a = np.ascontiguousarray(w.T)          # [I, O]
    a = a.reshape(nk, 128, nm, 128)        # k, p, m, f
    a = np.transpose(a, (1, 0, 2, 3))      # p, k, m, f
    return np.ascontiguousarray(a.reshape(128, nk * nm * 128)).astype(dt)


def _chunk_vec(v):
    """[H] -> [128, C] chunked per-partition layout (col c = chunk c)."""
    return np.ascontiguousarray(v.reshape(-1, 128).T).astype(np.float32)


def host_prep(inputs):
    """Build the per-core in_maps + metadata from the full inputs."""
    x = np.asarray(inputs["x"], np.float32)
    t = np.asarray(inputs["t"], np.float32)

    n_intervals = S - 1
    dts = (t[0, 1:, 0] - t[0, :-1, 0]).astype(np.float32)
    hs = (dts / np.float32(N_SUB)).astype(np.float32)

    coefs = np.zeros((n_intervals, COEF_COLS), np.float32)
    for ji in range(n_intervals):
        cols = []
        for srow in range(1, 7):
            for j in NZ_ROWS[srow]:
                cols.append(np.float32(hs[ji]) * np.float32(DP_A[srow][j]))
        coefs[ji, :len(cols)] = cols
    # uniform observation spacing (the setup_inputs case: t = arange*0.1 —
    # fp32 rounding makes consecutive diffs differ in the last ulp, so use
    # allclose; the ~1e-7 relative dt perturbation is far below the error
    # scale): every interval shares one coef vector -> compile-time floats
    uniform_dt = bool(np.allclose(dts, dts[0], rtol=1e-5, atol=0.0))
    if uniform_dt:
        coefs_full = np.repeat(coefs[0:1, :], 128, axis=0)  # [128, COEF_COLS]
    else:
        coefs_full = np.repeat(coefs[:, None, :], 128, axis=1).reshape(
            n_intervals * 128, COEF_COLS)

    bias_names = ("gru_b_ih", "gru_b_hh", "b0", "b1", "b2", "mu_b", "lv_b")
    zero_bias = all(not np.any(np.asarray(inputs[k])) for k in bias_names)

    biases = np.zeros((128, 74), np.float32)
    biases[:, 0:8] = _chunk_vec(np.asarray(inputs["b0"], np.float32))
    biases[:, 8:16] = _chunk_vec(np.asarray(inputs["b1"], np.float32))
    biases[:, 16:24] = _chunk_vec(np.asarray(inputs["b2"], np.float32))
    bih = _chunk_vec(np.asarray(inputs["gru_b_ih"], np.float32))
    bhh = _chunk_vec(np.asarray(inputs["gru_b_hh"], np.float32))
    # r/z gates consume bih+bhh as one folded bias (cols 24..39); the n gate
    # needs them apart: n(ih) at 40..47, n(hh) at 64..71 (within bhh 48..71)
    biases[:, 24:40] = (bih + bhh)[:, 0:16]
    biases[:, 40:48] = bih[:, 16:24]
    biases[:, 48:72] = bhh
    biases[:, 72] = np.asarray(inputs["mu_b"], np.float32)
    biases[:, 73] = np.asarray(inputs["lv_b"], np.float32)

    mwdt = fp8 if MLP_W_FP8 else bf16
    shared = {
        "w0t": _chunk_wT(np.asarray(inputs["w0"], np.float32), mwdt),
        "w1t": _chunk_wT(np.asarray(inputs["w1"], np.float32), mwdt),
        "w2t": _chunk_wT(np.asarray(inputs["w2"], np.float32), mwdt),
        "whht": _chunk_wT(np.asarray(inputs["gru_w_hh"], np.float32)),
        "wiht": _chunk_wT(np.asarray(inputs["gru_w_ih"], np.float32)),
        "muwt": _chunk_wT(np.asarray(inputs["mu_w"], np.float32)),
        "lvwt": _chunk_wT(np.asarray(inputs["lv_w"], np.float32)),
        "coefs": coefs_full,
        "biases": biases,
    }

    in_maps = []
    for cidx in range(N_CORES):
        xc = x[cidx * BL:(cidx + 1) * BL]               # [BL, S, D]
        xT = np.ascontiguousarray(np.transpose(xc, (2, 1, 0)))  # [D, S, BL]
        m = dict(shared)
        m["xT"] = xT.reshape(128, S * BL).astype(bf16)
        in_maps.append(m)
    coef_vals = [float(v) for v in coefs[0, :N_COEF]] if uniform_dt else None
    return in_maps, zero_bias, uniform_dt, coef_vals


def kernel(**inputs):
    from concourse import bass_utils

    in_maps, zero_bias, uniform_dt, coef_vals = host_prep(inputs)
    nc = _build_program(S - 1, zero_bias, uniform_dt, coef_vals)
    _patch_to_json(nc)
    res = bass_utils.run_bass_kernel_spmd(
        nc, in_maps, core_ids=list(range(N_CORES)))
    mu = np.empty((B, L), np.float32)
    lv = np.empty((B, L), np.float32)
    for cidx in range(N_CORES):
        mu[cidx * BL:(cidx + 1) * BL] = np.asarray(
            res.results[cidx]["mu_out"], np.float32).T
        lv[cidx * BL:(cidx + 1) * BL] = np.asarray(
            res.results[cidx]["lv_out"], np.float32).T
    return mu, lv

